# revision 1
# baseline (speedup 1.0000x reference)
"""Trainium2 Bass kernel for nn_DecoderBlock (shape-guided RWKV decoder block).

Data-parallel over batch: B=8 samples -> 8 NeuronCores, one NEFF.

Per-core layout: channels on partitions (256ch -> 2 "ctiles" of 128), spatial
(h, w) flattened on the free dim (4096).

- LayerNorm over channels: square (ACT) -> ones-matmul partition reduction ->
  DRAM-bounce reshape -> tiny stat math -> K=1 matmul broadcast -> TT applies.
- q_shift / mask blend via shifted access patterns; per-channel mixes folded
  into Wk/Wv/Wr host-side (k = Wk@xn + (Wk*diag(1-mk))@md, md = mask*(xs-xn)).
- WKV: unstabilized linear recurrence A_t = lam*A_{t-1} + e^{k_t} v_t via the
  DVE TensorTensorScan instruction chained across rows (data0=0 resets at each
  row start); vertical orientation scans read E/EV through transposed APs.
- channel_fusion: grouped 3x3 conv = 9 shifted-AP matmuls accumulated in PSUM
  over zero-padded [c, 66, 66] inputs; BN folded into the next conv
  host-side; GELU+bias fused into the PSUM->SBUF evacuation on ACT.
- patch_expand: up-proj rows permuted host-side so pixel shuffle becomes a
  strided DMA.

Matmuls in float32r (full rate) or bf16; bulky intermediates bf16.
"""
import sys
import os

for _p in ('/opt/trn_rl_repo', '/root/.axon_site/_ro/trn_rl_repo'):
    if _p not in sys.path and os.path.isdir(_p):
        sys.path.append(_p)

import numpy as np

B, C, CS, COUT, H, W = 8, 256, 512, 128, 64, 64
S = H * W          # 4096
NCH = 8            # spatial chunks
CH = S // NCH      # 512
EPS = 1e-5

_CACHE = {}


def _build(weights, probe=False):
    const_inputs = {}
    import concourse.bass as bass
    from concourse import bacc
    import concourse.tile as tile
    import concourse.mybir as mybir
    import ml_dtypes

    F32 = mybir.dt.float32
    F32R = mybir.dt.float32r
    BF16 = mybir.dt.bfloat16
    Alu = mybir.AluOpType
    Act = mybir.ActivationFunctionType
    MM, AD, SU = Alu.mult, Alu.add, Alu.subtract

    w = weights
    f64 = lambda x: np.asarray(x, np.float64)
    bf = lambda a: np.asarray(a, dtype=ml_dtypes.bfloat16)

    # ---------------- host-side folding
    bnscale = 1.0 / np.sqrt(1.0 + EPS)
    g1p = f64(w['bn1_g']) * bnscale
    b1p = f64(w['bn1_b'])
    g2p = f64(w['bn2_g']) * bnscale
    b2p = f64(w['bn2_b'])
    g3p = (f64(w['bn3_g']) * bnscale).astype(np.float32)
    b3p = f64(w['bn3_b']).astype(np.float32)

    c2_eff = f64(w['c2_w']) * g1p[None, :]
    c2b_eff = (f64(w['c2_b']) + f64(w['c2_w']) @ b1p).astype(np.float32)
    c3_eff = f64(w['c3_w']) * g2p[None, :]
    c3b_eff = (f64(w['c3_b']) + f64(w['c3_w']) @ b2p).astype(np.float32)

    wk_x = f64(w['Wk']).T
    wk_d = (f64(w['Wk']) * (1.0 - f64(w['mix_k']))[None, :]).T
    wv_x = f64(w['Wv']).T
    wv_d = (f64(w['Wv']) * (1.0 - f64(w['mix_v']))[None, :]).T
    wr_x = f64(w['Wr']).T
    wr_d = (f64(w['Wr']) * (1.0 - f64(w['mix_r']))[None, :]).T
    wo_t = f64(w['Wo']).T
    sp_t = f64(w['sp_w']).T.astype(np.float32)

    lam = np.exp(-np.exp(f64(w['decay']))).astype(np.float32)
    lam64 = np.tile(lam[:, None], (1, 64))
    lam64[:, 0] = 0.0
    lam64 = lam64.astype(np.float32)
    eu = np.exp(f64(w['first'])).astype(np.float32)

    pidx = np.arange(512)
    old = (pidx % 128) * 4 + (pidx // 128)
    up_t = f64(w['up_w'])[old].T.astype(np.float32)             # [256, 512]
    upb_p = f64(w['up_b'])[old].astype(np.float32)

    c1w = f64(w['c1_w'])
    c1_l = np.zeros((9, 2, 256, 256), np.float32)
    for ti in range(9):
        dy, dx = ti // 3, ti % 3
        for g in range(2):
            c1_l[ti, g] = c1w[g * 256:(g + 1) * 256, :, dy, dx].T

    # per-channel vectors as columns of one [128, ncol] const
    cols, order = {}, []

    def addcol(name, vec):
        v = np.asarray(vec, np.float32).reshape(-1, 128)
        cols[name] = v
        order.append(name)

    addcol('eu', eu)
    addcol('ln1w', w['ln1_w'])
    addcol('ln1b', w['ln1_b'])
    addcol('knw', w['kn_w'])
    addcol('knb', w['kn_b'])
    addcol('ln2w', w['ln2_w'])
    addcol('ln2b', w['ln2_b'])
    addcol('g3p', g3p)
    addcol('b3p', b3p)
    addcol('spb', w['sp_b'])
    addcol('c3b', c3b_eff)
    addcol('c1b', w['c1_b'])
    addcol('c2b', c2b_eff)
    addcol('upb', upb_p)
    colidx, ncol = {}, 0
    for n in order:
        colidx[n] = ncol
        ncol += cols[n].shape[0]
    cvec_np = np.zeros((128, ncol), np.float32)
    for n in order:
        for i in range(cols[n].shape[0]):
            cvec_np[:, colidx[n] + i] = cols[n][i]

    ln1_triv = np.all(w['ln1_w'] == 1.0) and np.all(w['ln1_b'] == 0.0)
    kn_triv = np.all(w['kn_w'] == 1.0) and np.all(w['kn_b'] == 0.0)
    ln2_triv = np.all(w['ln2_w'] == 1.0) and np.all(w['ln2_b'] == 0.0)
    bn3_triv = np.all(g3p == g3p[0]) and np.all(b3p == 0.0)
    # uniform bn3 scale commutes with LN2 -> drop it entirely when trivial

    # ---------------- bass module
    nc = bacc.Bacc("TRN2", target_bir_lowering=False, debug=False, name="decblk")

    xin = nc.dram_tensor("xin", [C, S], F32, kind="ExternalInput")
    skin = nc.dram_tensor("skin", [CS, S], F32, kind="ExternalInput")
    mrow = nc.dram_tensor("mrow", [1, S], F32, kind="ExternalInput")
    yout = nc.dram_tensor("yout", [COUT, 2 * H, 2 * W], F32, kind="ExternalOutput")
    probes = {}

    def mkprobe(name, shape):
        if probe:
            probes[name] = nc.dram_tensor(name, shape, F32, kind="ExternalOutput")
        return probes.get(name)

    def it(arr, name):
        arr = np.ascontiguousarray(arr)
        import ml_dtypes as _md
        dt_ = {np.dtype(np.float32): F32, np.dtype(_md.bfloat16): BF16}[arr.dtype]
        const_inputs[name] = arr
        return nc.dram_tensor(name, list(arr.shape), dt_, kind="ExternalInput")
    d_lam = it(lam64, "lam64")
    d_cvec = it(cvec_np, "cvec")
    d_wk = [it(bf(wk_x), "wkx"), it(bf(wk_d), "wkd")]
    d_wv = [it(bf(wv_x), "wvx"), it(bf(wv_d), "wvd")]
    d_wr = [it(bf(wr_x), "wrx"), it(bf(wr_d), "wrd")]
    d_wo = it(bf(wo_t), "wo")
    d_sp = it(sp_t, "sp")
    d_up = it(up_t, "up")
    c1_r = c1_l.reshape(9, 2, 2, 128, 2, 128).transpose(1, 4, 3, 0, 2, 5)
    d_c1 = it(bf(c1_r), "c1")   # [g, mt, p, t, kt, m]
    d_c2 = it(bf(c2_eff.T), "c2")
    d_c3 = it(bf(c3_eff.T), "c3")
    red_np = np.zeros((128, 2, 2), np.float32)
    red_np[:, 0, 0] = 1.0
    red_np[:, 1, 1] = 1.0
    d_red = it(red_np, "red")
    d_redb = it(bf(red_np), "redb")
    bc2_np = np.zeros((2, 2, 128), np.float32)
    bc2_np[0, 0, :] = 1.0
    bc2_np[1, 1, :] = 1.0
    d_bc1 = it(bc2_np, "bc2")
    d_eps = it(np.full((128, 1), EPS, np.float32), "epsc")

    def scan_raw(out, d0, d1):
        eng = nc.vector
        if os.environ.get('BASSK_NOSCAN'):
            return eng.tensor_copy(out=out, in_=d1)
        return eng.add_instruction(mybir.InstTensorScalarPtr(
            name=nc.get_next_instruction_name(),
            is_tensor_tensor_scan=True,
            is_scalar_tensor_tensor=True,
            op0=MM, op1=AD,
            ins=[eng.lower_ap(d0), eng.lower_ap_or_imm(0.0), eng.lower_ap(d1)],
            outs=[eng.lower_ap(out)],
        ))

    def recip(out, in_):
        if os.environ.get('BASSK_SLOWRECIP'):
            return nc.vector.reciprocal(out=out, in_=in_)
        return nc.vector.reciprocal_approx_fast(out=out, in_=in_)

    def view(ap, dims, off=0):
        return bass.AP(tensor=ap.tensor, offset=ap.offset + off, ap=dims)

    with tile.TileContext(nc) as tc:
        with tc.tile_pool(name="big", bufs=1) as big, \
             tc.tile_pool(name="wres", bufs=1) as wres, \
             tc.tile_pool(name="scr", bufs=4) as scr, \
             tc.tile_pool(name="sml", bufs=3) as sml, \
             tc.tile_pool(name="y2b", bufs=9) as y2b, \
             tc.tile_pool(name="yupp", bufs=2) as yupp, \
             tc.tile_pool(name="wstr", bufs=2) as wstr, \
             tc.tile_pool(name="dsc", bufs=2, space="DRAM") as dsc, \
             tc.tile_pool(name="psmm", bufs=3, space="PSUM") as psmm, \
             tc.tile_pool(name="psst", bufs=1, space="PSUM") as psst, \
             tc.tile_pool(name="psbc", bufs=2, space="PSUM") as psbc:

            # ---- resident constants
            lt = wres.tile([128, 2, 64], F32, name="lt")
            nc.sync.dma_start(out=lt, in_=d_lam[:, :].rearrange("(t p) j -> p t j", p=128))
            cv = wres.tile([128, ncol], F32, name="cv")
            nc.sync.dma_start(out=cv, in_=d_cvec[:, :])
            red = wres.tile([128, 2, 2], F32R, name="red")
            nc.sync.dma_start(out=red, in_=d_red[:, :, :].bitcast(F32R))
            redb = wres.tile([128, 2, 2], BF16, name="redb")
            nc.sync.dma_start(out=redb, in_=d_redb[:, :, :])
            bc1 = wres.tile([2, 2, 128], F32R, name="bc1")
            nc.sync.dma_start(out=bc1, in_=d_bc1[:, :, :].bitcast(F32R))
            epsc = wres.tile([128, 1], F32, name="epsc")
            nc.sync.dma_start(out=epsc, in_=d_eps[:, :])
            c2wt = wres.tile([128, 4, 1024], BF16, name="c2wt")
            nc.sync.dma_start(out=c2wt, in_=d_c2[:, :].rearrange("(kt p) m -> p kt m", p=128))
            c3wt = wres.tile([128, 8, 256], BF16, name="c3wt")
            nc.sync.dma_start(out=c3wt, in_=d_c3[:, :].rearrange("(kt p) m -> p kt m", p=128))

            def col(name, i=0):
                return cv[:, colidx[name] + i:colidx[name] + i + 1]

            # ============ LayerNorm over channels (2 ctiles) ============
            def ln256(Xr, out_wr, name, wb=None, bf16_in=False):
                redm = redb if bf16_in else red
                dstat = dsc.tile([2, S], F32, name=f"dstat_{name}", tag="dstat")
                for ch in range(NCH):
                    sl = slice(ch * CH, (ch + 1) * CH)
                    ps = psst.tile([2, CH], F32, name=f"lnps_{name}", tag="st")
                    for ct in range(2):
                        nc.tensor.matmul(out=ps, lhsT=redm[:, 0, :], rhs=Xr(ct)[:, sl],
                                         start=(ct == 0), stop=False)
                    for ct in range(2):
                        sq = sml.tile([128, CH], BF16 if bf16_in else F32R,
                                      name=f"sq_{name}", tag="sqc", bufs=3)
                        nc.scalar.activation(
                            out=sq,
                            in_=Xr(ct)[:, sl] if bf16_in else Xr(ct)[:, sl].bitcast(F32),
                            func=Act.Square)
                        nc.tensor.matmul(out=ps, lhsT=redm[:, 1, :], rhs=sq,
                                         start=False, stop=(ct == 1))
                    stc = sml.tile([2, CH], F32, name=f"stc_{name}", tag="stc", bufs=3)
                    nc.scalar.copy(out=stc, in_=ps)
                    nc.sync.dma_start(out=dstat[:, sl], in_=stc)
                # small stat math in [128, 2, 32] layout: element (p,q,j) = stat[q, j*128+p]
                sm = sml.tile([128, 2, 32], F32, name=f"sm_{name}", tag="sm", bufs=2)
                nc.sync.dma_start(out=sm, in_=view(dstat[:, :], [[1, 128], [S, 2], [128, 32]]))
                nc.vector.tensor_scalar_mul(out=sm, in0=sm, scalar1=1.0 / C)
                t2 = sml.tile([128, 32], F32, name=f"t2_{name}", tag="t2", bufs=2)
                nc.vector.tensor_tensor(out=t2, in0=sm[:, 0, :], in1=sm[:, 0, :], op=MM)
                nc.vector.tensor_tensor(out=t2, in0=sm[:, 1, :], in1=t2, op=SU)
                nc.scalar.activation(out=t2, in_=t2, func=Act.Sqrt, bias=epsc)
                nc.vector.reciprocal(out=t2, in_=t2)                      # rstd
                nc.vector.tensor_tensor(out=sm[:, 0, :], in0=sm[:, 0, :], in1=t2, op=MM)
                drow = dsc.tile([2, S], F32, name=f"drow_{name}", tag="dstat")
                nc.sync.dma_start(out=view(drow[:, :], [[1, 128], [128, 32]]), in_=t2)
                nc.sync.dma_start(out=view(drow[:, :], [[1, 128], [128, 32]], off=S),
                                  in_=sm[:, 0, :])
                bcr = big.tile([2, S], F32R, name=f"bcr_{name}", tag="rowsbig")
                nc.sync.dma_start(out=bcr, in_=drow[:, :].bitcast(F32R))
                for ch in range(NCH):
                    sl = slice(ch * CH, (ch + 1) * CH)
                    pr = psbc.tile([128, CH], F32, name=f"pr_{name}", tag="pr")
                    pm = psbc.tile([128, CH], F32, name=f"pm_{name}", tag="pm")
                    nc.tensor.matmul(out=pr, lhsT=bc1[:, 0, :], rhs=bcr[:, sl])
                    nc.tensor.matmul(out=pm, lhsT=bc1[:, 1, :], rhs=bcr[:, sl])
                    for ct in range(2):
                        out_wr(ct, sl, pr, pm)
                if wb is not None:
                    wn, bn_, apfn = wb
                    for ct in range(2):
                        ap = apfn(ct)
                        nc.vector.tensor_scalar(
                            out=ap, in0=ap, scalar1=col(wn, ct), scalar2=col(bn_, ct),
                            op0=MM, op1=AD)

            # ============ S0/S1: load x, LN1 -> xn (bf16) ============
            x0 = big.tile([128, S], F32R, name="x0", tag="A")
            x1 = big.tile([128, S], F32R, name="x1", tag="B")
            nc.sync.dma_start(out=x0, in_=xin[0:128, :].bitcast(F32R))
            nc.sync.dma_start(out=x1, in_=xin[128:256, :].bitcast(F32R))
            mf = big.tile([128, S], F32, name="mf", tag="D")
            nc.sync.dma_start(out=mf, in_=view(mrow[:, :], [[0, 128], [1, S]]))

            xn = big.tile([128, 2, S], BF16, name="xn", tag="Cxn")
            xt = [x0, x1]

            def ln1_wr(ct, sl, pr, pm):
                nc.vector.tensor_tensor(out=xn[:, ct, sl], in0=xt[ct][:, sl].bitcast(F32),
                                        in1=pr, op=MM)
                nc.vector.tensor_tensor(out=xn[:, ct, sl], in0=xn[:, ct, sl],
                                        in1=pm, op=SU)

            ln256(lambda ct: xt[ct][:, :], ln1_wr, "ln1",
                  wb=None if ln1_triv else ("ln1w", "ln1b", lambda ct: xn[:, ct, :]))
            if probe:
                pxn = mkprobe("p_xn", [C, S])
                for ct in range(2):
                    nc.gpsimd.dma_start(out=pxn[128 * ct:128 * (ct + 1), :],
                                        in_=xn[:, ct, :])

            # ============ S2: q_shift diff * mask -> md (bf16) ============
            xn4 = xn.rearrange("p t (h w) -> p t h w", h=H)
            md = big.tile([128, 2, H, W], BF16, name="md", tag="Emd")
            nc.vector.tensor_tensor(out=md[0:64, 0, :, 1:], in0=xn4[0:64, 0, :, 0:63],
                                    in1=xn4[0:64, 0, :, 1:], op=SU)
            nc.vector.tensor_scalar_mul(out=md[0:64, 0, :, 0:1],
                                        in0=xn4[0:64, 0, :, 0:1], scalar1=-1.0)
            nc.vector.tensor_tensor(out=md[64:128, 0, :, 0:63], in0=xn4[64:128, 0, :, 1:],
                                    in1=xn4[64:128, 0, :, 0:63], op=SU)
            nc.vector.tensor_scalar_mul(out=md[64:128, 0, :, 63:64],
                                        in0=xn4[64:128, 0, :, 63:64], scalar1=-1.0)
            nc.vector.tensor_tensor(out=md[0:64, 1, 1:, :], in0=xn4[0:64, 1, 0:63, :],
                                    in1=xn4[0:64, 1, 1:, :], op=SU)
            nc.vector.tensor_scalar_mul(out=md[0:64, 1, 0:1, :],
                                        in0=xn4[0:64, 1, 0:1, :], scalar1=-1.0)
            nc.vector.tensor_tensor(out=md[64:128, 1, 0:63, :], in0=xn4[64:128, 1, 1:, :],
                                    in1=xn4[64:128, 1, 0:63, :], op=SU)
            nc.vector.tensor_scalar_mul(out=md[64:128, 1, 63:64, :],
                                        in0=xn4[64:128, 1, 63:64, :], scalar1=-1.0)
            mdf = md.rearrange("p t h w -> p t (h w)")
            for ct in range(2):
                nc.vector.tensor_tensor(out=mdf[:, ct, :], in0=mdf[:, ct, :],
                                        in1=mf, op=MM)

            # ============ S3: k/v/r matmuls -> E, V, SR; scans ============
            ev = big.tile([128, 2, S], BF16, name="ev", tag="B")
            et = big.tile([128, 2, S], BF16, name="et", tag="A")
            vv = big.tile([128, 2, S], BF16, name="vv", tag="D")
            sr = big.tile([128, 2, S], BF16, name="sr", tag="Fsr")

            def kvloop(dws, evac):
                wxt = wstr.tile([128, 2, 256], BF16, name="wxt", tag="wst", bufs=2)
                wdt = wstr.tile([128, 2, 256], BF16, name="wdt", tag="wst", bufs=2)
                nc.sync.dma_start(out=wxt, in_=dws[0][:, :].rearrange("(kt p) m -> p kt m", p=128))
                nc.sync.dma_start(out=wdt, in_=dws[1][:, :].rearrange("(kt p) m -> p kt m", p=128))
                for mt in range(2):
                    for ch in range(NCH):
                        sl = slice(ch * CH, (ch + 1) * CH)
                        ps = psmm.tile([128, CH], F32, name="kv_ps", tag="mm")
                        for kt in range(2):
                            nc.tensor.matmul(out=ps, lhsT=wxt[:, kt, 128 * mt:128 * (mt + 1)],
                                             rhs=xn[:, kt, sl], start=(kt == 0), stop=False)
                        for kt in range(2):
                            nc.tensor.matmul(out=ps, lhsT=wdt[:, kt, 128 * mt:128 * (mt + 1)],
                                             rhs=mdf[:, kt, sl], start=False, stop=(kt == 1))
                        evac(mt, sl, ps)

            kvloop(d_wk, lambda mt, sl, ps: nc.scalar.activation(
                out=et[:, mt, sl], in_=ps, func=Act.Exp))
            kvloop(d_wv, lambda mt, sl, ps: nc.scalar.copy(out=vv[:, mt, sl], in_=ps))
            kvloop(d_wr, lambda mt, sl, ps: nc.scalar.activation(
                out=sr[:, mt, sl], in_=ps, func=Act.Sigmoid))

            nc.vector.tensor_tensor(out=ev, in0=et, in1=vv, op=MM)

            ev4 = ev.rearrange("p t (h w) -> p t h w", h=H)
            et4 = et.rearrange("p t (h w) -> p t h w", h=H)
            outv = big.tile([128, 2, W, H], BF16, name="outv", tag="D")
            lt_ap = lt[:, :, :]

            def lamview(ct, nseq):
                return view(lt_ap, [lt_ap.ap[0], [0, nseq], [1, 64]], off=ct * 64)

            # vertical orientation first (reads pristine ev/et via transposed APs)
            for half in range(2):
                wr_ = slice(half * 32, (half + 1) * 32)
                av = scr.tile([128, 2, 32, 64], BF16, name="av", tag="scrt")
                bv = scr.tile([128, 2, 32, 64], BF16, name="bv", tag="scrt")
                for ct in range(2):
                    dv_ev = view(ev[:, :, :], [ev.ap[0], [1, 32], [64, 64]],
                                 off=ct * S + half * 32)
                    dv_et = view(et[:, :, :], [et.ap[0], [1, 32], [64, 64]],
                                 off=ct * S + half * 32)
                    scan_raw(av[:, ct], lamview(ct, 32), dv_ev)
                    scan_raw(bv[:, ct], lamview(ct, 32), dv_et)
                for ct in range(2):
                    base = ct * S + half * 32
                    den = scr.tile([128, 32, 64], F32, name="den", tag="scrt")
                    nc.vector.scalar_tensor_tensor(
                        out=den[:, :, 1:],
                        in0=view(et[:, :, :], [et.ap[0], [1, 32], [64, 63]], off=base + 64),
                        scalar=col('eu', ct), in1=bv[:, ct, :, 0:63], op0=MM, op1=AD)
                    nc.vector.tensor_scalar_mul(
                        out=den[:, :, 0:1],
                        in0=view(et[:, :, :], [et.ap[0], [1, 32], [64, 1]], off=base),
                        scalar1=col('eu', ct))
                    recip(out=den, in_=den)
                    nc.vector.scalar_tensor_tensor(
                        out=outv[:, ct, wr_, 1:],
                        in0=view(ev[:, :, :], [ev.ap[0], [1, 32], [64, 63]], off=base + 64),
                        scalar=col('eu', ct), in1=av[:, ct, :, 0:63], op0=MM, op1=AD)
                    nc.vector.tensor_scalar_mul(
                        out=outv[:, ct, wr_, 0:1],
                        in0=view(ev[:, :, :], [ev.ap[0], [1, 32], [64, 1]], off=base),
                        scalar1=col('eu', ct))
                    nc.vector.tensor_tensor(out=outv[:, ct, wr_, :], in0=outv[:, ct, wr_, :],
                                            in1=den, op=MM)

            # horizontal orientation; num/out in place on ev
            for half in range(2):
                hr = slice(half * 32, (half + 1) * 32)
                ah = scr.tile([128, 2, 32, 64], BF16, name="ah", tag="scrt")
                bh = scr.tile([128, 2, 32, 64], BF16, name="bh", tag="scrt")
                for ct in range(2):
                    scan_raw(ah[:, ct], lamview(ct, 32), ev4[:, ct, hr, :])
                    scan_raw(bh[:, ct], lamview(ct, 32), et4[:, ct, hr, :])
                for ct in range(2):
                    den = scr.tile([128, 32, 64], F32, name="den2", tag="scrt")
                    nc.vector.scalar_tensor_tensor(
                        out=den[:, :, 1:], in0=et4[:, ct, hr, 1:],
                        scalar=col('eu', ct), in1=bh[:, ct, :, 0:63], op0=MM, op1=AD)
                    nc.vector.tensor_scalar_mul(
                        out=den[:, :, 0:1], in0=et4[:, ct, hr, 0:1], scalar1=col('eu', ct))
                    recip(out=den, in_=den)
                    nc.vector.scalar_tensor_tensor(
                        out=ev4[:, ct, hr, 1:], in0=ev4[:, ct, hr, 1:],
                        scalar=col('eu', ct), in1=ah[:, ct, :, 0:63], op0=MM, op1=AD)
                    nc.vector.tensor_scalar_mul(
                        out=ev4[:, ct, hr, 0:1], in0=ev4[:, ct, hr, 0:1],
                        scalar1=col('eu', ct))
                    nc.vector.tensor_tensor(out=ev4[:, ct, hr, :], in0=ev4[:, ct, hr, :],
                                            in1=den, op=MM)

            # wkv = out_h + out_v^T (0.5 factor dropped: LN-invariant)
            for ct in range(2):
                ovT = view(outv[:, :, :, :], [outv.ap[0], [1, 64], [64, 64]], off=ct * S)
                nc.vector.tensor_tensor(out=ev4[:, ct, :, :], in0=ev4[:, ct, :, :],
                                        in1=ovT, op=AD)
            if probe:
                pwkv = mkprobe("p_wkv", [C, S])
                for ct in range(2):
                    nc.gpsimd.dma_start(out=pwkv[128 * ct:128 * (ct + 1), :],
                                        in_=ev[:, ct, :])

            # ============ S4: key-LN, srw, Wo+residual, skip feat ============
            def kn_wr(ct, sl, pr, pm):
                nc.vector.tensor_tensor(out=ev[:, ct, sl], in0=ev[:, ct, sl], in1=pr, op=MM)
                nc.vector.tensor_tensor(out=ev[:, ct, sl], in0=ev[:, ct, sl], in1=pm, op=SU)

            ln256(lambda ct: ev[:, ct, :], kn_wr, "kn", bf16_in=True,
                  wb=None if kn_triv else ("knw", "knb", lambda ct: ev[:, ct, :]))

            nc.vector.tensor_tensor(out=sr, in0=sr, in1=ev, op=MM)   # srw

            xcp = [scr.tile([128, 66, 66], BF16, name=f"xcp{i}", tag="scrt")
                   for i in range(4)]
            for t in xcp:
                nc.vector.memset(t[:, 0:1, :], 0.0)
                nc.vector.memset(t[:, 65:66, :], 0.0)
                nc.vector.memset(t[:, 1:65, 0:1], 0.0)
                nc.vector.memset(t[:, 1:65, 65:66], 0.0)

            wot = wstr.tile([128, 2, 256], BF16, name="wot", tag="wst", bufs=2)
            nc.sync.dma_start(out=wot, in_=d_wo[:, :].rearrange("(kt p) m -> p kt m", p=128))
            for mt in range(2):
                for ch in range(NCH):
                    sl = slice(ch * CH, (ch + 1) * CH)
                    h0 = ch * 8
                    ps = psmm.tile([128, CH], F32, name="wo_ps", tag="mm")
                    for kt in range(2):
                        nc.tensor.matmul(out=ps, lhsT=wot[:, kt, 128 * mt:128 * (mt + 1)],
                                         rhs=sr[:, kt, sl], start=(kt == 0), stop=(kt == 1))
                    nc.vector.tensor_tensor(
                        out=xcp[mt][:, 1 + h0:9 + h0, 1:65],
                        in0=xn4[:, mt, h0:h0 + 8, :],
                        in1=ps.rearrange("p (a b) -> p a b", a=8), op=AD)

            spt = wstr.tile([128, 4, 256], F32R, name="spt", tag="wst4", bufs=1)
            nc.sync.dma_start(out=spt,
                              in_=d_sp[:, :].rearrange("(kt p) m -> p kt m", p=128).bitcast(F32R))
            for ch in range(NCH):
                sl = slice(ch * CH, (ch + 1) * CH)
                h0 = ch * 8
                skc = big.tile([128, 4, CH], F32R, name="skc",
                               tag="A" if ch % 2 == 0 else "B")
                nc.sync.dma_start(
                    out=skc,
                    in_=skin[:, sl].rearrange("(kt p) n -> p kt n", p=128).bitcast(F32R))
                for mt in range(2):
                    ps = psmm.tile([128, CH], F32, name="sp_ps", tag="mm")
                    for kt in range(4):
                        nc.tensor.matmul(out=ps, lhsT=spt[:, kt, 128 * mt:128 * (mt + 1)],
                                         rhs=skc[:, kt, :], start=(kt == 0), stop=(kt == 3))
                    nc.scalar.activation(
                        out=xcp[2 + mt][:, 1 + h0:9 + h0, 1:65],
                        in_=ps.rearrange("p (a b) -> p a b", a=8),
                        func=Act.Identity, bias=col('spb', mt))

            if probe:
                pxc = mkprobe("p_xcat", [CS, S])
                for i in range(4):
                    nc.gpsimd.dma_start(
                        out=pxc[128 * i:128 * (i + 1), :].rearrange("p (a b) -> p a b", a=64),
                        in_=xcp[i][:, 1:65, 1:65])

            # ============ S5: grouped 3x3 conv -> gelu -> y1 (bf16) ============
            y1a = big.tile([128, 2, S], BF16, name="y1a", tag="A")
            y1b = big.tile([128, 2, S], BF16, name="y1b", tag="Cxn")
            y1t = [y1a, y1b]
            # prime the wst9 slots so the c1 weight DMAs land after the
            # scan/Wo stages (works around early-SBUF corruption of the
            # first-loaded tiles)
            if not os.environ.get('BASSK_NOPRIME'):
                for i in range(2):
                    pr_ = wstr.tile([128, 1], BF16, name=f"prime{i}", tag="wst9")
                    nc.vector.tensor_copy(out=pr_, in_=xcp[i][:, 0, 0:1])
            if probe and os.environ.get('BASSK_CANARY'):
                cnry = wstr.tile([128, 9, 2, 128], BF16, name="cnry", tag="wst9")
                nc.sync.dma_start(out=cnry, in_=d_c1[1, 0, :, :, :, :])
                marks = [("m0", cnry[:, 0, 0, 0:64]),
                         ("m1", xn[:, 0, 0:64]),
                         ("m2", ev[:, 0, 0:64]),
                         ("m3", sr[:, 0, 0:64])]
                for mi, (mn, mark) in enumerate(marks):
                    stg_c = sml.tile([128, 64], BF16, name=f"cst{mi}",
                                     tag="cst", bufs=4)
                    nc.vector.tensor_tensor(
                        out=stg_c, in0=cnry[:, 0, 0, 0:64],
                        in1=mark, op=Alu.bypass)
                    pc = mkprobe(f"p_cn{mi}", [128, 64])
                    nc.gpsimd.dma_start(out=pc[:, :], in_=stg_c)
            for g in (1, 0):
                for mt in range(2):
                    c1gm = wstr.tile([128, 9, 2, 128], BF16, name="c1gm", tag="wst9")
                    nc.sync.dma_start(out=c1gm, in_=d_c1[g, mt, :, :, :, :])
                    if probe and mt == 0:
                        pw = mkprobe(f"p_c1w_{g}", [128, 9 * 2 * 128])
                        nc.gpsimd.dma_start(out=pw[:, :],
                                            in_=c1gm.rearrange("p a b c -> p (a b c)"))
                    for ch in range(NCH):
                        h0 = ch * 8
                        ps = psmm.tile([128, CH], F32, name="c1_ps", tag="mm")
                        i = 0
                        for ti in range(9):
                            dy, dx = ti // 3 - 1, ti % 3 - 1
                            for kt in range(2):
                                nc.tensor.matmul(
                                    out=ps.rearrange("p (a b) -> p a b", a=8),
                                    lhsT=c1gm[:, ti, kt, :],
                                    rhs=xcp[2 * g + kt][:, 1 + h0 + dy:9 + h0 + dy,
                                                        1 + dx:65 + dx],
                                    start=(i == 0), stop=(i == 17))
                                i += 1
                        if probe and mt == 0 and ch == 0:
                            pps = mkprobe(f"p_c1ps_{g}", [128, CH])
                            stg = sml.tile([128, CH], F32, name="stg", tag="sqc")
                            nc.scalar.copy(out=stg, in_=ps)
                            nc.gpsimd.dma_start(out=pps[:, :], in_=stg)
                        nc.scalar.activation(
                            out=y1t[g][:, mt, ch * CH:(ch + 1) * CH], in_=ps,
                            func=Act.Gelu, bias=col('c1b', 2 * g + mt))

            if probe:
                py1 = mkprobe("p_y1", [CS, S])
                for i in range(4):
                    nc.gpsimd.dma_start(out=py1[128 * i:128 * (i + 1), :],
                                        in_=y1t[i // 2][:, i % 2, :])

            # ============ S6: c2 -> gelu -> c3 -> gelu(+bn3) -> y3 ============
            y3 = [big.tile([128, S], F32R, name="y3_0", tag="Emd"),
                  big.tile([128, S], F32R, name="y3_1", tag="D")]
            for ch in range(NCH):
                sl = slice(ch * CH, (ch + 1) * CH)
                ytiles = []
                for mt in range(8):
                    ps = psmm.tile([128, CH], F32, name="c2_ps", tag="mm")
                    for kt in range(4):
                        nc.tensor.matmul(out=ps, lhsT=c2wt[:, kt, 128 * mt:128 * (mt + 1)],
                                         rhs=y1t[kt // 2][:, kt % 2, sl],
                                         start=(kt == 0), stop=(kt == 3))
                    yt = y2b.tile([128, CH], BF16, name="y2t", tag="y2t")
                    nc.scalar.activation(out=yt, in_=ps, func=Act.Gelu, bias=col('c2b', mt))
                    ytiles.append(yt)
                for mt in range(2):
                    ps = psmm.tile([128, CH], F32, name="c3_ps", tag="mm")
                    for kt in range(8):
                        nc.tensor.matmul(out=ps, lhsT=c3wt[:, kt, 128 * mt:128 * (mt + 1)],
                                         rhs=ytiles[kt], start=(kt == 0), stop=(kt == 7))
                    nc.scalar.activation(out=y3[mt][:, sl], in_=ps, func=Act.Gelu,
                                         bias=col('c3b', mt))
                    if not bn3_triv:
                        nc.vector.tensor_scalar(out=y3[mt][:, sl],
                                                in0=y3[mt][:, sl].bitcast(F32),
                                                scalar1=col('g3p', mt),
                                                scalar2=col('b3p', mt), op0=MM, op1=AD)

            if probe:
                py3 = mkprobe("p_y3", [C, S])
                for i in range(2):
                    nc.gpsimd.dma_start(out=py3[128 * i:128 * (i + 1), :],
                                        in_=y3[i][:, :].bitcast(F32))

            # ============ S7: LN2, up-proj, pixel-shuffle DMA out ============
            def ln2_wr(ct, sl, pr, pm):
                nc.vector.tensor_tensor(out=y3[ct][:, sl], in0=y3[ct][:, sl].bitcast(F32),
                                        in1=pr, op=MM)
                nc.vector.tensor_tensor(out=y3[ct][:, sl], in0=y3[ct][:, sl].bitcast(F32),
                                        in1=pm, op=SU)

            ln256(lambda ct: y3[ct][:, :], ln2_wr, "ln2",
                  wb=None if ln2_triv else ("ln2w", "ln2b", lambda ct: y3[ct][:, :]))

            upt = wstr.tile([128, 2, 512], F32R, name="upt", tag="wst4", bufs=1)
            nc.sync.dma_start(out=upt,
                              in_=d_up[:, :].rearrange("(kt p) m -> p kt m", p=128).bitcast(F32R))
            for r in range(2):
                for ch in range(NCH):
                    sl = slice(ch * CH, (ch + 1) * CH)
                    h0 = ch * 8
                    ub = yupp.tile([128, 8, 64, 2], F32, name="ub", tag="ub")
                    for q in range(2):
                        rq = 2 * r + q
                        ps = psmm.tile([128, CH], F32, name="up_ps", tag="mm")
                        for kt in range(2):
                            nc.tensor.matmul(out=ps,
                                             lhsT=upt[:, kt, 128 * rq:128 * (rq + 1)],
                                             rhs=y3[kt][:, sl],
                                             start=(kt == 0), stop=(kt == 1))
                        nc.scalar.activation(out=ub[:, :, :, q],
                                             in_=ps.rearrange("p (a b) -> p a b", a=8),
                                             func=Act.Identity, bias=col('upb', rq))
                    dst = view(yout[:, :, :], [[128 * 128, 128], [256, 8], [1, 128]],
                               off=(2 * h0 + r) * 128)
                    nc.sync.dma_start(out=dst, in_=ub.rearrange("p a b q -> p a (b q)"))

    nc.compile()
    return nc, const_inputs


def _get_nc(weights, probe=False):
    import hashlib
    hsh = hashlib.sha1()
    for k in sorted(weights):
        hsh.update(k.encode())
        hsh.update(np.ascontiguousarray(weights[k]).tobytes())
    key = (hsh.hexdigest(), probe)
    if key not in _CACHE:
        _CACHE[key] = _build(weights, probe=probe)
    return _CACHE[key]


def kernel(**inputs):
    from concourse.bass_utils import run_bass_kernel_spmd

    x = np.asarray(inputs['x'], np.float32)
    skip = np.asarray(inputs['skip'], np.float32)
    mask = np.asarray(inputs['saliency_mask'], np.float32)
    weights = {k: np.asarray(v, np.float32) for k, v in inputs.items()
               if k not in ('x', 'skip', 'saliency_mask')}

    probe = bool(os.environ.get('BASSK_PROBE'))
    nc, const_inputs = _get_nc(weights, probe=probe)

    in_maps = []
    for b in range(B):
        m = dict(
            xin=np.ascontiguousarray(x[b].reshape(C, S)),
            skin=np.ascontiguousarray(skip[b].reshape(CS, S)),
            mrow=np.ascontiguousarray(mask[b].reshape(1, S)),
        )
        m.update(const_inputs)
        in_maps.append(m)
    res = run_bass_kernel_spmd(nc, in_maps, core_ids=list(range(B)),
                               trace=bool(os.environ.get('BASSK_TRACE')))
    kernel.last_results = res
    out = np.stack([res.results[b]['yout'] for b in range(B)], axis=0)
    return out



# revision 16
# speedup vs baseline: 1.1244x; 1.1244x over previous
"""Trainium2 Bass kernel for nn_DecoderBlock (shape-guided RWKV decoder block).

Data-parallel over batch: B=8 samples -> 8 NeuronCores, one NEFF.

Per-core layout: channels on partitions (256ch -> 2 "ctiles" of 128), spatial
(h, w) flattened on the free dim (4096).

- LayerNorm over channels: square (ACT) -> ones-matmul partition reduction ->
  DRAM-bounce reshape -> tiny stat math -> K=1 matmul broadcast -> TT applies.
- q_shift / mask blend via shifted access patterns; per-channel mixes folded
  into Wk/Wv/Wr host-side (k = Wk@xn + (Wk*diag(1-mk))@md, md = mask*(xs-xn)).
- WKV: unstabilized linear recurrence A_t = lam*A_{t-1} + e^{k_t} v_t via the
  DVE TensorTensorScan instruction chained across rows (data0=0 resets at each
  row start); vertical orientation scans read E/EV through transposed APs.
- channel_fusion: grouped 3x3 conv = 9 shifted-AP matmuls accumulated in PSUM
  over zero-padded [c, 66, 66] inputs; BN folded into the next conv
  host-side; GELU+bias fused into the PSUM->SBUF evacuation on ACT.
- patch_expand: up-proj rows permuted host-side so pixel shuffle becomes a
  strided DMA.

Matmuls in float32r (full rate) or bf16; bulky intermediates bf16.
"""
import sys
import os

for _p in ('/opt/trn_rl_repo', '/root/.axon_site/_ro/trn_rl_repo'):
    if _p not in sys.path and os.path.isdir(_p):
        sys.path.append(_p)

import numpy as np

B, C, CS, COUT, H, W = 8, 256, 512, 128, 64, 64
S = H * W          # 4096
NCH = 8            # spatial chunks
CH = S // NCH      # 512
EPS = 1e-5

_CACHE = {}


def _build(weights, probe=False):
    const_inputs = {}
    import concourse.bass as bass
    from concourse import bacc
    import concourse.tile as tile
    import concourse.mybir as mybir
    import ml_dtypes

    F32 = mybir.dt.float32
    F32R = mybir.dt.float32r
    BF16 = mybir.dt.bfloat16
    Alu = mybir.AluOpType
    Act = mybir.ActivationFunctionType
    MM, AD, SU = Alu.mult, Alu.add, Alu.subtract

    w = weights
    f64 = lambda x: np.asarray(x, np.float64)
    bf = lambda a: np.asarray(a, dtype=ml_dtypes.bfloat16)

    # ---------------- host-side folding
    bnscale = 1.0 / np.sqrt(1.0 + EPS)
    g1p = f64(w['bn1_g']) * bnscale
    b1p = f64(w['bn1_b'])
    g2p = f64(w['bn2_g']) * bnscale
    b2p = f64(w['bn2_b'])
    g3p = (f64(w['bn3_g']) * bnscale).astype(np.float32)
    b3p = f64(w['bn3_b']).astype(np.float32)

    c2_eff = f64(w['c2_w']) * g1p[None, :]
    c2b_eff = (f64(w['c2_b']) + f64(w['c2_w']) @ b1p).astype(np.float32)
    c3_eff = f64(w['c3_w']) * g2p[None, :]
    c3b_eff = (f64(w['c3_b']) + f64(w['c3_w']) @ b2p).astype(np.float32)

    wk_x = f64(w['Wk']).T
    wk_d = (f64(w['Wk']) * (1.0 - f64(w['mix_k']))[None, :]).T
    wv_x = f64(w['Wv']).T
    wv_d = (f64(w['Wv']) * (1.0 - f64(w['mix_v']))[None, :]).T
    wr_x = f64(w['Wr']).T
    wr_d = (f64(w['Wr']) * (1.0 - f64(w['mix_r']))[None, :]).T
    wo_t = f64(w['Wo']).T
    sp_t = f64(w['sp_w']).T.astype(np.float32)

    lam = np.exp(-np.exp(f64(w['decay']))).astype(np.float32)
    lam64 = np.tile(lam[:, None], (1, 64))
    lam64[:, 0] = 0.0
    lam64 = lam64.astype(np.float32)
    eu = np.exp(f64(w['first'])).astype(np.float32)

    pidx = np.arange(512)
    old = (pidx % 128) * 4 + (pidx // 128)
    up_t = f64(w['up_w'])[old].T.astype(np.float32)             # [256, 512]
    up_tb = bf(up_t)
    upb_p = f64(w['up_b'])[old].astype(np.float32)

    c1w = f64(w['c1_w'])
    c1_l = np.zeros((9, 2, 256, 256), np.float32)
    for ti in range(9):
        dy, dx = ti // 3, ti % 3
        for g in range(2):
            c1_l[ti, g] = c1w[g * 256:(g + 1) * 256, :, dy, dx].T

    # per-channel vectors as columns of one [128, ncol] const
    cols, order = {}, []

    def addcol(name, vec):
        v = np.asarray(vec, np.float32).reshape(-1, 128)
        cols[name] = v
        order.append(name)

    addcol('eu', eu)
    addcol('ln1w', w['ln1_w'])
    addcol('ln1b', w['ln1_b'])
    addcol('knw', w['kn_w'])
    addcol('knb', w['kn_b'])
    addcol('ln2w', w['ln2_w'])
    addcol('ln2b', w['ln2_b'])
    addcol('g3p', g3p)
    addcol('b3p', b3p)
    addcol('spb', w['sp_b'])
    addcol('c3b', c3b_eff)
    addcol('c1b', w['c1_b'])
    addcol('c2b', c2b_eff)
    addcol('upb', upb_p)
    colidx, ncol = {}, 0
    for n in order:
        colidx[n] = ncol
        ncol += cols[n].shape[0]
    cvec_np = np.zeros((128, ncol), np.float32)
    for n in order:
        for i in range(cols[n].shape[0]):
            cvec_np[:, colidx[n] + i] = cols[n][i]

    ln1_triv = np.all(w['ln1_w'] == 1.0) and np.all(w['ln1_b'] == 0.0)
    kn_triv = np.all(w['kn_w'] == 1.0) and np.all(w['kn_b'] == 0.0)
    ln2_triv = np.all(w['ln2_w'] == 1.0) and np.all(w['ln2_b'] == 0.0)
    bn3_triv = np.all(g3p == g3p[0]) and np.all(b3p == 0.0)
    # uniform bn3 scale commutes with LN2 -> drop it entirely when trivial

    # ---------------- bass module
    nc = bacc.Bacc("TRN2", target_bir_lowering=False, debug=False, name="decblk")

    xin = nc.dram_tensor("xin", [C, S], F32, kind="ExternalInput")
    skin = nc.dram_tensor("skin", [CS, S], F32, kind="ExternalInput")
    mrow = nc.dram_tensor("mrow", [1, S], F32, kind="ExternalInput")
    yout = nc.dram_tensor("yout", [COUT, 2 * H, 2 * W], F32, kind="ExternalOutput")
    probes = {}

    def mkprobe(name, shape):
        if probe:
            probes[name] = nc.dram_tensor(name, shape, F32, kind="ExternalOutput")
        return probes.get(name)

    def it(arr, name):
        arr = np.ascontiguousarray(arr)
        import ml_dtypes as _md
        dt_ = {np.dtype(np.float32): F32, np.dtype(_md.bfloat16): BF16}[arr.dtype]
        const_inputs[name] = arr
        return nc.dram_tensor(name, list(arr.shape), dt_, kind="ExternalInput")
    d_lam = it(lam64, "lam64")
    d_cvec = it(cvec_np, "cvec")
    d_wk = [it(bf(wk_x), "wkx"), it(bf(wk_d), "wkd")]
    d_wv = [it(bf(wv_x), "wvx"), it(bf(wv_d), "wvd")]
    d_wr = [it(bf(wr_x), "wrx"), it(bf(wr_d), "wrd")]
    d_wo = it(bf(wo_t), "wo")
    d_sp = it(sp_t, "sp")
    d_upb = it(up_tb, "up")
    c1_r = c1_l.reshape(9, 2, 2, 128, 2, 128).transpose(1, 4, 3, 0, 2, 5)
    d_c1 = it(bf(c1_r), "c1")   # [g, mt, p, t, kt, m]
    d_c2 = it(bf(c2_eff.T), "c2")
    d_c3 = it(bf(c3_eff.T), "c3")
    red_np = np.zeros((128, 2, 2), np.float32)
    red_np[:, 0, 0] = 1.0
    red_np[:, 1, 1] = 1.0
    d_red = it(red_np, "red")
    d_redb = it(bf(red_np), "redb")
    bc2_np = np.zeros((2, 2, 128), np.float32)
    bc2_np[0, 0, :] = 1.0
    bc2_np[1, 1, :] = 1.0
    d_bc1 = it(bc2_np, "bc2")
    d_eps = it(np.full((128, 1), EPS, np.float32), "epsc")
    d_i2 = it(np.eye(2, dtype=np.float32), "i2c")
    d_i128 = it(np.eye(128, dtype=np.float32), "i128c")
    d_i128b = it(bf(np.eye(128, dtype=np.float32)), "i128b")

    def scan_raw(out, d0, d1):
        eng = nc.vector
        if os.environ.get('BASSK_NOSCAN'):
            return eng.tensor_copy(out=out, in_=d1)
        return eng.add_instruction(mybir.InstTensorScalarPtr(
            name=nc.get_next_instruction_name(),
            is_tensor_tensor_scan=True,
            is_scalar_tensor_tensor=True,
            op0=MM, op1=AD,
            ins=[eng.lower_ap(d0), eng.lower_ap_or_imm(0.0), eng.lower_ap(d1)],
            outs=[eng.lower_ap(out)],
        ))

    def recip(out, in_):
        if os.environ.get('BASSK_SLOWRECIP'):
            return nc.vector.reciprocal(out=out, in_=in_)
        return nc.vector.reciprocal_approx_fast(out=out, in_=in_)

    def view(ap, dims, off=0):
        return bass.AP(tensor=ap.tensor, offset=ap.offset + off, ap=dims)

    with tile.TileContext(nc) as tc:
        with tc.tile_pool(name="big", bufs=1) as big, \
             tc.tile_pool(name="wres", bufs=1) as wres, \
             tc.tile_pool(name="scr", bufs=4) as scr, \
             tc.tile_pool(name="sml", bufs=3) as sml, \
             tc.tile_pool(name="y2b", bufs=9) as y2b, \
             tc.tile_pool(name="yupp", bufs=2) as yupp, \
             tc.tile_pool(name="wstr", bufs=2) as wstr, \
             tc.tile_pool(name="psmm", bufs=3, space="PSUM") as psmm, \
             tc.tile_pool(name="psst", bufs=1, space="PSUM") as psst, \
             tc.tile_pool(name="psln", bufs=1, space="PSUM") as psln, \
             tc.tile_pool(name="psbc", bufs=1, space="PSUM") as psbc:

            # ---- resident constants
            lt = wres.tile([128, 2, 64], F32, name="lt")
            nc.sync.dma_start(out=lt, in_=d_lam[:, :].rearrange("(t p) j -> p t j", p=128))
            cv = wres.tile([128, ncol], F32, name="cv")
            nc.sync.dma_start(out=cv, in_=d_cvec[:, :])
            red = wres.tile([128, 2, 2], F32R, name="red")
            nc.sync.dma_start(out=red, in_=d_red[:, :, :].bitcast(F32R))
            redb = wres.tile([128, 2, 2], BF16, name="redb")
            nc.sync.dma_start(out=redb, in_=d_redb[:, :, :])
            bc1 = wres.tile([2, 2, 128], F32R, name="bc1")
            nc.sync.dma_start(out=bc1, in_=d_bc1[:, :, :].bitcast(F32R))
            epsc = wres.tile([128, 1], F32, name="epsc")
            nc.sync.dma_start(out=epsc, in_=d_eps[:, :])
            i2c = wres.tile([2, 2], F32R, name="i2c")
            nc.sync.dma_start(out=i2c, in_=d_i2[:, :].bitcast(F32R))
            i128c = wres.tile([128, 128], F32R, name="i128c")
            nc.sync.dma_start(out=i128c, in_=d_i128[:, :].bitcast(F32R))
            i128b = wres.tile([128, 128], BF16, name="i128b")
            nc.sync.dma_start(out=i128b, in_=d_i128b[:, :])
            c2wt = wres.tile([128, 4, 1024], BF16, name="c2wt")
            nc.sync.dma_start(out=c2wt, in_=d_c2[:, :].rearrange("(kt p) m -> p kt m", p=128))
            c3wt = wres.tile([128, 8, 256], BF16, name="c3wt")
            nc.sync.dma_start(out=c3wt, in_=d_c3[:, :].rearrange("(kt p) m -> p kt m", p=128))

            def col(name, i=0):
                return cv[:, colidx[name] + i:colidx[name] + i + 1]

            # ============ LayerNorm over channels (2 ctiles) ============
            # On-chip stats path: ones-matmul partition reduction -> PE
            # transpose to pixel-major [128, 32, 2] -> tiny stat math ->
            # PE transpose back to [2, S] -> K=2 broadcast matmuls.
            def ln256(Xr, out_wr, name, wb=None, bf16_in=False):
                redm = redb if bf16_in else red
                pst = psln.tile([128, 32, 2], F32, name=f"pst_{name}", tag="pst")
                for ch in range(NCH):
                    sl = slice(ch * CH, (ch + 1) * CH)
                    ps = psst.tile([2, CH], F32, name=f"lnps_{name}", tag="st")
                    for ct in range(2):
                        nc.tensor.matmul(out=ps, lhsT=redm[:, 0, :], rhs=Xr(ct)[:, sl],
                                         start=(ct == 0), stop=False)
                    for ct in range(2):
                        sq = sml.tile([128, CH], BF16 if bf16_in else F32R,
                                      name=f"sq_{name}", tag="sqc", bufs=3)
                        nc.scalar.activation(
                            out=sq,
                            in_=Xr(ct)[:, sl] if bf16_in else Xr(ct)[:, sl].bitcast(F32),
                            func=Act.Square)
                        nc.tensor.matmul(out=ps, lhsT=redm[:, 1, :], rhs=sq,
                                         start=False, stop=(ct == 1))
                    stc = sml.tile([2, CH], F32R, name=f"stc_{name}", tag="stc", bufs=3)
                    nc.scalar.copy(out=stc, in_=ps)
                    for j in range(4):
                        nc.tensor.matmul(
                            out=pst[:, ch * 4 + j, :],
                            lhsT=stc[:, j * 128:(j + 1) * 128],
                            rhs=i2c[:, :])
                # small stat math in [128, 32, 2] pixel-major layout
                smb = sml.tile([128, 32, 2], F32, name=f"smb_{name}", tag="sm", bufs=2)
                nc.vector.tensor_scalar_mul(out=smb, in0=pst, scalar1=1.0 / C)
                t2 = sml.tile([128, 32], F32, name=f"t2_{name}", tag="t2", bufs=2)
                nc.vector.tensor_tensor(out=t2, in0=smb[:, :, 0], in1=smb[:, :, 0], op=MM)
                nc.vector.tensor_tensor(out=t2, in0=smb[:, :, 1], in1=t2, op=SU)
                nc.scalar.activation(out=t2, in_=t2, func=Act.Sqrt, bias=epsc)
                sm2 = sml.tile([128, 32, 2], F32R, name=f"sm2_{name}", tag="sm2", bufs=2)
                with nc.allow_low_precision(reason="rstd in f32r for PE transpose"):
                    nc.vector.reciprocal(out=sm2[:, :, 0], in_=t2)         # rstd
                nc.vector.tensor_tensor(out=sm2[:, :, 1], in0=smb[:, :, 0],
                                        in1=sm2[:, :, 0].bitcast(F32), op=MM)  # mu*rstd
                # transpose back: bcr row0 = rstd, row1 = mu*rstd
                bcr = big.tile([2, S], F32R, name=f"bcr_{name}", tag="rowsbig")
                for ch in range(NCH):
                    sl = slice(ch * CH, (ch + 1) * CH)
                    psb = psst.tile([2, CH], F32, name=f"psb_{name}", tag="st")
                    for j in range(4):
                        nc.tensor.matmul(
                            out=psb[:, j * 128:(j + 1) * 128],
                            lhsT=sm2[:, ch * 4 + j, :],
                            rhs=i128c[:, :])
                    nc.scalar.copy(out=bcr[:, sl], in_=psb)
                    pr = psbc.tile([128, CH], F32, name=f"pr_{name}", tag="pr")
                    pm = psbc.tile([128, CH], F32, name=f"pm_{name}", tag="pm")
                    nc.tensor.matmul(out=pr, lhsT=bc1[:, 0, :], rhs=bcr[:, sl])
                    nc.tensor.matmul(out=pm, lhsT=bc1[:, 1, :], rhs=bcr[:, sl])
                    for ct in range(2):
                        out_wr(ct, sl, pr, pm)
                if wb is not None:
                    wn, bn_, apfn = wb
                    for ct in range(2):
                        ap = apfn(ct)
                        nc.vector.tensor_scalar(
                            out=ap, in0=ap, scalar1=col(wn, ct), scalar2=col(bn_, ct),
                            op0=MM, op1=AD)

            # ============ S0/S1: load x, LN1 -> xn (bf16) ============
            x0 = big.tile([128, S], F32R, name="x0", tag="A")
            x1 = big.tile([128, S], F32R, name="x1", tag="B")
            nc.sync.dma_start(out=x0, in_=xin[0:128, :].bitcast(F32R))
            nc.sync.dma_start(out=x1, in_=xin[128:256, :].bitcast(F32R))
            mf = big.tile([128, S], F32, name="mf", tag="D")
            nc.sync.dma_start(out=mf, in_=view(mrow[:, :], [[0, 128], [1, S]]))

            xn = big.tile([128, 2, S], BF16, name="xn", tag="Cxn")
            xt = [x0, x1]

            def ln1_wr(ct, sl, pr, pm):
                nc.vector.tensor_tensor(out=xn[:, ct, sl], in0=xt[ct][:, sl].bitcast(F32),
                                        in1=pr, op=MM)
                nc.vector.tensor_tensor(out=xn[:, ct, sl], in0=xn[:, ct, sl],
                                        in1=pm, op=SU)

            ln256(lambda ct: xt[ct][:, :], ln1_wr, "ln1",
                  wb=None if ln1_triv else ("ln1w", "ln1b", lambda ct: xn[:, ct, :]))
            if probe:
                pxn = mkprobe("p_xn", [C, S])
                for ct in range(2):
                    nc.gpsimd.dma_start(out=pxn[128 * ct:128 * (ct + 1), :],
                                        in_=xn[:, ct, :])

            # ============ S2: q_shift diff * mask -> md (bf16) ============
            xn4 = xn.rearrange("p t (h w) -> p t h w", h=H)
            md = big.tile([128, 2, H, W], BF16, name="md", tag="Emd")
            nc.vector.tensor_tensor(out=md[0:64, 0, :, 1:], in0=xn4[0:64, 0, :, 0:63],
                                    in1=xn4[0:64, 0, :, 1:], op=SU)
            nc.vector.tensor_scalar_mul(out=md[0:64, 0, :, 0:1],
                                        in0=xn4[0:64, 0, :, 0:1], scalar1=-1.0)
            nc.vector.tensor_tensor(out=md[64:128, 0, :, 0:63], in0=xn4[64:128, 0, :, 1:],
                                    in1=xn4[64:128, 0, :, 0:63], op=SU)
            nc.vector.tensor_scalar_mul(out=md[64:128, 0, :, 63:64],
                                        in0=xn4[64:128, 0, :, 63:64], scalar1=-1.0)
            nc.vector.tensor_tensor(out=md[0:64, 1, 1:, :], in0=xn4[0:64, 1, 0:63, :],
                                    in1=xn4[0:64, 1, 1:, :], op=SU)
            nc.vector.tensor_scalar_mul(out=md[0:64, 1, 0:1, :],
                                        in0=xn4[0:64, 1, 0:1, :], scalar1=-1.0)
            nc.vector.tensor_tensor(out=md[64:128, 1, 0:63, :], in0=xn4[64:128, 1, 1:, :],
                                    in1=xn4[64:128, 1, 0:63, :], op=SU)
            nc.vector.tensor_scalar_mul(out=md[64:128, 1, 63:64, :],
                                        in0=xn4[64:128, 1, 63:64, :], scalar1=-1.0)
            mdf = md.rearrange("p t h w -> p t (h w)")
            for ct in range(2):
                nc.vector.tensor_tensor(out=mdf[:, ct, :], in0=mdf[:, ct, :],
                                        in1=mf, op=MM)

            # ============ S3: k/v/r matmuls -> E, V, SR; scans ============
            ev = big.tile([128, 2, S], BF16, name="ev", tag="B")
            et = big.tile([128, 2, S], BF16, name="et", tag="A")
            vv = big.tile([128, 2, S], BF16, name="vv", tag="D")
            sr = big.tile([128, 2, S], BF16, name="sr", tag="Fsr")

            def kvloop(dws, evac):
                wxt = wstr.tile([128, 2, 256], BF16, name="wxt", tag="wst", bufs=2)
                wdt = wstr.tile([128, 2, 256], BF16, name="wdt", tag="wst", bufs=2)
                nc.sync.dma_start(out=wxt, in_=dws[0][:, :].rearrange("(kt p) m -> p kt m", p=128))
                nc.sync.dma_start(out=wdt, in_=dws[1][:, :].rearrange("(kt p) m -> p kt m", p=128))
                for mt in range(2):
                    for ch in range(NCH):
                        sl = slice(ch * CH, (ch + 1) * CH)
                        ps = psmm.tile([128, CH], F32, name="kv_ps", tag="mm")
                        for kt in range(2):
                            nc.tensor.matmul(out=ps, lhsT=wxt[:, kt, 128 * mt:128 * (mt + 1)],
                                             rhs=xn[:, kt, sl], start=(kt == 0), stop=False)
                        for kt in range(2):
                            nc.tensor.matmul(out=ps, lhsT=wdt[:, kt, 128 * mt:128 * (mt + 1)],
                                             rhs=mdf[:, kt, sl], start=False, stop=(kt == 1))
                        evac(mt, sl, ps)

            kvloop(d_wk, lambda mt, sl, ps: nc.scalar.activation(
                out=et[:, mt, sl], in_=ps, func=Act.Exp))
            kvloop(d_wv, lambda mt, sl, ps: nc.scalar.copy(out=vv[:, mt, sl], in_=ps))
            kvloop(d_wr, lambda mt, sl, ps: nc.scalar.activation(
                out=sr[:, mt, sl], in_=ps, func=Act.Sigmoid))

            nc.vector.tensor_tensor(out=ev, in0=et, in1=vv, op=MM)

            ev4 = ev.rearrange("p t (h w) -> p t h w", h=H)
            et4 = et.rearrange("p t (h w) -> p t h w", h=H)
            outv = big.tile([128, 2, W, H], BF16, name="outv", tag="D")
            lt_ap = lt[:, :, :]

            def lamview(ct, nseq):
                return view(lt_ap, [lt_ap.ap[0], [0, nseq], [1, 64]], off=ct * 64)

            # vertical orientation first (reads pristine ev/et via transposed APs)
            for half in range(2):
                wr_ = slice(half * 32, (half + 1) * 32)
                av = scr.tile([128, 2, 32, 64], BF16, name="av", tag="scrt")
                bv = scr.tile([128, 2, 32, 64], BF16, name="bv", tag="scrt")
                for ct in range(2):
                    dv_ev = view(ev[:, :, :], [ev.ap[0], [1, 32], [64, 64]],
                                 off=ct * S + half * 32)
                    dv_et = view(et[:, :, :], [et.ap[0], [1, 32], [64, 64]],
                                 off=ct * S + half * 32)
                    scan_raw(av[:, ct], lamview(ct, 32), dv_ev)
                    scan_raw(bv[:, ct], lamview(ct, 32), dv_et)
                for ct in range(2):
                    base = ct * S + half * 32
                    den = scr.tile([128, 32, 64], F32, name="den", tag="scrt")
                    nc.vector.scalar_tensor_tensor(
                        out=den[:, :, 1:],
                        in0=view(et[:, :, :], [et.ap[0], [1, 32], [64, 63]], off=base + 64),
                        scalar=col('eu', ct), in1=bv[:, ct, :, 0:63], op0=MM, op1=AD)
                    nc.vector.tensor_scalar_mul(
                        out=den[:, :, 0:1],
                        in0=view(et[:, :, :], [et.ap[0], [1, 32], [64, 1]], off=base),
                        scalar1=col('eu', ct))
                    recip(out=den, in_=den)
                    nc.vector.scalar_tensor_tensor(
                        out=outv[:, ct, wr_, 1:],
                        in0=view(ev[:, :, :], [ev.ap[0], [1, 32], [64, 63]], off=base + 64),
                        scalar=col('eu', ct), in1=av[:, ct, :, 0:63], op0=MM, op1=AD)
                    nc.vector.tensor_scalar_mul(
                        out=outv[:, ct, wr_, 0:1],
                        in0=view(ev[:, :, :], [ev.ap[0], [1, 32], [64, 1]], off=base),
                        scalar1=col('eu', ct))
                    nc.vector.tensor_tensor(out=outv[:, ct, wr_, :], in0=outv[:, ct, wr_, :],
                                            in1=den, op=MM)

            # horizontal orientation; num/out in place on ev
            for half in range(2):
                hr = slice(half * 32, (half + 1) * 32)
                ah = scr.tile([128, 2, 32, 64], BF16, name="ah", tag="scrt")
                bh = scr.tile([128, 2, 32, 64], BF16, name="bh", tag="scrt")
                for ct in range(2):
                    scan_raw(ah[:, ct], lamview(ct, 32), ev4[:, ct, hr, :])
                    scan_raw(bh[:, ct], lamview(ct, 32), et4[:, ct, hr, :])
                for ct in range(2):
                    den = scr.tile([128, 32, 64], F32, name="den2", tag="scrt")
                    nc.vector.scalar_tensor_tensor(
                        out=den[:, :, 1:], in0=et4[:, ct, hr, 1:],
                        scalar=col('eu', ct), in1=bh[:, ct, :, 0:63], op0=MM, op1=AD)
                    nc.vector.tensor_scalar_mul(
                        out=den[:, :, 0:1], in0=et4[:, ct, hr, 0:1], scalar1=col('eu', ct))
                    recip(out=den, in_=den)
                    nc.vector.scalar_tensor_tensor(
                        out=ev4[:, ct, hr, 1:], in0=ev4[:, ct, hr, 1:],
                        scalar=col('eu', ct), in1=ah[:, ct, :, 0:63], op0=MM, op1=AD)
                    nc.vector.tensor_scalar_mul(
                        out=ev4[:, ct, hr, 0:1], in0=ev4[:, ct, hr, 0:1],
                        scalar1=col('eu', ct))
                    nc.vector.tensor_tensor(out=ev4[:, ct, hr, :], in0=ev4[:, ct, hr, :],
                                            in1=den, op=MM)

            # wkv = out_h + out_v^T (0.5 factor dropped: LN-invariant)
            for ct in range(2):
                ovT = view(outv[:, :, :, :], [outv.ap[0], [1, 64], [64, 64]], off=ct * S)
                nc.vector.tensor_tensor(out=ev4[:, ct, :, :], in0=ev4[:, ct, :, :],
                                        in1=ovT, op=AD)
            if probe:
                pwkv = mkprobe("p_wkv", [C, S])
                for ct in range(2):
                    nc.gpsimd.dma_start(out=pwkv[128 * ct:128 * (ct + 1), :],
                                        in_=ev[:, ct, :])

            # ============ S4: key-LN, srw, Wo+residual, skip feat ============
            def kn_wr(ct, sl, pr, pm):
                nc.vector.tensor_tensor(out=ev[:, ct, sl], in0=ev[:, ct, sl], in1=pr, op=MM)
                nc.vector.tensor_tensor(out=ev[:, ct, sl], in0=ev[:, ct, sl], in1=pm, op=SU)

            ln256(lambda ct: ev[:, ct, :], kn_wr, "kn", bf16_in=True,
                  wb=None if kn_triv else ("knw", "knb", lambda ct: ev[:, ct, :]))

            nc.vector.tensor_tensor(out=sr, in0=sr, in1=ev, op=MM)   # srw

            xcp = [scr.tile([128, 66, 66], BF16, name=f"xcp{i}", tag="scrt")
                   for i in range(4)]
            for t in xcp:
                nc.vector.memset(t[:, 0:1, :], 0.0)
                nc.vector.memset(t[:, 65:66, :], 0.0)
                nc.vector.memset(t[:, 1:65, 0:1], 0.0)
                nc.vector.memset(t[:, 1:65, 65:66], 0.0)

            wot = wstr.tile([128, 2, 256], BF16, name="wot", tag="wst", bufs=2)
            nc.sync.dma_start(out=wot, in_=d_wo[:, :].rearrange("(kt p) m -> p kt m", p=128))
            for mt in range(2):
                for ch in range(NCH):
                    sl = slice(ch * CH, (ch + 1) * CH)
                    h0 = ch * 8
                    ps = psmm.tile([128, CH], F32, name="wo_ps", tag="mm")
                    for kt in range(2):
                        nc.tensor.matmul(out=ps, lhsT=wot[:, kt, 128 * mt:128 * (mt + 1)],
                                         rhs=sr[:, kt, sl], start=(kt == 0), stop=(kt == 1))
                    nc.vector.tensor_tensor(
                        out=xcp[mt][:, 1 + h0:9 + h0, 1:65],
                        in0=xn4[:, mt, h0:h0 + 8, :],
                        in1=ps.rearrange("p (a b) -> p a b", a=8), op=AD)

            spt = wstr.tile([128, 4, 256], F32R, name="spt", tag="wst4", bufs=1)
            nc.sync.dma_start(out=spt,
                              in_=d_sp[:, :].rearrange("(kt p) m -> p kt m", p=128).bitcast(F32R))
            for ch in range(NCH):
                sl = slice(ch * CH, (ch + 1) * CH)
                h0 = ch * 8
                skc = big.tile([128, 4, CH], F32R, name="skc",
                               tag="A" if ch % 2 == 0 else "B")
                nc.sync.dma_start(
                    out=skc,
                    in_=skin[:, sl].rearrange("(kt p) n -> p kt n", p=128).bitcast(F32R))
                for mt in range(2):
                    ps = psmm.tile([128, CH], F32, name="sp_ps", tag="mm")
                    for kt in range(4):
                        nc.tensor.matmul(out=ps, lhsT=spt[:, kt, 128 * mt:128 * (mt + 1)],
                                         rhs=skc[:, kt, :], start=(kt == 0), stop=(kt == 3))
                    nc.scalar.activation(
                        out=xcp[2 + mt][:, 1 + h0:9 + h0, 1:65],
                        in_=ps.rearrange("p (a b) -> p a b", a=8),
                        func=Act.Identity, bias=col('spb', mt))

            if probe:
                pxc = mkprobe("p_xcat", [CS, S])
                for i in range(4):
                    nc.gpsimd.dma_start(
                        out=pxc[128 * i:128 * (i + 1), :].rearrange("p (a b) -> p a b", a=64),
                        in_=xcp[i][:, 1:65, 1:65])

            # ============ S5: grouped 3x3 conv -> gelu -> y1 (bf16) ============
            y1a = big.tile([128, 2, S], BF16, name="y1a", tag="A")
            y1b = big.tile([128, 2, S], BF16, name="y1b", tag="Cxn")
            y1t = [y1a, y1b]
            # prime the wst9 slots so the c1 weight DMAs land after the
            # scan/Wo stages (works around early-SBUF corruption of the
            # first-loaded tiles)
            if not os.environ.get('BASSK_NOPRIME'):
                for i in range(2):
                    pr_ = wstr.tile([128, 1], BF16, name=f"prime{i}", tag="wst9")
                    nc.vector.tensor_copy(out=pr_, in_=xcp[i][:, 0, 0:1])
            if probe and os.environ.get('BASSK_CANARY'):
                cnry = wstr.tile([128, 9, 2, 128], BF16, name="cnry", tag="wst9")
                nc.sync.dma_start(out=cnry, in_=d_c1[1, 0, :, :, :, :])
                marks = [("m0", cnry[:, 0, 0, 0:64]),
                         ("m1", xn[:, 0, 0:64]),
                         ("m2", ev[:, 0, 0:64]),
                         ("m3", sr[:, 0, 0:64])]
                for mi, (mn, mark) in enumerate(marks):
                    stg_c = sml.tile([128, 64], BF16, name=f"cst{mi}",
                                     tag="cst", bufs=4)
                    nc.vector.tensor_tensor(
                        out=stg_c, in0=cnry[:, 0, 0, 0:64],
                        in1=mark, op=Alu.bypass)
                    pc = mkprobe(f"p_cn{mi}", [128, 64])
                    nc.gpsimd.dma_start(out=pc[:, :], in_=stg_c)
            for g in (1, 0):
                for mt in range(2):
                    c1gm = wstr.tile([128, 9, 2, 128], BF16, name="c1gm", tag="wst9")
                    nc.sync.dma_start(out=c1gm, in_=d_c1[g, mt, :, :, :, :])
                    if probe and mt == 0:
                        pw = mkprobe(f"p_c1w_{g}", [128, 9 * 2 * 128])
                        nc.gpsimd.dma_start(out=pw[:, :],
                                            in_=c1gm.rearrange("p a b c -> p (a b c)"))
                    for ch in range(NCH):
                        h0 = ch * 8
                        ps = psmm.tile([128, CH], F32, name="c1_ps", tag="mm")
                        i = 0
                        for ti in range(9):
                            dy, dx = ti // 3 - 1, ti % 3 - 1
                            for kt in range(2):
                                nc.tensor.matmul(
                                    out=ps.rearrange("p (a b) -> p a b", a=8),
                                    lhsT=c1gm[:, ti, kt, :],
                                    rhs=xcp[2 * g + kt][:, 1 + h0 + dy:9 + h0 + dy,
                                                        1 + dx:65 + dx],
                                    start=(i == 0), stop=(i == 17))
                                i += 1
                        if probe and mt == 0 and ch == 0:
                            pps = mkprobe(f"p_c1ps_{g}", [128, CH])
                            stg = sml.tile([128, CH], F32, name="stg", tag="sqc")
                            nc.scalar.copy(out=stg, in_=ps)
                            nc.gpsimd.dma_start(out=pps[:, :], in_=stg)
                        nc.scalar.activation(
                            out=y1t[g][:, mt, ch * CH:(ch + 1) * CH], in_=ps,
                            func=Act.Gelu, bias=col('c1b', 2 * g + mt))

            if probe:
                py1 = mkprobe("p_y1", [CS, S])
                for i in range(4):
                    nc.gpsimd.dma_start(out=py1[128 * i:128 * (i + 1), :],
                                        in_=y1t[i // 2][:, i % 2, :])

            # ============ S6: c2 -> gelu -> c3 -> gelu(+bn3) -> y3 ============
            y3 = [big.tile([128, S], BF16, name="y3_0", tag="Emd"),
                  big.tile([128, S], BF16, name="y3_1", tag="D")]
            for ch in range(NCH):
                sl = slice(ch * CH, (ch + 1) * CH)
                ytiles = []
                for mt in range(8):
                    ps = psmm.tile([128, CH], F32, name="c2_ps", tag="mm")
                    for kt in range(4):
                        nc.tensor.matmul(out=ps, lhsT=c2wt[:, kt, 128 * mt:128 * (mt + 1)],
                                         rhs=y1t[kt // 2][:, kt % 2, sl],
                                         start=(kt == 0), stop=(kt == 3))
                    yt = y2b.tile([128, CH], BF16, name="y2t", tag="y2t")
                    nc.scalar.activation(out=yt, in_=ps, func=Act.Gelu, bias=col('c2b', mt))
                    ytiles.append(yt)
                for mt in range(2):
                    ps = psmm.tile([128, CH], F32, name="c3_ps", tag="mm")
                    for kt in range(8):
                        nc.tensor.matmul(out=ps, lhsT=c3wt[:, kt, 128 * mt:128 * (mt + 1)],
                                         rhs=ytiles[kt], start=(kt == 0), stop=(kt == 7))
                    nc.scalar.activation(out=y3[mt][:, sl], in_=ps, func=Act.Gelu,
                                         bias=col('c3b', mt))
                    if not bn3_triv:
                        nc.vector.tensor_scalar(out=y3[mt][:, sl],
                                                in0=y3[mt][:, sl],
                                                scalar1=col('g3p', mt),
                                                scalar2=col('b3p', mt), op0=MM, op1=AD)

            if probe:
                py3 = mkprobe("p_y3", [C, S])
                for i in range(2):
                    stg3 = sml.tile([128, S], F32, name=f"stg3_{i}", tag="stg3")
                    nc.vector.tensor_copy(out=stg3, in_=y3[i][:, :])
                    nc.gpsimd.dma_start(out=py3[128 * i:128 * (i + 1), :], in_=stg3)

            # ============ S7: LN2, up-proj, pixel-shuffle DMA out ============
            def ln2_wr(ct, sl, pr, pm):
                nc.vector.tensor_tensor(out=y3[ct][:, sl], in0=y3[ct][:, sl],
                                        in1=pr, op=MM)
                nc.vector.tensor_tensor(out=y3[ct][:, sl], in0=y3[ct][:, sl],
                                        in1=pm, op=SU)

            ln256(lambda ct: y3[ct][:, :], ln2_wr, "ln2", bf16_in=True,
                  wb=None if ln2_triv else ("ln2w", "ln2b", lambda ct: y3[ct][:, :]))

            upt = wstr.tile([128, 2, 512], BF16, name="upt", tag="wst4", bufs=1)
            nc.sync.dma_start(out=upt,
                              in_=d_upb[:, :].rearrange("(kt p) m -> p kt m", p=128))
            for r in range(2):
                for ch in range(NCH):
                    sl = slice(ch * CH, (ch + 1) * CH)
                    h0 = ch * 8
                    ub = yupp.tile([128, 8, 64, 2], F32, name="ub", tag="ub")
                    for q in range(2):
                        rq = 2 * r + q
                        ps = psmm.tile([128, CH], F32, name="up_ps", tag="mm")
                        for kt in range(2):
                            nc.tensor.matmul(out=ps,
                                             lhsT=upt[:, kt, 128 * rq:128 * (rq + 1)],
                                             rhs=y3[kt][:, sl],
                                             start=(kt == 0), stop=(kt == 1))
                        nc.scalar.activation(out=ub[:, :, :, q],
                                             in_=ps.rearrange("p (a b) -> p a b", a=8),
                                             func=Act.Identity, bias=col('upb', rq))
                    dst = view(yout[:, :, :], [[128 * 128, 128], [256, 8], [1, 128]],
                               off=(2 * h0 + r) * 128)
                    nc.sync.dma_start(out=dst, in_=ub.rearrange("p a b q -> p a (b q)"))

    nc.compile()
    return nc, const_inputs


def _get_nc(weights, probe=False):
    import hashlib
    hsh = hashlib.sha1()
    for k in sorted(weights):
        hsh.update(k.encode())
        hsh.update(np.ascontiguousarray(weights[k]).tobytes())
    key = (hsh.hexdigest(), probe)
    if key not in _CACHE:
        _CACHE[key] = _build(weights, probe=probe)
    return _CACHE[key]


def kernel(**inputs):
    from concourse.bass_utils import run_bass_kernel_spmd

    x = np.asarray(inputs['x'], np.float32)
    skip = np.asarray(inputs['skip'], np.float32)
    mask = np.asarray(inputs['saliency_mask'], np.float32)
    weights = {k: np.asarray(v, np.float32) for k, v in inputs.items()
               if k not in ('x', 'skip', 'saliency_mask')}

    probe = bool(os.environ.get('BASSK_PROBE'))
    nc, const_inputs = _get_nc(weights, probe=probe)

    in_maps = []
    for b in range(B):
        m = dict(
            xin=np.ascontiguousarray(x[b].reshape(C, S)),
            skin=np.ascontiguousarray(skip[b].reshape(CS, S)),
            mrow=np.ascontiguousarray(mask[b].reshape(1, S)),
        )
        m.update(const_inputs)
        in_maps.append(m)
    res = run_bass_kernel_spmd(nc, in_maps, core_ids=list(range(B)),
                               trace=bool(os.environ.get('BASSK_TRACE')))
    kernel.last_results = res
    out = np.stack([res.results[b]['yout'] for b in range(B)], axis=0)
    return out



# revision 31
# speedup vs baseline: 1.2288x; 1.0928x over previous
"""Trainium2 Bass kernel for nn_DecoderBlock (shape-guided RWKV decoder block).

Data-parallel over batch: B=8 samples -> 8 NeuronCores, one NEFF.

Per-core layout: channels on partitions (256ch -> 2 "ctiles" of 128), spatial
(h, w) flattened on the free dim (4096).

- LayerNorm over channels: square (ACT) -> ones-matmul partition reduction ->
  DRAM-bounce reshape -> tiny stat math -> K=1 matmul broadcast -> TT applies.
- q_shift / mask blend via shifted access patterns; per-channel mixes folded
  into Wk/Wv/Wr host-side (k = Wk@xn + (Wk*diag(1-mk))@md, md = mask*(xs-xn)).
- WKV: unstabilized linear recurrence A_t = lam*A_{t-1} + e^{k_t} v_t via the
  DVE TensorTensorScan instruction chained across rows (data0=0 resets at each
  row start); vertical orientation scans read E/EV through transposed APs.
- channel_fusion: grouped 3x3 conv = 9 shifted-AP matmuls accumulated in PSUM
  over zero-padded [c, 66, 66] inputs; BN folded into the next conv
  host-side; GELU+bias fused into the PSUM->SBUF evacuation on ACT.
- patch_expand: up-proj rows permuted host-side so pixel shuffle becomes a
  strided DMA.

Matmuls in float32r (full rate) or bf16; bulky intermediates bf16.
"""
import sys
import os

for _p in ('/opt/trn_rl_repo', '/root/.axon_site/_ro/trn_rl_repo'):
    if _p not in sys.path and os.path.isdir(_p):
        sys.path.append(_p)

import numpy as np

B, C, CS, COUT, H, W = 8, 256, 512, 128, 64, 64
S = H * W          # 4096
NCH = 8            # spatial chunks
CH = S // NCH      # 512
EPS = 1e-5

_CACHE = {}


def _build(weights, probe=False):
    const_inputs = {}
    import concourse.bass as bass
    from concourse import bacc
    import concourse.tile as tile
    import concourse.mybir as mybir
    import ml_dtypes

    F32 = mybir.dt.float32
    F32R = mybir.dt.float32r
    BF16 = mybir.dt.bfloat16
    Alu = mybir.AluOpType
    Act = mybir.ActivationFunctionType
    MM, AD, SU = Alu.mult, Alu.add, Alu.subtract

    w = weights
    f64 = lambda x: np.asarray(x, np.float64)
    bf = lambda a: np.asarray(a, dtype=ml_dtypes.bfloat16)

    # ---------------- host-side folding
    bnscale = 1.0 / np.sqrt(1.0 + EPS)
    g1p = f64(w['bn1_g']) * bnscale
    b1p = f64(w['bn1_b'])
    g2p = f64(w['bn2_g']) * bnscale
    b2p = f64(w['bn2_b'])
    g3p = (f64(w['bn3_g']) * bnscale).astype(np.float32)
    b3p = f64(w['bn3_b']).astype(np.float32)

    c2_eff = f64(w['c2_w']) * g1p[None, :]
    c2b_eff = (f64(w['c2_b']) + f64(w['c2_w']) @ b1p).astype(np.float32)
    c3_eff = f64(w['c3_w']) * g2p[None, :]
    c3b_eff = (f64(w['c3_b']) + f64(w['c3_w']) @ b2p).astype(np.float32)

    wk_x = f64(w['Wk']).T
    wk_d = (f64(w['Wk']) * (1.0 - f64(w['mix_k']))[None, :]).T
    wv_x = f64(w['Wv']).T
    wv_d = (f64(w['Wv']) * (1.0 - f64(w['mix_v']))[None, :]).T
    wr_x = f64(w['Wr']).T
    wr_d = (f64(w['Wr']) * (1.0 - f64(w['mix_r']))[None, :]).T
    wo_t = f64(w['Wo']).T
    sp_t = f64(w['sp_w']).T.astype(np.float32)

    lam = np.exp(-np.exp(f64(w['decay']))).astype(np.float32)
    lam64 = np.tile(lam[:, None], (1, 64))
    lam64[:, 0] = 0.0
    lam64 = lam64.astype(np.float32)
    eu = np.exp(f64(w['first'])).astype(np.float32)

    pidx = np.arange(512)
    old = (pidx % 128) * 4 + (pidx // 128)
    up_t = f64(w['up_w'])[old].T.astype(np.float32)             # [256, 512]
    up_tb = bf(up_t)
    upb_p = f64(w['up_b'])[old].astype(np.float32)

    c1w = f64(w['c1_w'])
    c1_l = np.zeros((9, 2, 256, 256), np.float32)
    for ti in range(9):
        dy, dx = ti // 3, ti % 3
        for g in range(2):
            c1_l[ti, g] = c1w[g * 256:(g + 1) * 256, :, dy, dx].T

    # per-channel vectors as columns of one [128, ncol] const
    cols, order = {}, []

    def addcol(name, vec):
        v = np.asarray(vec, np.float32).reshape(-1, 128)
        cols[name] = v
        order.append(name)

    addcol('eu', eu)
    addcol('ln1w', w['ln1_w'])
    addcol('ln1b', w['ln1_b'])
    addcol('knw', w['kn_w'])
    addcol('knb', w['kn_b'])
    addcol('ln2w', w['ln2_w'])
    addcol('ln2b', w['ln2_b'])
    addcol('g3p', g3p)
    addcol('b3p', b3p)
    addcol('spb', w['sp_b'])
    addcol('c3b', c3b_eff)
    addcol('c1b', w['c1_b'])
    addcol('c2b', c2b_eff)
    addcol('upb', upb_p)
    colidx, ncol = {}, 0
    for n in order:
        colidx[n] = ncol
        ncol += cols[n].shape[0]
    cvec_np = np.zeros((128, ncol), np.float32)
    for n in order:
        for i in range(cols[n].shape[0]):
            cvec_np[:, colidx[n] + i] = cols[n][i]

    ln1_triv = np.all(w['ln1_w'] == 1.0) and np.all(w['ln1_b'] == 0.0)
    kn_triv = np.all(w['kn_w'] == 1.0) and np.all(w['kn_b'] == 0.0)
    ln2_triv = np.all(w['ln2_w'] == 1.0) and np.all(w['ln2_b'] == 0.0)
    bn3_triv = np.all(g3p == g3p[0]) and np.all(b3p == 0.0)
    # uniform bn3 scale commutes with LN2 -> drop it entirely when trivial

    # ---------------- bass module
    nc = bacc.Bacc("TRN2", target_bir_lowering=False, debug=False, name="decblk")

    xin = nc.dram_tensor("xin", [C, S], F32, kind="ExternalInput")
    skin = nc.dram_tensor("skin", [CS, S], F32, kind="ExternalInput")
    mrow = nc.dram_tensor("mrow", [1, S], F32, kind="ExternalInput")
    yout = nc.dram_tensor("yout", [COUT, 2 * H, 2 * W], BF16, kind="ExternalOutput")
    probes = {}

    def mkprobe(name, shape):
        if probe:
            probes[name] = nc.dram_tensor(name, shape, F32, kind="ExternalOutput")
        return probes.get(name)

    def it(arr, name):
        arr = np.ascontiguousarray(arr)
        import ml_dtypes as _md
        dt_ = {np.dtype(np.float32): F32, np.dtype(_md.bfloat16): BF16}[arr.dtype]
        const_inputs[name] = arr
        return nc.dram_tensor(name, list(arr.shape), dt_, kind="ExternalInput")
    d_lam = it(lam64, "lam64")
    d_cvec = it(cvec_np, "cvec")
    d_wk = [it(bf(wk_x), "wkx"), it(bf(wk_d), "wkd")]
    d_wv = [it(bf(wv_x), "wvx"), it(bf(wv_d), "wvd")]
    d_wr = [it(bf(wr_x), "wrx"), it(bf(wr_d), "wrd")]
    d_wo = it(bf(wo_t), "wo")
    d_sp = it(sp_t, "sp")
    d_upb = it(up_tb, "up")
    c1_r = c1_l.reshape(9, 2, 2, 128, 2, 128).transpose(1, 4, 3, 0, 2, 5)
    d_c1 = it(bf(c1_r), "c1")   # [g, mt, p, t, kt, m]
    d_c2 = it(bf(c2_eff.T), "c2")
    d_c3 = it(bf(c3_eff.T), "c3")
    red_np = np.zeros((128, 2, 2), np.float32)
    red_np[:, 0, 0] = 1.0
    red_np[:, 1, 1] = 1.0
    d_red = it(red_np, "red")
    d_redb = it(bf(red_np), "redb")
    bc8_np = np.zeros((8, 8, 128), np.float32)
    for _v in range(8):
        bc8_np[_v, _v, :] = 1.0
    d_bc8 = it(bc8_np, "bc8")
    d_eps = it(np.full((128, 1), EPS, np.float32), "epsc")
    d_i2 = it(np.eye(2, dtype=np.float32), "i2c")
    d_i128 = it(np.eye(128, dtype=np.float32), "i128c")

    def scan_raw(out, d0, d1, eng=None):
        eng = eng or nc.vector
        if os.environ.get('BASSK_NOSCAN'):
            return eng.tensor_copy(out=out, in_=d1)
        return eng.add_instruction(mybir.InstTensorScalarPtr(
            name=nc.get_next_instruction_name(),
            is_tensor_tensor_scan=True,
            is_scalar_tensor_tensor=True,
            op0=MM, op1=AD,
            ins=[eng.lower_ap(d0), eng.lower_ap_or_imm(0.0), eng.lower_ap(d1)],
            outs=[eng.lower_ap(out)],
        ))

    def recip(out, in_):
        if os.environ.get('BASSK_SLOWRECIP'):
            return nc.vector.reciprocal(out=out, in_=in_)
        return nc.vector.reciprocal_approx_fast(out=out, in_=in_)

    def view(ap, dims, off=0):
        return bass.AP(tensor=ap.tensor, offset=ap.offset + off, ap=dims)

    with tile.TileContext(nc) as tc:
        with tc.tile_pool(name="big", bufs=1) as big, \
             tc.tile_pool(name="wres", bufs=1) as wres, \
             tc.tile_pool(name="scr", bufs=4) as scr, \
             tc.tile_pool(name="sml", bufs=3) as sml, \
             tc.tile_pool(name="y2b", bufs=8) as y2b, \
             tc.tile_pool(name="yupp", bufs=2) as yupp, \
             tc.tile_pool(name="xcs", bufs=2) as xcs, \
             tc.tile_pool(name="wstr", bufs=2) as wstr, \
             tc.tile_pool(name="psmm", bufs=3, space="PSUM") as psmm, \
             tc.tile_pool(name="psst", bufs=1, space="PSUM") as psst, \
             tc.tile_pool(name="psln", bufs=1, space="PSUM") as psln, \
             tc.tile_pool(name="psbc", bufs=1, space="PSUM") as psbc:

            # ---- resident constants
            lt = wres.tile([128, 2, 64], F32, name="lt")
            nc.sync.dma_start(out=lt, in_=d_lam[:, :].rearrange("(t p) j -> p t j", p=128))
            cv = wres.tile([128, ncol], F32, name="cv")
            nc.sync.dma_start(out=cv, in_=d_cvec[:, :])
            red = wres.tile([128, 2, 2], F32R, name="red")
            nc.sync.dma_start(out=red, in_=d_red[:, :, :].bitcast(F32R))
            redb = wres.tile([128, 2, 2], BF16, name="redb")
            nc.sync.dma_start(out=redb, in_=d_redb[:, :, :])
            bc8 = wres.tile([8, 8, 128], F32R, name="bc8")
            nc.sync.dma_start(out=bc8, in_=d_bc8[:, :, :].bitcast(F32R))
            epsc = wres.tile([128, 1], F32, name="epsc")
            nc.sync.dma_start(out=epsc, in_=d_eps[:, :])
            i2c = wres.tile([2, 2], F32R, name="i2c")
            nc.sync.dma_start(out=i2c, in_=d_i2[:, :].bitcast(F32R))
            i128c = wres.tile([128, 128], F32R, name="i128c")
            nc.sync.dma_start(out=i128c, in_=d_i128[:, :].bitcast(F32R))
            c2wt = wres.tile([128, 4, 1024], BF16, name="c2wt")
            nc.sync.dma_start(out=c2wt, in_=d_c2[:, :].rearrange("(kt p) m -> p kt m", p=128))

            def col(name, i=0):
                return cv[:, colidx[name] + i:colidx[name] + i + 1]

            # ============ LayerNorm over channels (2 ctiles) ============
            # On-chip stats path: ones-matmul partition reduction -> PE
            # transpose to pixel-major [128, 32, 2] -> tiny stat math ->
            # PE transpose back to [2, S] -> K=2 broadcast matmuls.
            def ln256(Xr, out_wr, name, wb=None, bf16_in=False):
                redm = redb if bf16_in else red
                pst = psln.tile([128, 32, 2], F32, name=f"pst_{name}", tag="pst")
                for ch in range(NCH):
                    sl = slice(ch * CH, (ch + 1) * CH)
                    ps = psst.tile([2, CH], F32, name=f"lnps_{name}", tag="st")
                    for ct in range(2):
                        nc.tensor.matmul(out=ps, lhsT=redm[:, 0, :], rhs=Xr(ct)[:, sl],
                                         start=(ct == 0), stop=False)
                    for ct in range(2):
                        sq = sml.tile([128, CH], BF16,
                                      name=f"sq_{name}", tag="sqc", bufs=3)
                        nc.scalar.activation(
                            out=sq,
                            in_=Xr(ct)[:, sl] if bf16_in else Xr(ct)[:, sl].bitcast(F32),
                            func=Act.Square)
                        nc.tensor.matmul(out=ps, lhsT=redb[:, 1, :], rhs=sq,
                                         start=False, stop=(ct == 1))
                    stc = sml.tile([2, CH], F32R, name=f"stc_{name}", tag="stc", bufs=1)
                    nc.scalar.copy(out=stc, in_=ps)
                    for j in range(4):
                        nc.tensor.matmul(
                            out=pst[:, ch * 4 + j, :],
                            lhsT=stc[:, j * 128:(j + 1) * 128],
                            rhs=i2c[:, :])
                # small stat math in [128, 32, 2] pixel-major layout
                smb = sml.tile([128, 32, 2], F32, name=f"smb_{name}", tag="sm", bufs=1)
                nc.vector.tensor_scalar_mul(out=smb, in0=pst, scalar1=1.0 / C)
                t2 = sml.tile([128, 32], F32, name=f"t2_{name}", tag="t2", bufs=1)
                nc.vector.tensor_tensor(out=t2, in0=smb[:, :, 0], in1=smb[:, :, 0], op=MM)
                nc.vector.tensor_tensor(out=t2, in0=smb[:, :, 1], in1=t2, op=SU)
                nc.scalar.activation(out=t2, in_=t2, func=Act.Sqrt, bias=epsc)
                # stats scattered into one-hot columns (col = 2*sub + q) so
                # the PE transpose lands rows 0-7 partition-aligned; [8, S/4]
                # bcr is 4x narrower in SBUF than a [2, S] row pair.
                sm8 = sml.tile([128, 32, 8], F32R, name=f"sm8_{name}", tag="sm2", bufs=1)
                s8v = sm8.rearrange("p a b -> p (a b)")
                with nc.allow_low_precision(reason="zeroing f32r one-hot scatter"):
                    nc.vector.tensor_scalar_mul(
                        out=s8v, in0=view(epsc[:, :], [epsc[:, :].ap[0], [0, 256]]),
                        scalar1=0.0)

                def s8view(q):
                    return view(s8v[:, :], [s8v.ap[0], [66, 4], [8, 8]], off=q)

                with nc.allow_low_precision(reason="rstd in f32r for PE transpose"):
                    nc.vector.reciprocal(out=s8view(0), in_=t2)            # rstd
                nc.vector.tensor_tensor(out=s8view(1), in0=smb[:, :, 0],
                                        in1=s8view(0).bitcast(F32), op=MM)  # mu*rstd
                bcr = big.tile([8, S // 4], F32R, name=f"bcr_{name}", tag="rowsbig")
                for hoff in range(2):
                    psb8 = psst.tile([8, CH], F32, name=f"psb_{name}", tag="st")
                    for j in range(4):
                        for sub in range(4):
                            nc.tensor.matmul(
                                out=psb8[:, j * 128:(j + 1) * 128],
                                lhsT=sm8[:, (sub * 2 + hoff) * 4 + j, :],
                                rhs=i128c[:, :],
                                start=(sub == 0), stop=(sub == 3))
                    nc.scalar.copy(out=bcr[:, hoff * CH:(hoff + 1) * CH], in_=psb8)
                for ch in range(NCH):
                    sl = slice(ch * CH, (ch + 1) * CH)
                    sub, hoff = ch // 2, ch % 2
                    pr = psbc.tile([128, CH], F32, name=f"pr_{name}", tag="pr")
                    pm = psbc.tile([128, CH], F32, name=f"pm_{name}", tag="pm")
                    nc.tensor.matmul(out=pr, lhsT=bc8[:, 2 * sub + 0, :],
                                     rhs=bcr[:, hoff * CH:(hoff + 1) * CH])
                    nc.tensor.matmul(out=pm, lhsT=bc8[:, 2 * sub + 1, :],
                                     rhs=bcr[:, hoff * CH:(hoff + 1) * CH])
                    for ct in range(2):
                        out_wr(ct, sl, pr, pm)
                if wb is not None:
                    wn, bn_, apfn = wb
                    for ct in range(2):
                        ap = apfn(ct)
                        nc.vector.tensor_scalar(
                            out=ap, in0=ap, scalar1=col(wn, ct), scalar2=col(bn_, ct),
                            op0=MM, op1=AD)

            # ============ S0/S1: load x, LN1 -> xn (bf16) ============
            x0 = big.tile([128, S], F32R, name="x0", tag="A")
            x1 = big.tile([128, S], F32R, name="x1", tag="B")
            nc.sync.dma_start(out=x0, in_=xin[0:128, :].bitcast(F32R))
            nc.sync.dma_start(out=x1, in_=xin[128:256, :].bitcast(F32R))
            mf = big.tile([128, S], F32, name="mf", tag="D")
            nc.sync.dma_start(out=mf, in_=view(mrow[:, :], [[0, 128], [1, S]]))

            xn = big.tile([128, 2, S], BF16, name="xn", tag="Cxn")
            xt = [x0, x1]

            def ln1_wr(ct, sl, pr, pm):
                nc.vector.tensor_tensor(out=xn[:, ct, sl], in0=xt[ct][:, sl].bitcast(F32),
                                        in1=pr, op=MM)
                nc.vector.tensor_tensor(out=xn[:, ct, sl], in0=xn[:, ct, sl],
                                        in1=pm, op=SU)

            ln256(lambda ct: xt[ct][:, :], ln1_wr, "ln1",
                  wb=None if ln1_triv else ("ln1w", "ln1b", lambda ct: xn[:, ct, :]))
            if probe:
                pxn = mkprobe("p_xn", [C, S])
                for ct in range(2):
                    nc.gpsimd.dma_start(out=pxn[128 * ct:128 * (ct + 1), :],
                                        in_=xn[:, ct, :])

            # ============ S2: q_shift diff * mask -> md (bf16) ============
            xn4 = xn.rearrange("p t (h w) -> p t h w", h=H)
            md = big.tile([128, 2, H, W], BF16, name="md", tag="Emd")
            nc.vector.tensor_tensor(out=md[0:64, 0, :, 1:], in0=xn4[0:64, 0, :, 0:63],
                                    in1=xn4[0:64, 0, :, 1:], op=SU)
            nc.vector.tensor_scalar_mul(out=md[0:64, 0, :, 0:1],
                                        in0=xn4[0:64, 0, :, 0:1], scalar1=-1.0)
            nc.vector.tensor_tensor(out=md[64:128, 0, :, 0:63], in0=xn4[64:128, 0, :, 1:],
                                    in1=xn4[64:128, 0, :, 0:63], op=SU)
            nc.vector.tensor_scalar_mul(out=md[64:128, 0, :, 63:64],
                                        in0=xn4[64:128, 0, :, 63:64], scalar1=-1.0)
            nc.vector.tensor_tensor(out=md[0:64, 1, 1:, :], in0=xn4[0:64, 1, 0:63, :],
                                    in1=xn4[0:64, 1, 1:, :], op=SU)
            nc.vector.tensor_scalar_mul(out=md[0:64, 1, 0:1, :],
                                        in0=xn4[0:64, 1, 0:1, :], scalar1=-1.0)
            nc.vector.tensor_tensor(out=md[64:128, 1, 0:63, :], in0=xn4[64:128, 1, 1:, :],
                                    in1=xn4[64:128, 1, 0:63, :], op=SU)
            nc.vector.tensor_scalar_mul(out=md[64:128, 1, 63:64, :],
                                        in0=xn4[64:128, 1, 63:64, :], scalar1=-1.0)
            mdf = md.rearrange("p t h w -> p t (h w)")
            for ct in range(2):
                nc.vector.tensor_tensor(out=mdf[:, ct, :], in0=mdf[:, ct, :],
                                        in1=mf, op=MM)

            # ============ S3: k/v/r matmuls -> E, V, SR; scans ============
            ev = big.tile([128, 2, S], BF16, name="ev", tag="B")
            et = big.tile([128, 2, S], BF16, name="et", tag="A")
            vv = big.tile([128, 2, S], BF16, name="vv", tag="D")
            sr = big.tile([128, 2, S], BF16, name="sr", tag="Fsr")

            def kvloop(dws, evac):
                wxt = wstr.tile([128, 2, 256], BF16, name="wxt", tag="wst", bufs=2)
                wdt = wstr.tile([128, 2, 256], BF16, name="wdt", tag="wst", bufs=2)
                nc.sync.dma_start(out=wxt, in_=dws[0][:, :].rearrange("(kt p) m -> p kt m", p=128))
                nc.sync.dma_start(out=wdt, in_=dws[1][:, :].rearrange("(kt p) m -> p kt m", p=128))
                for mt in range(2):
                    for ch in range(NCH):
                        sl = slice(ch * CH, (ch + 1) * CH)
                        ps = psmm.tile([128, CH], F32, name="kv_ps", tag="mm")
                        for kt in range(2):
                            nc.tensor.matmul(out=ps, lhsT=wxt[:, kt, 128 * mt:128 * (mt + 1)],
                                             rhs=xn[:, kt, sl], start=(kt == 0), stop=False)
                        for kt in range(2):
                            nc.tensor.matmul(out=ps, lhsT=wdt[:, kt, 128 * mt:128 * (mt + 1)],
                                             rhs=mdf[:, kt, sl], start=False, stop=(kt == 1))
                        evac(mt, sl, ps)

            kvloop(d_wk, lambda mt, sl, ps: nc.scalar.activation(
                out=et[:, mt, sl], in_=ps, func=Act.Exp))
            kvloop(d_wv, lambda mt, sl, ps: nc.scalar.copy(out=vv[:, mt, sl], in_=ps))

            nc.vector.tensor_tensor(out=ev, in0=et, in1=vv, op=MM)

            ev4 = ev.rearrange("p t (h w) -> p t h w", h=H)
            et4 = et.rearrange("p t (h w) -> p t h w", h=H)

            # transposed copies for the vertical orientation (ACT engine):
            # etT/evT[p, ct, w, h] = et/ev[p, ct, h, w]
            etT = big.tile([128, 2, W, H], BF16, name="etT", tag="D")
            evT = big.tile([128, 2, W, H], BF16, name="evT", tag="Emd")
            nc.scalar.copy(out=etT,
                           in_=view(et[:, :, :], [et.ap[0], [S, 2], [1, W], [W, H]]))
            nc.scalar.copy(out=evT,
                           in_=view(ev[:, :, :], [ev.ap[0], [S, 2], [1, W], [W, H]]))

            kvloop(d_wr, lambda mt, sl, ps: nc.scalar.activation(
                out=sr[:, mt, sl], in_=ps, func=Act.Sigmoid))

            # prime the wst9 slots early so c1 weight DMAs can land during scans
            if not os.environ.get('BASSK_NOPRIME'):
                for i in range(2):
                    pr_ = wstr.tile([128, 1], BF16, name=f"prime{i}", tag="wst9")
                    nc.vector.tensor_copy(out=pr_, in_=xn[:, 0, 0:1])

            # ---- early prefetch + skip-feature GEMM (overlaps scan phase) ----
            wot = wstr.tile([128, 2, 256], BF16, name="wot", tag="wst", bufs=2)
            nc.sync.dma_start(out=wot, in_=d_wo[:, :].rearrange("(kt p) m -> p kt m", p=128))
            spt = wstr.tile([128, 4, 256], F32R, name="spt", tag="wst4", bufs=1)
            nc.sync.dma_start(out=spt,
                              in_=d_sp[:, :].rearrange("(kt p) m -> p kt m", p=128).bitcast(F32R))
            xcp23 = [xcs.tile([128, 66, 66], BF16, name=f"xcs{i}", tag="xcs")
                     for i in range(2)]
            for t in xcp23:
                nc.vector.memset(t[:, 0:1, :], 0.0)
                nc.vector.memset(t[:, 65:66, :], 0.0)
                nc.vector.memset(t[:, 1:65, 0:1], 0.0)
                nc.vector.memset(t[:, 1:65, 65:66], 0.0)
            CQ = CH // 2
            for ch in range(2 * NCH):
                h0 = ch * 4
                sl = slice(ch * CQ, (ch + 1) * CQ)
                skc = sml.tile([128, 4, CQ], F32R, name="skc", tag="skcf", bufs=2)
                nc.sync.dma_start(
                    out=skc,
                    in_=skin[:, sl].rearrange("(kt p) n -> p kt n", p=128).bitcast(F32R))
                for mt in range(2):
                    ps = psmm.tile([128, CQ], F32, name="sp_ps", tag="mm")
                    for kt in range(4):
                        nc.tensor.matmul(out=ps, lhsT=spt[:, kt, 128 * mt:128 * (mt + 1)],
                                         rhs=skc[:, kt, :], start=(kt == 0), stop=(kt == 3))
                    nc.scalar.activation(
                        out=xcp23[mt][:, 1 + h0:5 + h0, 1:65],
                        in_=ps.rearrange("p (a b) -> p a b", a=4),
                        func=Act.Identity, bias=col('spb', mt))

            # ---- WKV scans: zero-padded 66-wide scan outputs keep every
            # consumer read packed + 4B-aligned (2x DVE mode); vertical
            # orientation runs on the transposed copies so it is packed too.
            lt_ap = lt[:, :, :]

            def lamview(ct, nseq):
                return view(lt_ap, [lt_ap.ap[0], [0, nseq], [1, 64]], off=ct * 64)

            pool_flag = os.environ.get('BASSK_POOLSCAN', '')

            def orient(e4v, v4v, scan_eng):
                for half in range(2):
                    rows = slice(half * 32, (half + 1) * 32)
                    a = scr.tile([128, 2, 32, 66], BF16, name="a_sc", tag="scrt")
                    b = scr.tile([128, 2, 32, 66], BF16, name="b_sc", tag="scrt")
                    nc.vector.memset(a[:, :, :, 0:1], 0.0)
                    nc.vector.memset(b[:, :, :, 0:1], 0.0)
                    for ct in range(2):
                        scan_raw(a[:, ct, :, 1:65], lamview(ct, 32),
                                 v4v[:, ct, rows, :], eng=scan_eng)
                        scan_raw(b[:, ct, :, 1:65], lamview(ct, 32),
                                 e4v[:, ct, rows, :], eng=scan_eng)
                    for ct in range(2):
                        den = scr.tile([128, 32, 64], F32, name="den", tag="scrt")
                        nc.vector.scalar_tensor_tensor(
                            out=den, in0=e4v[:, ct, rows, :], scalar=col('eu', ct),
                            in1=b[:, ct, :, 0:64], op0=MM, op1=AD)
                        recip(out=den, in_=den)
                        rdb = sml.tile([128, 32, 64], BF16, name="rdb", tag="stc", bufs=1)
                        nc.scalar.copy(out=rdb, in_=den)
                        nc.vector.scalar_tensor_tensor(
                            out=v4v[:, ct, rows, :], in0=v4v[:, ct, rows, :],
                            scalar=col('eu', ct), in1=a[:, ct, :, 0:64], op0=MM, op1=AD)
                        nc.vector.tensor_tensor(out=v4v[:, ct, rows, :],
                                                in0=v4v[:, ct, rows, :], in1=rdb, op=MM)

            orient(et4, ev4, nc.gpsimd if pool_flag == 'a' else nc.vector)
            orient(etT, evT, nc.gpsimd if pool_flag in ('a', 'v') else nc.vector)

            # wkv = out_h + out_v^T, per h-half so kn stats can start early
            for half in range(2):
                hr = slice(half * 32, (half + 1) * 32)
                for ct in range(2):
                    ovT = view(evT[:, :, :, :], [evT.ap[0], [1, 32], [64, 64]],
                               off=ct * S + half * 32)
                    nc.vector.tensor_tensor(out=ev4[:, ct, hr, :], in0=ev4[:, ct, hr, :],
                                            in1=ovT, op=AD)
            if probe:
                pwkv = mkprobe("p_wkv", [C, S])
                for ct in range(2):
                    nc.gpsimd.dma_start(out=pwkv[128 * ct:128 * (ct + 1), :],
                                        in_=ev[:, ct, :])

            # ============ S4: key-LN, srw, Wo+residual, skip feat ============
            def kn_wr(ct, sl, pr, pm):
                nc.vector.tensor_tensor(out=ev[:, ct, sl], in0=ev[:, ct, sl], in1=pr, op=MM)
                nc.vector.tensor_tensor(out=ev[:, ct, sl], in0=ev[:, ct, sl], in1=pm, op=SU)

            ln256(lambda ct: ev[:, ct, :], kn_wr, "kn", bf16_in=True,
                  wb=None if kn_triv else ("knw", "knb", lambda ct: ev[:, ct, :]))

            nc.vector.tensor_tensor(out=sr, in0=sr, in1=ev, op=MM)   # srw

            xcp = [scr.tile([128, 66, 66], BF16, name=f"xcp{i}", tag="scrt")
                   for i in range(2)] + xcp23
            for t in xcp[:2]:
                nc.vector.memset(t[:, 0:1, :], 0.0)
                nc.vector.memset(t[:, 65:66, :], 0.0)
                nc.vector.memset(t[:, 1:65, 0:1], 0.0)
                nc.vector.memset(t[:, 1:65, 65:66], 0.0)

            for mt in range(2):
                for ch in range(NCH):
                    sl = slice(ch * CH, (ch + 1) * CH)
                    h0 = ch * 8
                    ps = psmm.tile([128, CH], F32, name="wo_ps", tag="mm")
                    for kt in range(2):
                        nc.tensor.matmul(out=ps, lhsT=wot[:, kt, 128 * mt:128 * (mt + 1)],
                                         rhs=sr[:, kt, sl], start=(kt == 0), stop=(kt == 1))
                    nc.vector.tensor_tensor(
                        out=xcp[mt][:, 1 + h0:9 + h0, 1:65],
                        in0=xn4[:, mt, h0:h0 + 8, :],
                        in1=ps.rearrange("p (a b) -> p a b", a=8), op=AD)

            if probe:
                pxc = mkprobe("p_xcat", [CS, S])
                for i in range(4):
                    nc.gpsimd.dma_start(
                        out=pxc[128 * i:128 * (i + 1), :].rearrange("p (a b) -> p a b", a=64),
                        in_=xcp[i][:, 1:65, 1:65])

            # ============ S5: grouped 3x3 conv -> gelu -> y1 (bf16) ============
            y1a = big.tile([128, 2, S], BF16, name="y1a", tag="A")
            y1b = big.tile([128, 2, S], BF16, name="y1b", tag="Cxn")
            y1t = [y1a, y1b]
            # prime the wst9 slots so the c1 weight DMAs land after the
            # scan/Wo stages (works around early-SBUF corruption of the
            # first-loaded tiles)
            if not os.environ.get('BASSK_NOPRIME'):
                for i in range(2):
                    pr_ = wstr.tile([128, 1], BF16, name=f"prime{i}", tag="wst9")
                    nc.vector.tensor_copy(out=pr_, in_=xcp[i][:, 0, 0:1])
            if probe and os.environ.get('BASSK_CANARY'):
                cnry = wstr.tile([128, 9, 2, 128], BF16, name="cnry", tag="wst9")
                nc.sync.dma_start(out=cnry, in_=d_c1[1, 0, :, :, :, :])
                marks = [("m0", cnry[:, 0, 0, 0:64]),
                         ("m1", xn[:, 0, 0:64]),
                         ("m2", ev[:, 0, 0:64]),
                         ("m3", sr[:, 0, 0:64])]
                for mi, (mn, mark) in enumerate(marks):
                    stg_c = sml.tile([128, 64], BF16, name=f"cst{mi}",
                                     tag="cst", bufs=4)
                    nc.vector.tensor_tensor(
                        out=stg_c, in0=cnry[:, 0, 0, 0:64],
                        in1=mark, op=Alu.bypass)
                    pc = mkprobe(f"p_cn{mi}", [128, 64])
                    nc.gpsimd.dma_start(out=pc[:, :], in_=stg_c)
            for g in (1, 0):
                for mt in range(2):
                    c1gm = wstr.tile([128, 9, 2, 128], BF16, name="c1gm", tag="wst9")
                    nc.sync.dma_start(out=c1gm, in_=d_c1[g, mt, :, :, :, :])
                    if probe and mt == 0:
                        pw = mkprobe(f"p_c1w_{g}", [128, 9 * 2 * 128])
                        nc.gpsimd.dma_start(out=pw[:, :],
                                            in_=c1gm.rearrange("p a b c -> p (a b c)"))
                    for ch in range(NCH):
                        h0 = ch * 8
                        ps = psmm.tile([128, CH], F32, name="c1_ps", tag="mm")
                        i = 0
                        for ti in range(9):
                            dy, dx = ti // 3 - 1, ti % 3 - 1
                            for kt in range(2):
                                nc.tensor.matmul(
                                    out=ps.rearrange("p (a b) -> p a b", a=8),
                                    lhsT=c1gm[:, ti, kt, :],
                                    rhs=xcp[2 * g + kt][:, 1 + h0 + dy:9 + h0 + dy,
                                                        1 + dx:65 + dx],
                                    start=(i == 0), stop=(i == 17))
                                i += 1
                        if probe and mt == 0 and ch == 0:
                            pps = mkprobe(f"p_c1ps_{g}", [128, CH])
                            stg = sml.tile([128, CH], F32, name="stg", tag="sqc")
                            nc.scalar.copy(out=stg, in_=ps)
                            nc.gpsimd.dma_start(out=pps[:, :], in_=stg)
                        nc.scalar.activation(
                            out=y1t[g][:, mt, ch * CH:(ch + 1) * CH], in_=ps,
                            func=Act.Gelu, bias=col('c1b', 2 * g + mt))

            if probe:
                py1 = mkprobe("p_y1", [CS, S])
                for i in range(4):
                    nc.gpsimd.dma_start(out=py1[128 * i:128 * (i + 1), :],
                                        in_=y1t[i // 2][:, i % 2, :])

            # ============ S6: c2 -> gelu -> c3 -> gelu(+bn3) -> y3 ============
            c3wt = wstr.tile([128, 8, 256], BF16, name="c3wt", tag="wst4", bufs=1)
            nc.sync.dma_start(out=c3wt, in_=d_c3[:, :].rearrange("(kt p) m -> p kt m", p=128))
            y3 = [big.tile([128, S], BF16, name="y3_0", tag="Emd"),
                  big.tile([128, S], BF16, name="y3_1", tag="D")]
            for ch in range(NCH):
                sl = slice(ch * CH, (ch + 1) * CH)
                ytiles = []
                for mt in range(8):
                    ps = psmm.tile([128, CH], F32, name="c2_ps", tag="mm")
                    for kt in range(4):
                        nc.tensor.matmul(out=ps, lhsT=c2wt[:, kt, 128 * mt:128 * (mt + 1)],
                                         rhs=y1t[kt // 2][:, kt % 2, sl],
                                         start=(kt == 0), stop=(kt == 3))
                    yt = y2b.tile([128, CH], BF16, name="y2t", tag="y2t")
                    nc.scalar.activation(out=yt, in_=ps, func=Act.Gelu, bias=col('c2b', mt))
                    ytiles.append(yt)
                for mt in range(2):
                    ps = psmm.tile([128, CH], F32, name="c3_ps", tag="mm")
                    for kt in range(8):
                        nc.tensor.matmul(out=ps, lhsT=c3wt[:, kt, 128 * mt:128 * (mt + 1)],
                                         rhs=ytiles[kt], start=(kt == 0), stop=(kt == 7))
                    nc.scalar.activation(out=y3[mt][:, sl], in_=ps, func=Act.Gelu,
                                         bias=col('c3b', mt))
                    if not bn3_triv:
                        nc.vector.tensor_scalar(out=y3[mt][:, sl],
                                                in0=y3[mt][:, sl],
                                                scalar1=col('g3p', mt),
                                                scalar2=col('b3p', mt), op0=MM, op1=AD)

            if probe:
                py3 = mkprobe("p_y3", [C, S])
                for i in range(2):
                    stg3 = sml.tile([128, S], F32, name=f"stg3_{i}", tag="stg3")
                    nc.vector.tensor_copy(out=stg3, in_=y3[i][:, :])
                    nc.gpsimd.dma_start(out=py3[128 * i:128 * (i + 1), :], in_=stg3)

            # ============ S7: LN2, up-proj, pixel-shuffle DMA out ============
            def ln2_wr(ct, sl, pr, pm):
                nc.vector.tensor_tensor(out=y3[ct][:, sl], in0=y3[ct][:, sl],
                                        in1=pr, op=MM)
                nc.vector.tensor_tensor(out=y3[ct][:, sl], in0=y3[ct][:, sl],
                                        in1=pm, op=SU)

            ln256(lambda ct: y3[ct][:, :], ln2_wr, "ln2", bf16_in=True,
                  wb=None if ln2_triv else ("ln2w", "ln2b", lambda ct: y3[ct][:, :]))

            upt = wstr.tile([128, 2, 512], BF16, name="upt", tag="wst4", bufs=1)
            nc.sync.dma_start(out=upt,
                              in_=d_upb[:, :].rearrange("(kt p) m -> p kt m", p=128))
            for r in range(2):
                for ch in range(NCH):
                    sl = slice(ch * CH, (ch + 1) * CH)
                    h0 = ch * 8
                    ub = yupp.tile([128, 8, 64, 2], BF16, name="ub", tag="ub")
                    for q in range(2):
                        rq = 2 * r + q
                        ps = psmm.tile([128, CH], F32, name="up_ps", tag="mm")
                        for kt in range(2):
                            nc.tensor.matmul(out=ps,
                                             lhsT=upt[:, kt, 128 * rq:128 * (rq + 1)],
                                             rhs=y3[kt][:, sl],
                                             start=(kt == 0), stop=(kt == 1))
                        nc.scalar.activation(out=ub[:, :, :, q],
                                             in_=ps.rearrange("p (a b) -> p a b", a=8),
                                             func=Act.Identity, bias=col('upb', rq))
                    dst = view(yout[:, :, :], [[128 * 128, 128], [256, 8], [1, 128]],
                               off=(2 * h0 + r) * 128)
                    nc.sync.dma_start(out=dst, in_=ub.rearrange("p a b q -> p a (b q)"))

    nc.compile()
    return nc, const_inputs


def _get_nc(weights, probe=False):
    import hashlib
    hsh = hashlib.sha1()
    for k in sorted(weights):
        hsh.update(k.encode())
        hsh.update(np.ascontiguousarray(weights[k]).tobytes())
    key = (hsh.hexdigest(), probe)
    if key not in _CACHE:
        _CACHE[key] = _build(weights, probe=probe)
    return _CACHE[key]


def kernel(**inputs):
    from concourse.bass_utils import run_bass_kernel_spmd

    x = np.asarray(inputs['x'], np.float32)
    skip = np.asarray(inputs['skip'], np.float32)
    mask = np.asarray(inputs['saliency_mask'], np.float32)
    weights = {k: np.asarray(v, np.float32) for k, v in inputs.items()
               if k not in ('x', 'skip', 'saliency_mask')}

    probe = bool(os.environ.get('BASSK_PROBE'))
    nc, const_inputs = _get_nc(weights, probe=probe)

    in_maps = []
    for b in range(B):
        m = dict(
            xin=np.ascontiguousarray(x[b].reshape(C, S)),
            skin=np.ascontiguousarray(skip[b].reshape(CS, S)),
            mrow=np.ascontiguousarray(mask[b].reshape(1, S)),
        )
        m.update(const_inputs)
        in_maps.append(m)
    res = run_bass_kernel_spmd(nc, in_maps, core_ids=list(range(B)),
                               trace=bool(os.environ.get('BASSK_TRACE')))
    kernel.last_results = res
    out = np.stack([np.asarray(res.results[b]['yout'], np.float32) for b in range(B)], axis=0)
    return out



# revision 33
# speedup vs baseline: 1.2466x; 1.0145x over previous
"""Trainium2 Bass kernel for nn_DecoderBlock (shape-guided RWKV decoder block).

Data-parallel over batch: B=8 samples -> 8 NeuronCores, one NEFF.

Per-core layout: channels on partitions (256ch -> 2 "ctiles" of 128), spatial
(h, w) flattened on the free dim (4096).

- LayerNorm over channels: square (ACT) -> ones-matmul partition reduction ->
  DRAM-bounce reshape -> tiny stat math -> K=1 matmul broadcast -> TT applies.
- q_shift / mask blend via shifted access patterns; per-channel mixes folded
  into Wk/Wv/Wr host-side (k = Wk@xn + (Wk*diag(1-mk))@md, md = mask*(xs-xn)).
- WKV: unstabilized linear recurrence A_t = lam*A_{t-1} + e^{k_t} v_t via the
  DVE TensorTensorScan instruction chained across rows (data0=0 resets at each
  row start); vertical orientation scans read E/EV through transposed APs.
- channel_fusion: grouped 3x3 conv = 9 shifted-AP matmuls accumulated in PSUM
  over zero-padded [c, 66, 66] inputs; BN folded into the next conv
  host-side; GELU+bias fused into the PSUM->SBUF evacuation on ACT.
- patch_expand: up-proj rows permuted host-side so pixel shuffle becomes a
  strided DMA.

Matmuls in float32r (full rate) or bf16; bulky intermediates bf16.
"""
import sys
import os

for _p in ('/opt/trn_rl_repo', '/root/.axon_site/_ro/trn_rl_repo'):
    if _p not in sys.path and os.path.isdir(_p):
        sys.path.append(_p)

import numpy as np

B, C, CS, COUT, H, W = 8, 256, 512, 128, 64, 64
S = H * W          # 4096
NCH = 8            # spatial chunks
CH = S // NCH      # 512
EPS = 1e-5

_CACHE = {}


def _build(weights, probe=False):
    const_inputs = {}
    import concourse.bass as bass
    from concourse import bacc
    import concourse.tile as tile
    import concourse.mybir as mybir
    import ml_dtypes

    F32 = mybir.dt.float32
    F32R = mybir.dt.float32r
    BF16 = mybir.dt.bfloat16
    Alu = mybir.AluOpType
    Act = mybir.ActivationFunctionType
    MM, AD, SU = Alu.mult, Alu.add, Alu.subtract

    w = weights
    f64 = lambda x: np.asarray(x, np.float64)
    bf = lambda a: np.asarray(a, dtype=ml_dtypes.bfloat16)

    # ---------------- host-side folding
    bnscale = 1.0 / np.sqrt(1.0 + EPS)
    g1p = f64(w['bn1_g']) * bnscale
    b1p = f64(w['bn1_b'])
    g2p = f64(w['bn2_g']) * bnscale
    b2p = f64(w['bn2_b'])
    g3p = (f64(w['bn3_g']) * bnscale).astype(np.float32)
    b3p = f64(w['bn3_b']).astype(np.float32)

    c2_eff = f64(w['c2_w']) * g1p[None, :]
    c2b_eff = (f64(w['c2_b']) + f64(w['c2_w']) @ b1p).astype(np.float32)
    c3_eff = f64(w['c3_w']) * g2p[None, :]
    c3b_eff = (f64(w['c3_b']) + f64(w['c3_w']) @ b2p).astype(np.float32)

    wk_x = f64(w['Wk']).T
    wk_d = (f64(w['Wk']) * (1.0 - f64(w['mix_k']))[None, :]).T
    wv_x = f64(w['Wv']).T
    wv_d = (f64(w['Wv']) * (1.0 - f64(w['mix_v']))[None, :]).T
    wr_x = f64(w['Wr']).T
    wr_d = (f64(w['Wr']) * (1.0 - f64(w['mix_r']))[None, :]).T
    wo_t = f64(w['Wo']).T
    sp_t = f64(w['sp_w']).T.astype(np.float32)

    lam = np.exp(-np.exp(f64(w['decay']))).astype(np.float32)
    lam64 = np.tile(lam[:, None], (1, 64))
    lam64[:, 0] = 0.0
    lam64 = lam64.astype(np.float32)
    eu = np.exp(f64(w['first'])).astype(np.float32)

    pidx = np.arange(512)
    old = (pidx % 128) * 4 + (pidx // 128)
    up_t = f64(w['up_w'])[old].T.astype(np.float32)             # [256, 512]
    up_tb = bf(up_t)
    upb_p = f64(w['up_b'])[old].astype(np.float32)

    c1w = f64(w['c1_w'])
    c1_l = np.zeros((9, 2, 256, 256), np.float32)
    for ti in range(9):
        dy, dx = ti // 3, ti % 3
        for g in range(2):
            c1_l[ti, g] = c1w[g * 256:(g + 1) * 256, :, dy, dx].T

    # per-channel vectors as columns of one [128, ncol] const
    cols, order = {}, []

    def addcol(name, vec):
        v = np.asarray(vec, np.float32).reshape(-1, 128)
        cols[name] = v
        order.append(name)

    addcol('eu', eu)
    addcol('ln1w', w['ln1_w'])
    addcol('ln1b', w['ln1_b'])
    addcol('knw', w['kn_w'])
    addcol('knb', w['kn_b'])
    addcol('ln2w', w['ln2_w'])
    addcol('ln2b', w['ln2_b'])
    addcol('g3p', g3p)
    addcol('b3p', b3p)
    addcol('spb', w['sp_b'])
    addcol('c3b', c3b_eff)
    addcol('c1b', w['c1_b'])
    addcol('c2b', c2b_eff)
    addcol('upb', upb_p)
    colidx, ncol = {}, 0
    for n in order:
        colidx[n] = ncol
        ncol += cols[n].shape[0]
    cvec_np = np.zeros((128, ncol), np.float32)
    for n in order:
        for i in range(cols[n].shape[0]):
            cvec_np[:, colidx[n] + i] = cols[n][i]

    ln1_triv = np.all(w['ln1_w'] == 1.0) and np.all(w['ln1_b'] == 0.0)
    kn_triv = np.all(w['kn_w'] == 1.0) and np.all(w['kn_b'] == 0.0)
    ln2_triv = np.all(w['ln2_w'] == 1.0) and np.all(w['ln2_b'] == 0.0)
    bn3_triv = np.all(g3p == g3p[0]) and np.all(b3p == 0.0)
    # uniform bn3 scale commutes with LN2 -> drop it entirely when trivial

    # ---------------- bass module
    nc = bacc.Bacc("TRN2", target_bir_lowering=False, debug=False, name="decblk")

    xin = nc.dram_tensor("xin", [C, S], F32, kind="ExternalInput")
    skin = nc.dram_tensor("skin", [CS, S], F32, kind="ExternalInput")
    mrow = nc.dram_tensor("mrow", [1, S], F32, kind="ExternalInput")
    yout = nc.dram_tensor("yout", [COUT, 2 * H, 2 * W], BF16, kind="ExternalOutput")
    probes = {}

    def mkprobe(name, shape):
        if probe:
            probes[name] = nc.dram_tensor(name, shape, F32, kind="ExternalOutput")
        return probes.get(name)

    def it(arr, name):
        arr = np.ascontiguousarray(arr)
        import ml_dtypes as _md
        dt_ = {np.dtype(np.float32): F32, np.dtype(_md.bfloat16): BF16}[arr.dtype]
        const_inputs[name] = arr
        return nc.dram_tensor(name, list(arr.shape), dt_, kind="ExternalInput")
    d_lam = it(lam64, "lam64")
    d_cvec = it(cvec_np, "cvec")
    d_wk = [it(bf(wk_x), "wkx"), it(bf(wk_d), "wkd")]
    d_wv = [it(bf(wv_x), "wvx"), it(bf(wv_d), "wvd")]
    d_wr = [it(bf(wr_x), "wrx"), it(bf(wr_d), "wrd")]
    d_wo = it(bf(wo_t), "wo")
    d_sp = it(sp_t, "sp")
    d_upb = it(up_tb, "up")
    c1_r = c1_l.reshape(9, 2, 2, 128, 2, 128).transpose(1, 4, 3, 0, 2, 5)
    d_c1 = it(bf(c1_r), "c1")   # [g, mt, p, t, kt, m]
    d_c2 = it(bf(c2_eff.T), "c2")
    d_c3 = it(bf(c3_eff.T), "c3")
    red_np = np.zeros((128, 2, 2), np.float32)
    red_np[:, 0, 0] = 1.0
    red_np[:, 1, 1] = 1.0
    d_red = it(red_np, "red")
    d_redb = it(bf(red_np), "redb")
    bc8_np = np.zeros((8, 8, 128), np.float32)
    for _v in range(8):
        bc8_np[_v, _v, :] = 1.0
    d_bc8 = it(bc8_np, "bc8")
    d_eps = it(np.full((128, 1), EPS, np.float32), "epsc")
    d_i2 = it(np.eye(2, dtype=np.float32), "i2c")
    d_i128 = it(np.eye(128, dtype=np.float32), "i128c")

    def scan_raw(out, d0, d1, eng=None):
        eng = eng or nc.vector
        if os.environ.get('BASSK_NOSCAN'):
            return eng.tensor_copy(out=out, in_=d1)
        return eng.add_instruction(mybir.InstTensorScalarPtr(
            name=nc.get_next_instruction_name(),
            is_tensor_tensor_scan=True,
            is_scalar_tensor_tensor=True,
            op0=MM, op1=AD,
            ins=[eng.lower_ap(d0), eng.lower_ap_or_imm(0.0), eng.lower_ap(d1)],
            outs=[eng.lower_ap(out)],
        ))

    def recip(out, in_):
        if os.environ.get('BASSK_SLOWRECIP'):
            return nc.vector.reciprocal(out=out, in_=in_)
        return nc.vector.reciprocal_approx_fast(out=out, in_=in_)

    def view(ap, dims, off=0):
        return bass.AP(tensor=ap.tensor, offset=ap.offset + off, ap=dims)

    with tile.TileContext(nc) as tc:
        with tc.tile_pool(name="big", bufs=1) as big, \
             tc.tile_pool(name="wres", bufs=1) as wres, \
             tc.tile_pool(name="scr", bufs=4) as scr, \
             tc.tile_pool(name="sml", bufs=3) as sml, \
             tc.tile_pool(name="y2b", bufs=8) as y2b, \
             tc.tile_pool(name="yupp", bufs=2) as yupp, \
             tc.tile_pool(name="xcs", bufs=2) as xcs, \
             tc.tile_pool(name="wstr", bufs=2) as wstr, \
             tc.tile_pool(name="dsc", bufs=2, space="DRAM") as dsc, \
             tc.tile_pool(name="psmm", bufs=3, space="PSUM") as psmm, \
             tc.tile_pool(name="psst", bufs=1, space="PSUM") as psst, \
             tc.tile_pool(name="psln", bufs=1, space="PSUM") as psln, \
             tc.tile_pool(name="psbc", bufs=1, space="PSUM") as psbc:

            # ---- resident constants
            lt = wres.tile([128, 2, 64], F32, name="lt")
            nc.sync.dma_start(out=lt, in_=d_lam[:, :].rearrange("(t p) j -> p t j", p=128))
            cv = wres.tile([128, ncol], F32, name="cv")
            nc.sync.dma_start(out=cv, in_=d_cvec[:, :])
            red = wres.tile([128, 2, 2], F32R, name="red")
            nc.sync.dma_start(out=red, in_=d_red[:, :, :].bitcast(F32R))
            redb = wres.tile([128, 2, 2], BF16, name="redb")
            nc.sync.dma_start(out=redb, in_=d_redb[:, :, :])
            bc8 = wres.tile([8, 8, 128], F32R, name="bc8")
            nc.sync.dma_start(out=bc8, in_=d_bc8[:, :, :].bitcast(F32R))
            epsc = wres.tile([128, 1], F32, name="epsc")
            nc.sync.dma_start(out=epsc, in_=d_eps[:, :])
            i2c = wres.tile([2, 2], F32R, name="i2c")
            nc.sync.dma_start(out=i2c, in_=d_i2[:, :].bitcast(F32R))
            i128c = wres.tile([128, 128], F32R, name="i128c")
            nc.sync.dma_start(out=i128c, in_=d_i128[:, :].bitcast(F32R))
            c2wt = wres.tile([128, 4, 1024], BF16, name="c2wt")
            nc.sync.dma_start(out=c2wt, in_=d_c2[:, :].rearrange("(kt p) m -> p kt m", p=128))

            def col(name, i=0):
                return cv[:, colidx[name] + i:colidx[name] + i + 1]

            # ============ LayerNorm over channels (2 ctiles) ============
            # On-chip stats path: ones-matmul partition reduction -> PE
            # transpose to pixel-major [128, 32, 2] -> tiny stat math ->
            # PE transpose back to [2, S] -> K=2 broadcast matmuls.
            def ln256(Xr, out_wr, name, wb=None, bf16_in=False):
                redm = redb if bf16_in else red
                pst = psln.tile([128, 32, 2], F32, name=f"pst_{name}", tag="pst")
                for ch in range(NCH):
                    sl = slice(ch * CH, (ch + 1) * CH)
                    ps = psst.tile([2, CH], F32, name=f"lnps_{name}", tag="st")
                    for ct in range(2):
                        nc.tensor.matmul(out=ps, lhsT=redm[:, 0, :], rhs=Xr(ct)[:, sl],
                                         start=(ct == 0), stop=False)
                    for ct in range(2):
                        sq = sml.tile([128, CH], BF16,
                                      name=f"sq_{name}", tag="sqc", bufs=3)
                        nc.scalar.activation(
                            out=sq,
                            in_=Xr(ct)[:, sl] if bf16_in else Xr(ct)[:, sl].bitcast(F32),
                            func=Act.Square)
                        nc.tensor.matmul(out=ps, lhsT=redb[:, 1, :], rhs=sq,
                                         start=False, stop=(ct == 1))
                    stc = sml.tile([2, CH], F32R, name=f"stc_{name}", tag="stc", bufs=1)
                    nc.scalar.copy(out=stc, in_=ps)
                    for j in range(4):
                        nc.tensor.matmul(
                            out=pst[:, ch * 4 + j, :],
                            lhsT=stc[:, j * 128:(j + 1) * 128],
                            rhs=i2c[:, :])
                # small stat math in [128, 32, 2] pixel-major layout
                smb = sml.tile([128, 32, 2], F32, name=f"smb_{name}", tag="sm", bufs=1)
                nc.vector.tensor_scalar_mul(out=smb, in0=pst, scalar1=1.0 / C)
                t2 = sml.tile([128, 32], F32, name=f"t2_{name}", tag="t2", bufs=1)
                nc.vector.tensor_tensor(out=t2, in0=smb[:, :, 0], in1=smb[:, :, 0], op=MM)
                nc.vector.tensor_tensor(out=t2, in0=smb[:, :, 1], in1=t2, op=SU)
                nc.scalar.activation(out=t2, in_=t2, func=Act.Sqrt, bias=epsc)
                # stats scattered into one-hot columns (col = 2*sub + q) so
                # the PE transpose lands rows 0-7 partition-aligned; [8, S/4]
                # bcr is 4x narrower in SBUF than a [2, S] row pair.
                sm8 = sml.tile([128, 32, 8], F32R, name=f"sm8_{name}", tag="sm2", bufs=1)
                s8v = sm8.rearrange("p a b -> p (a b)")
                with nc.allow_low_precision(reason="zeroing f32r one-hot scatter"):
                    nc.vector.tensor_scalar_mul(
                        out=s8v, in0=view(epsc[:, :], [epsc[:, :].ap[0], [0, 256]]),
                        scalar1=0.0)

                def s8view(q):
                    return view(s8v[:, :], [s8v.ap[0], [66, 4], [8, 8]], off=q)

                with nc.allow_low_precision(reason="rstd in f32r for PE transpose"):
                    nc.vector.reciprocal(out=s8view(0), in_=t2)            # rstd
                nc.vector.tensor_tensor(out=s8view(1), in0=smb[:, :, 0],
                                        in1=s8view(0).bitcast(F32), op=MM)  # mu*rstd
                bcr = big.tile([8, S // 4], F32R, name=f"bcr_{name}", tag="rowsbig")
                for hoff in range(2):
                    psb8 = psst.tile([8, CH], F32, name=f"psb_{name}", tag="st")
                    for j in range(4):
                        for sub in range(4):
                            nc.tensor.matmul(
                                out=psb8[:, j * 128:(j + 1) * 128],
                                lhsT=sm8[:, (sub * 2 + hoff) * 4 + j, :],
                                rhs=i128c[:, :],
                                start=(sub == 0), stop=(sub == 3))
                    nc.scalar.copy(out=bcr[:, hoff * CH:(hoff + 1) * CH], in_=psb8)
                for ch in range(NCH):
                    sl = slice(ch * CH, (ch + 1) * CH)
                    sub, hoff = ch // 2, ch % 2
                    pr = psbc.tile([128, CH], F32, name=f"pr_{name}", tag="pr")
                    pm = psbc.tile([128, CH], F32, name=f"pm_{name}", tag="pm")
                    nc.tensor.matmul(out=pr, lhsT=bc8[:, 2 * sub + 0, :],
                                     rhs=bcr[:, hoff * CH:(hoff + 1) * CH])
                    nc.tensor.matmul(out=pm, lhsT=bc8[:, 2 * sub + 1, :],
                                     rhs=bcr[:, hoff * CH:(hoff + 1) * CH])
                    for ct in range(2):
                        out_wr(ct, sl, pr, pm)
                if wb is not None:
                    wn, bn_, apfn = wb
                    for ct in range(2):
                        ap = apfn(ct)
                        nc.vector.tensor_scalar(
                            out=ap, in0=ap, scalar1=col(wn, ct), scalar2=col(bn_, ct),
                            op0=MM, op1=AD)

            # ============ S0/S1: load x, LN1 -> xn (bf16) ============
            x0 = big.tile([128, S], F32R, name="x0", tag="A")
            x1 = big.tile([128, S], F32R, name="x1", tag="B")
            nc.sync.dma_start(out=x0, in_=xin[0:128, :].bitcast(F32R))
            nc.sync.dma_start(out=x1, in_=xin[128:256, :].bitcast(F32R))
            mf = big.tile([128, S], F32, name="mf", tag="D")
            nc.sync.dma_start(out=mf, in_=view(mrow[:, :], [[0, 128], [1, S]]))

            xn = big.tile([128, 2, S], BF16, name="xn", tag="Cxn")
            xt = [x0, x1]

            def ln1_wr(ct, sl, pr, pm):
                nc.vector.tensor_tensor(out=xn[:, ct, sl], in0=xt[ct][:, sl].bitcast(F32),
                                        in1=pr, op=MM)
                nc.vector.tensor_tensor(out=xn[:, ct, sl], in0=xn[:, ct, sl],
                                        in1=pm, op=SU)

            ln256(lambda ct: xt[ct][:, :], ln1_wr, "ln1",
                  wb=None if ln1_triv else ("ln1w", "ln1b", lambda ct: xn[:, ct, :]))
            if probe:
                pxn = mkprobe("p_xn", [C, S])
                for ct in range(2):
                    nc.gpsimd.dma_start(out=pxn[128 * ct:128 * (ct + 1), :],
                                        in_=xn[:, ct, :])

            # ============ S2: q_shift diff * mask -> md (bf16) ============
            xn4 = xn.rearrange("p t (h w) -> p t h w", h=H)
            md = big.tile([128, 2, H, W], BF16, name="md", tag="Emd")
            nc.vector.tensor_tensor(out=md[0:64, 0, :, 1:], in0=xn4[0:64, 0, :, 0:63],
                                    in1=xn4[0:64, 0, :, 1:], op=SU)
            nc.vector.tensor_scalar_mul(out=md[0:64, 0, :, 0:1],
                                        in0=xn4[0:64, 0, :, 0:1], scalar1=-1.0)
            nc.vector.tensor_tensor(out=md[64:128, 0, :, 0:63], in0=xn4[64:128, 0, :, 1:],
                                    in1=xn4[64:128, 0, :, 0:63], op=SU)
            nc.vector.tensor_scalar_mul(out=md[64:128, 0, :, 63:64],
                                        in0=xn4[64:128, 0, :, 63:64], scalar1=-1.0)
            nc.vector.tensor_tensor(out=md[0:64, 1, 1:, :], in0=xn4[0:64, 1, 0:63, :],
                                    in1=xn4[0:64, 1, 1:, :], op=SU)
            nc.vector.tensor_scalar_mul(out=md[0:64, 1, 0:1, :],
                                        in0=xn4[0:64, 1, 0:1, :], scalar1=-1.0)
            nc.vector.tensor_tensor(out=md[64:128, 1, 0:63, :], in0=xn4[64:128, 1, 1:, :],
                                    in1=xn4[64:128, 1, 0:63, :], op=SU)
            nc.vector.tensor_scalar_mul(out=md[64:128, 1, 63:64, :],
                                        in0=xn4[64:128, 1, 63:64, :], scalar1=-1.0)
            mdf = md.rearrange("p t h w -> p t (h w)")
            for ct in range(2):
                nc.vector.tensor_tensor(out=mdf[:, ct, :], in0=mdf[:, ct, :],
                                        in1=mf, op=MM)

            # ============ S3: k/v/r matmuls -> E, V, SR; scans ============
            ev = big.tile([128, 2, S], BF16, name="ev", tag="B")
            et = big.tile([128, 2, S], BF16, name="et", tag="A")
            vv = big.tile([128, 2, S], BF16, name="vv", tag="D")
            sr = big.tile([128, 2, S], BF16, name="sr", tag="Fsr")

            def kvloop(dws, evac):
                wxt = wstr.tile([128, 2, 256], BF16, name="wxt", tag="wst", bufs=2)
                wdt = wstr.tile([128, 2, 256], BF16, name="wdt", tag="wst", bufs=2)
                nc.sync.dma_start(out=wxt, in_=dws[0][:, :].rearrange("(kt p) m -> p kt m", p=128))
                nc.sync.dma_start(out=wdt, in_=dws[1][:, :].rearrange("(kt p) m -> p kt m", p=128))
                for mt in range(2):
                    for ch in range(NCH):
                        sl = slice(ch * CH, (ch + 1) * CH)
                        ps = psmm.tile([128, CH], F32, name="kv_ps", tag="mm")
                        for kt in range(2):
                            nc.tensor.matmul(out=ps, lhsT=wxt[:, kt, 128 * mt:128 * (mt + 1)],
                                             rhs=xn[:, kt, sl], start=(kt == 0), stop=False)
                        for kt in range(2):
                            nc.tensor.matmul(out=ps, lhsT=wdt[:, kt, 128 * mt:128 * (mt + 1)],
                                             rhs=mdf[:, kt, sl], start=False, stop=(kt == 1))
                        evac(mt, sl, ps)

            kvloop(d_wk, lambda mt, sl, ps: nc.scalar.activation(
                out=et[:, mt, sl], in_=ps, func=Act.Exp))
            kvloop(d_wv, lambda mt, sl, ps: nc.scalar.copy(out=vv[:, mt, sl], in_=ps))

            nc.vector.tensor_tensor(out=ev, in0=et, in1=vv, op=MM)

            ev4 = ev.rearrange("p t (h w) -> p t h w", h=H)
            et4 = et.rearrange("p t (h w) -> p t h w", h=H)

            # transposed copies for the vertical orientation (ACT engine):
            # etT/evT[p, ct, w, h] = et/ev[p, ct, h, w]
            etT = big.tile([128, 2, W, H], BF16, name="etT", tag="D")
            evT = big.tile([128, 2, W, H], BF16, name="evT", tag="Emd")
            nc.scalar.copy(out=etT,
                           in_=view(et[:, :, :], [et.ap[0], [S, 2], [1, W], [W, H]]))
            nc.scalar.copy(out=evT,
                           in_=view(ev[:, :, :], [ev.ap[0], [S, 2], [1, W], [W, H]]))

            kvloop(d_wr, lambda mt, sl, ps: nc.scalar.activation(
                out=sr[:, mt, sl], in_=ps, func=Act.Sigmoid))

            # prime the wst9 slots early so c1 weight DMAs can land during scans
            if not os.environ.get('BASSK_NOPRIME'):
                for i in range(2):
                    pr_ = wstr.tile([128, 1], BF16, name=f"prime{i}", tag="wst9")
                    nc.vector.tensor_copy(out=pr_, in_=xn[:, 0, 0:1])

            # ---- early prefetch + skip-feature GEMM (overlaps scan phase) ----
            wot = wstr.tile([128, 2, 256], BF16, name="wot", tag="wst", bufs=2)
            nc.sync.dma_start(out=wot, in_=d_wo[:, :].rearrange("(kt p) m -> p kt m", p=128))
            spt = wstr.tile([128, 4, 256], F32R, name="spt", tag="wst4", bufs=1)
            nc.sync.dma_start(out=spt,
                              in_=d_sp[:, :].rearrange("(kt p) m -> p kt m", p=128).bitcast(F32R))
            xcp23 = [xcs.tile([128, 66, 66], BF16, name=f"xcs{i}", tag="xcs")
                     for i in range(2)]
            for t in xcp23:
                nc.vector.memset(t[:, 0:1, :], 0.0)
                nc.vector.memset(t[:, 65:66, :], 0.0)
                nc.vector.memset(t[:, 1:65, 0:1], 0.0)
                nc.vector.memset(t[:, 1:65, 65:66], 0.0)
            CQ = CH // 2
            for ch in range(2 * NCH):
                h0 = ch * 4
                sl = slice(ch * CQ, (ch + 1) * CQ)
                skc = sml.tile([128, 4, CQ], F32R, name="skc", tag="skcf", bufs=1)
                nc.sync.dma_start(
                    out=skc,
                    in_=skin[:, sl].rearrange("(kt p) n -> p kt n", p=128).bitcast(F32R))
                for mt in range(2):
                    ps = psmm.tile([128, CQ], F32, name="sp_ps", tag="mm")
                    for kt in range(4):
                        nc.tensor.matmul(out=ps, lhsT=spt[:, kt, 128 * mt:128 * (mt + 1)],
                                         rhs=skc[:, kt, :], start=(kt == 0), stop=(kt == 3))
                    nc.scalar.activation(
                        out=xcp23[mt][:, 1 + h0:5 + h0, 1:65],
                        in_=ps.rearrange("p (a b) -> p a b", a=4),
                        func=Act.Identity, bias=col('spb', mt))

            # ---- c1 group 1 (skip-feature half) early: its inputs are ready
            # before the scans, so the 3x3-conv matmuls fill the PE during the
            # vector-only scan phase; outputs bounce through DRAM until c2.
            dy1 = [dsc.tile([128, S], BF16, name=f"dy1_{mt}", tag=f"dy1{mt}")
                   for mt in range(2)]
            for mt in range(2):
                c1gm = wstr.tile([128, 9, 2, 128], BF16, name="c1gm", tag="wst9")
                nc.sync.dma_start(out=c1gm, in_=d_c1[1, mt, :, :, :, :])
                for ch in range(NCH):
                    h0 = ch * 8
                    ps = psmm.tile([128, CH], F32, name="c1_ps", tag="mm")
                    i = 0
                    for ti in range(9):
                        dy, dx = ti // 3 - 1, ti % 3 - 1
                        for kt in range(2):
                            nc.tensor.matmul(
                                out=ps.rearrange("p (a b) -> p a b", a=8),
                                lhsT=c1gm[:, ti, kt, :],
                                rhs=xcp23[kt][:, 1 + h0 + dy:9 + h0 + dy,
                                              1 + dx:65 + dx],
                                start=(i == 0), stop=(i == 17))
                            i += 1
                    yst = y2b.tile([128, CH], BF16, name="y1g1", tag="y2t")
                    nc.scalar.activation(out=yst, in_=ps, func=Act.Gelu,
                                         bias=col('c1b', 2 + mt))
                    nc.sync.dma_start(out=dy1[mt][:, ch * CH:(ch + 1) * CH], in_=yst)

            # ---- WKV scans: zero-padded 66-wide scan outputs keep every
            # consumer read packed + 4B-aligned (2x DVE mode); vertical
            # orientation runs on the transposed copies so it is packed too.
            lt_ap = lt[:, :, :]

            def lamview(ct, nseq):
                return view(lt_ap, [lt_ap.ap[0], [0, nseq], [1, 64]], off=ct * 64)

            pool_flag = os.environ.get('BASSK_POOLSCAN', '')

            def orient(e4v, v4v, scan_eng):
                for half in range(2):
                    rows = slice(half * 32, (half + 1) * 32)
                    a = scr.tile([128, 2, 32, 66], BF16, name="a_sc", tag="scrt")
                    b = scr.tile([128, 2, 32, 66], BF16, name="b_sc", tag="scrt")
                    nc.vector.memset(a[:, :, :, 0:1], 0.0)
                    nc.vector.memset(b[:, :, :, 0:1], 0.0)
                    for ct in range(2):
                        scan_raw(a[:, ct, :, 1:65], lamview(ct, 32),
                                 v4v[:, ct, rows, :], eng=scan_eng)
                        scan_raw(b[:, ct, :, 1:65], lamview(ct, 32),
                                 e4v[:, ct, rows, :], eng=scan_eng)
                    for ct in range(2):
                        den = scr.tile([128, 32, 64], F32, name="den", tag="scrt")
                        nc.vector.scalar_tensor_tensor(
                            out=den, in0=e4v[:, ct, rows, :], scalar=col('eu', ct),
                            in1=b[:, ct, :, 0:64], op0=MM, op1=AD)
                        recip(out=den, in_=den)
                        rdb = sml.tile([128, 32, 64], BF16, name="rdb", tag="stc", bufs=1)
                        nc.scalar.copy(out=rdb, in_=den)
                        nc.vector.scalar_tensor_tensor(
                            out=v4v[:, ct, rows, :], in0=v4v[:, ct, rows, :],
                            scalar=col('eu', ct), in1=a[:, ct, :, 0:64], op0=MM, op1=AD)
                        nc.vector.tensor_tensor(out=v4v[:, ct, rows, :],
                                                in0=v4v[:, ct, rows, :], in1=rdb, op=MM)

            orient(et4, ev4, nc.gpsimd if pool_flag == 'a' else nc.vector)
            orient(etT, evT, nc.gpsimd if pool_flag in ('a', 'v') else nc.vector)

            # wkv = out_h + out_v^T, per h-half so kn stats can start early
            for half in range(2):
                hr = slice(half * 32, (half + 1) * 32)
                for ct in range(2):
                    ovT = view(evT[:, :, :, :], [evT.ap[0], [1, 32], [64, 64]],
                               off=ct * S + half * 32)
                    nc.vector.tensor_tensor(out=ev4[:, ct, hr, :], in0=ev4[:, ct, hr, :],
                                            in1=ovT, op=AD)
            if probe:
                pwkv = mkprobe("p_wkv", [C, S])
                for ct in range(2):
                    nc.gpsimd.dma_start(out=pwkv[128 * ct:128 * (ct + 1), :],
                                        in_=ev[:, ct, :])

            # ============ S4: key-LN, srw, Wo+residual, skip feat ============
            def kn_wr(ct, sl, pr, pm):
                nc.vector.tensor_tensor(out=ev[:, ct, sl], in0=ev[:, ct, sl], in1=pr, op=MM)
                nc.vector.tensor_tensor(out=ev[:, ct, sl], in0=ev[:, ct, sl], in1=pm, op=SU)

            ln256(lambda ct: ev[:, ct, :], kn_wr, "kn", bf16_in=True,
                  wb=None if kn_triv else ("knw", "knb", lambda ct: ev[:, ct, :]))

            nc.vector.tensor_tensor(out=sr, in0=sr, in1=ev, op=MM)   # srw

            xcp = [scr.tile([128, 66, 66], BF16, name=f"xcp{i}", tag="scrt")
                   for i in range(2)] + xcp23
            for t in xcp[:2]:
                nc.vector.memset(t[:, 0:1, :], 0.0)
                nc.vector.memset(t[:, 65:66, :], 0.0)
                nc.vector.memset(t[:, 1:65, 0:1], 0.0)
                nc.vector.memset(t[:, 1:65, 65:66], 0.0)

            for mt in range(2):
                for ch in range(NCH):
                    sl = slice(ch * CH, (ch + 1) * CH)
                    h0 = ch * 8
                    ps = psmm.tile([128, CH], F32, name="wo_ps", tag="mm")
                    for kt in range(2):
                        nc.tensor.matmul(out=ps, lhsT=wot[:, kt, 128 * mt:128 * (mt + 1)],
                                         rhs=sr[:, kt, sl], start=(kt == 0), stop=(kt == 1))
                    nc.vector.tensor_tensor(
                        out=xcp[mt][:, 1 + h0:9 + h0, 1:65],
                        in0=xn4[:, mt, h0:h0 + 8, :],
                        in1=ps.rearrange("p (a b) -> p a b", a=8), op=AD)

            if probe:
                pxc = mkprobe("p_xcat", [CS, S])
                for i in range(4):
                    nc.gpsimd.dma_start(
                        out=pxc[128 * i:128 * (i + 1), :].rearrange("p (a b) -> p a b", a=64),
                        in_=xcp[i][:, 1:65, 1:65])

            # ============ S5: grouped 3x3 conv -> gelu -> y1 (bf16) ============
            y1a = big.tile([128, 2, S], BF16, name="y1a", tag="A")
            if probe and os.environ.get('BASSK_CANARY'):
                cnry = wstr.tile([128, 9, 2, 128], BF16, name="cnry", tag="wst9")
                nc.sync.dma_start(out=cnry, in_=d_c1[1, 0, :, :, :, :])
                marks = [("m0", cnry[:, 0, 0, 0:64]),
                         ("m1", xn[:, 0, 0:64]),
                         ("m2", ev[:, 0, 0:64]),
                         ("m3", sr[:, 0, 0:64])]
                for mi, (mn, mark) in enumerate(marks):
                    stg_c = sml.tile([128, 64], BF16, name=f"cst{mi}",
                                     tag="cst", bufs=4)
                    nc.vector.tensor_tensor(
                        out=stg_c, in0=cnry[:, 0, 0, 0:64],
                        in1=mark, op=Alu.bypass)
                    pc = mkprobe(f"p_cn{mi}", [128, 64])
                    nc.gpsimd.dma_start(out=pc[:, :], in_=stg_c)
            for mt in range(2):
                c1gm = wstr.tile([128, 9, 2, 128], BF16, name="c1gm", tag="wst9")
                nc.sync.dma_start(out=c1gm, in_=d_c1[0, mt, :, :, :, :])
                for ch in range(NCH):
                    h0 = ch * 8
                    ps = psmm.tile([128, CH], F32, name="c1_ps", tag="mm")
                    i = 0
                    for ti in range(9):
                        dy, dx = ti // 3 - 1, ti % 3 - 1
                        for kt in range(2):
                            nc.tensor.matmul(
                                out=ps.rearrange("p (a b) -> p a b", a=8),
                                lhsT=c1gm[:, ti, kt, :],
                                rhs=xcp[kt][:, 1 + h0 + dy:9 + h0 + dy,
                                            1 + dx:65 + dx],
                                start=(i == 0), stop=(i == 17))
                            i += 1
                    nc.scalar.activation(
                        out=y1a[:, mt, ch * CH:(ch + 1) * CH], in_=ps,
                        func=Act.Gelu, bias=col('c1b', mt))

            if probe:
                py1 = mkprobe("p_y1", [CS, S])
                for i in range(2):
                    nc.gpsimd.dma_start(out=py1[128 * i:128 * (i + 1), :],
                                        in_=y1a[:, i, :])
                for mt in range(2):
                    nc.gpsimd.dma_start(out=py1[128 * (2 + mt):128 * (3 + mt), :],
                                        in_=dy1[mt][:, :])

            # ============ S6: c2 -> gelu -> c3 -> gelu(+bn3) -> y3 ============
            c3wt = wstr.tile([128, 8, 256], BF16, name="c3wt", tag="wst4", bufs=1)
            nc.sync.dma_start(out=c3wt, in_=d_c3[:, :].rearrange("(kt p) m -> p kt m", p=128))
            y3 = [big.tile([128, S], BF16, name="y3_0", tag="Emd"),
                  big.tile([128, S], BF16, name="y3_1", tag="D")]
            for ch in range(NCH):
                sl = slice(ch * CH, (ch + 1) * CH)
                rls = []
                for j in range(2):
                    rl = sml.tile([128, CH], BF16, name="y1r", tag="y1r", bufs=2)
                    nc.sync.dma_start(out=rl, in_=dy1[j][:, sl])
                    rls.append(rl)
                ytiles = []
                for mt in range(8):
                    ps = psmm.tile([128, CH], F32, name="c2_ps", tag="mm")
                    for kt in range(4):
                        nc.tensor.matmul(out=ps, lhsT=c2wt[:, kt, 128 * mt:128 * (mt + 1)],
                                         rhs=y1a[:, kt, sl] if kt < 2 else rls[kt - 2],
                                         start=(kt == 0), stop=(kt == 3))
                    yt = y2b.tile([128, CH], BF16, name="y2t", tag="y2t")
                    nc.scalar.activation(out=yt, in_=ps, func=Act.Gelu, bias=col('c2b', mt))
                    ytiles.append(yt)
                for mt in range(2):
                    ps = psmm.tile([128, CH], F32, name="c3_ps", tag="mm")
                    for kt in range(8):
                        nc.tensor.matmul(out=ps, lhsT=c3wt[:, kt, 128 * mt:128 * (mt + 1)],
                                         rhs=ytiles[kt], start=(kt == 0), stop=(kt == 7))
                    nc.scalar.activation(out=y3[mt][:, sl], in_=ps, func=Act.Gelu,
                                         bias=col('c3b', mt))
                    if not bn3_triv:
                        nc.vector.tensor_scalar(out=y3[mt][:, sl],
                                                in0=y3[mt][:, sl],
                                                scalar1=col('g3p', mt),
                                                scalar2=col('b3p', mt), op0=MM, op1=AD)

            if probe:
                py3 = mkprobe("p_y3", [C, S])
                for i in range(2):
                    stg3 = sml.tile([128, S], F32, name=f"stg3_{i}", tag="stg3")
                    nc.vector.tensor_copy(out=stg3, in_=y3[i][:, :])
                    nc.gpsimd.dma_start(out=py3[128 * i:128 * (i + 1), :], in_=stg3)

            # ============ S7: LN2, up-proj, pixel-shuffle DMA out ============
            def ln2_wr(ct, sl, pr, pm):
                nc.vector.tensor_tensor(out=y3[ct][:, sl], in0=y3[ct][:, sl],
                                        in1=pr, op=MM)
                nc.vector.tensor_tensor(out=y3[ct][:, sl], in0=y3[ct][:, sl],
                                        in1=pm, op=SU)

            ln256(lambda ct: y3[ct][:, :], ln2_wr, "ln2", bf16_in=True,
                  wb=None if ln2_triv else ("ln2w", "ln2b", lambda ct: y3[ct][:, :]))

            upt = wstr.tile([128, 2, 512], BF16, name="upt", tag="wst4", bufs=1)
            nc.sync.dma_start(out=upt,
                              in_=d_upb[:, :].rearrange("(kt p) m -> p kt m", p=128))
            for r in range(2):
                for ch in range(NCH):
                    sl = slice(ch * CH, (ch + 1) * CH)
                    h0 = ch * 8
                    ub = yupp.tile([128, 8, 64, 2], BF16, name="ub", tag="ub")
                    for q in range(2):
                        rq = 2 * r + q
                        ps = psmm.tile([128, CH], F32, name="up_ps", tag="mm")
                        for kt in range(2):
                            nc.tensor.matmul(out=ps,
                                             lhsT=upt[:, kt, 128 * rq:128 * (rq + 1)],
                                             rhs=y3[kt][:, sl],
                                             start=(kt == 0), stop=(kt == 1))
                        nc.scalar.activation(out=ub[:, :, :, q],
                                             in_=ps.rearrange("p (a b) -> p a b", a=8),
                                             func=Act.Identity, bias=col('upb', rq))
                    dst = view(yout[:, :, :], [[128 * 128, 128], [256, 8], [1, 128]],
                               off=(2 * h0 + r) * 128)
                    nc.sync.dma_start(out=dst, in_=ub.rearrange("p a b q -> p a (b q)"))

    nc.compile()
    return nc, const_inputs


def _get_nc(weights, probe=False):
    import hashlib
    hsh = hashlib.sha1()
    for k in sorted(weights):
        hsh.update(k.encode())
        hsh.update(np.ascontiguousarray(weights[k]).tobytes())
    key = (hsh.hexdigest(), probe)
    if key not in _CACHE:
        _CACHE[key] = _build(weights, probe=probe)
    return _CACHE[key]


def kernel(**inputs):
    from concourse.bass_utils import run_bass_kernel_spmd

    x = np.asarray(inputs['x'], np.float32)
    skip = np.asarray(inputs['skip'], np.float32)
    mask = np.asarray(inputs['saliency_mask'], np.float32)
    weights = {k: np.asarray(v, np.float32) for k, v in inputs.items()
               if k not in ('x', 'skip', 'saliency_mask')}

    probe = bool(os.environ.get('BASSK_PROBE'))
    nc, const_inputs = _get_nc(weights, probe=probe)

    in_maps = []
    for b in range(B):
        m = dict(
            xin=np.ascontiguousarray(x[b].reshape(C, S)),
            skin=np.ascontiguousarray(skip[b].reshape(CS, S)),
            mrow=np.ascontiguousarray(mask[b].reshape(1, S)),
        )
        m.update(const_inputs)
        in_maps.append(m)
    res = run_bass_kernel_spmd(nc, in_maps, core_ids=list(range(B)),
                               trace=bool(os.environ.get('BASSK_TRACE')))
    kernel.last_results = res
    out = np.stack([np.asarray(res.results[b]['yout'], np.float32) for b in range(B)], axis=0)
    return out



# revision 34
# speedup vs baseline: 1.2902x; 1.0350x over previous
"""Trainium2 Bass kernel for nn_DecoderBlock (shape-guided RWKV decoder block).

Data-parallel over batch: B=8 samples -> 8 NeuronCores, one NEFF.

Per-core layout: channels on partitions (256ch -> 2 "ctiles" of 128), spatial
(h, w) flattened on the free dim (4096).

- LayerNorm over channels: square (ACT) -> ones-matmul partition reduction ->
  DRAM-bounce reshape -> tiny stat math -> K=1 matmul broadcast -> TT applies.
- q_shift / mask blend via shifted access patterns; per-channel mixes folded
  into Wk/Wv/Wr host-side (k = Wk@xn + (Wk*diag(1-mk))@md, md = mask*(xs-xn)).
- WKV: unstabilized linear recurrence A_t = lam*A_{t-1} + e^{k_t} v_t via the
  DVE TensorTensorScan instruction chained across rows (data0=0 resets at each
  row start); vertical orientation scans read E/EV through transposed APs.
- channel_fusion: grouped 3x3 conv = 9 shifted-AP matmuls accumulated in PSUM
  over zero-padded [c, 66, 66] inputs; BN folded into the next conv
  host-side; GELU+bias fused into the PSUM->SBUF evacuation on ACT.
- patch_expand: up-proj rows permuted host-side so pixel shuffle becomes a
  strided DMA.

Matmuls in float32r (full rate) or bf16; bulky intermediates bf16.
"""
import sys
import os

for _p in ('/opt/trn_rl_repo', '/root/.axon_site/_ro/trn_rl_repo'):
    if _p not in sys.path and os.path.isdir(_p):
        sys.path.append(_p)

import numpy as np

B, C, CS, COUT, H, W = 8, 256, 512, 128, 64, 64
S = H * W          # 4096
NCH = 8            # spatial chunks
CH = S // NCH      # 512
EPS = 1e-5

_CACHE = {}


def _build(weights, probe=False):
    const_inputs = {}
    import concourse.bass as bass
    from concourse import bacc
    import concourse.tile as tile
    import concourse.mybir as mybir
    import ml_dtypes

    F32 = mybir.dt.float32
    F32R = mybir.dt.float32r
    BF16 = mybir.dt.bfloat16
    Alu = mybir.AluOpType
    Act = mybir.ActivationFunctionType
    MM, AD, SU = Alu.mult, Alu.add, Alu.subtract

    w = weights
    f64 = lambda x: np.asarray(x, np.float64)
    bf = lambda a: np.asarray(a, dtype=ml_dtypes.bfloat16)

    # ---------------- host-side folding
    bnscale = 1.0 / np.sqrt(1.0 + EPS)
    g1p = f64(w['bn1_g']) * bnscale
    b1p = f64(w['bn1_b'])
    g2p = f64(w['bn2_g']) * bnscale
    b2p = f64(w['bn2_b'])
    g3p = (f64(w['bn3_g']) * bnscale).astype(np.float32)
    b3p = f64(w['bn3_b']).astype(np.float32)

    c2_eff = f64(w['c2_w']) * g1p[None, :]
    c2b_eff = (f64(w['c2_b']) + f64(w['c2_w']) @ b1p).astype(np.float32)
    c3_eff = f64(w['c3_w']) * g2p[None, :]
    c3b_eff = (f64(w['c3_b']) + f64(w['c3_w']) @ b2p).astype(np.float32)

    wk_x = f64(w['Wk']).T
    wk_d = (f64(w['Wk']) * (1.0 - f64(w['mix_k']))[None, :]).T
    wv_x = f64(w['Wv']).T
    wv_d = (f64(w['Wv']) * (1.0 - f64(w['mix_v']))[None, :]).T
    wr_x = f64(w['Wr']).T
    wr_d = (f64(w['Wr']) * (1.0 - f64(w['mix_r']))[None, :]).T
    wo_t = f64(w['Wo']).T
    sp_t = f64(w['sp_w']).T.astype(np.float32)

    lam = np.exp(-np.exp(f64(w['decay']))).astype(np.float32)
    lam64 = np.tile(lam[:, None], (1, 64))
    lam64[:, 0] = 0.0
    lam64 = lam64.astype(np.float32)
    eu = np.exp(f64(w['first'])).astype(np.float32)

    pidx = np.arange(512)
    old = (pidx % 128) * 4 + (pidx // 128)
    up_t = f64(w['up_w'])[old].T.astype(np.float32)             # [256, 512]
    up_tb = bf(up_t)
    upb_p = f64(w['up_b'])[old].astype(np.float32)

    c1w = f64(w['c1_w'])
    c1_l = np.zeros((9, 2, 256, 256), np.float32)
    for ti in range(9):
        dy, dx = ti // 3, ti % 3
        for g in range(2):
            c1_l[ti, g] = c1w[g * 256:(g + 1) * 256, :, dy, dx].T

    # per-channel vectors as columns of one [128, ncol] const
    cols, order = {}, []

    def addcol(name, vec):
        v = np.asarray(vec, np.float32).reshape(-1, 128)
        cols[name] = v
        order.append(name)

    addcol('eu', eu)
    addcol('ln1w', w['ln1_w'])
    addcol('ln1b', w['ln1_b'])
    addcol('knw', w['kn_w'])
    addcol('knb', w['kn_b'])
    addcol('ln2w', w['ln2_w'])
    addcol('ln2b', w['ln2_b'])
    addcol('g3p', g3p)
    addcol('b3p', b3p)
    addcol('spb', w['sp_b'])
    addcol('c3b', c3b_eff)
    addcol('c1b', w['c1_b'])
    addcol('c2b', c2b_eff)
    addcol('upb', upb_p)
    colidx, ncol = {}, 0
    for n in order:
        colidx[n] = ncol
        ncol += cols[n].shape[0]
    cvec_np = np.zeros((128, ncol), np.float32)
    for n in order:
        for i in range(cols[n].shape[0]):
            cvec_np[:, colidx[n] + i] = cols[n][i]

    ln1_triv = np.all(w['ln1_w'] == 1.0) and np.all(w['ln1_b'] == 0.0)
    kn_triv = np.all(w['kn_w'] == 1.0) and np.all(w['kn_b'] == 0.0)
    ln2_triv = np.all(w['ln2_w'] == 1.0) and np.all(w['ln2_b'] == 0.0)
    bn3_triv = np.all(g3p == g3p[0]) and np.all(b3p == 0.0)
    # uniform bn3 scale commutes with LN2 -> drop it entirely when trivial

    # ---------------- bass module
    nc = bacc.Bacc("TRN2", target_bir_lowering=False, debug=False, name="decblk")

    xin = nc.dram_tensor("xin", [C, S], F32, kind="ExternalInput")
    skin = nc.dram_tensor("skin", [CS, S], F32, kind="ExternalInput")
    mrow = nc.dram_tensor("mrow", [1, S], F32, kind="ExternalInput")
    yout = nc.dram_tensor("yout", [COUT, 2 * H, 2 * W], BF16, kind="ExternalOutput")
    probes = {}

    def mkprobe(name, shape):
        if probe:
            probes[name] = nc.dram_tensor(name, shape, F32, kind="ExternalOutput")
        return probes.get(name)

    def it(arr, name):
        arr = np.ascontiguousarray(arr)
        import ml_dtypes as _md
        dt_ = {np.dtype(np.float32): F32, np.dtype(_md.bfloat16): BF16}[arr.dtype]
        const_inputs[name] = arr
        return nc.dram_tensor(name, list(arr.shape), dt_, kind="ExternalInput")
    d_lam = it(lam64, "lam64")
    d_cvec = it(cvec_np, "cvec")
    d_wk = [it(bf(wk_x), "wkx"), it(bf(wk_d), "wkd")]
    d_wv = [it(bf(wv_x), "wvx"), it(bf(wv_d), "wvd")]
    d_wr = [it(bf(wr_x), "wrx"), it(bf(wr_d), "wrd")]
    d_wo = it(bf(wo_t), "wo")
    d_sp = it(sp_t, "sp")
    d_upb = it(up_tb, "up")
    c1_r = c1_l.reshape(9, 2, 2, 128, 2, 128).transpose(1, 4, 3, 0, 2, 5)
    d_c1 = it(bf(c1_r), "c1")   # [g, mt, p, t, kt, m]
    d_c2 = it(bf(c2_eff.T), "c2")
    d_c3 = it(bf(c3_eff.T), "c3")
    red_np = np.zeros((128, 2, 2), np.float32)
    red_np[:, 0, 0] = 1.0
    red_np[:, 1, 1] = 1.0
    d_red = it(red_np, "red")
    d_redb = it(bf(red_np), "redb")
    bc8_np = np.zeros((8, 8, 128), np.float32)
    for _v in range(8):
        bc8_np[_v, _v, :] = 1.0
    d_bc8 = it(bc8_np, "bc8")
    d_eps = it(np.full((128, 1), EPS, np.float32), "epsc")
    d_i2 = it(np.eye(2, dtype=np.float32), "i2c")
    d_i128 = it(np.eye(128, dtype=np.float32), "i128c")

    def scan_raw(out, d0, d1, eng=None):
        eng = eng or nc.vector
        if os.environ.get('BASSK_NOSCAN'):
            return eng.tensor_copy(out=out, in_=d1)
        return eng.add_instruction(mybir.InstTensorScalarPtr(
            name=nc.get_next_instruction_name(),
            is_tensor_tensor_scan=True,
            is_scalar_tensor_tensor=True,
            op0=MM, op1=AD,
            ins=[eng.lower_ap(d0), eng.lower_ap_or_imm(0.0), eng.lower_ap(d1)],
            outs=[eng.lower_ap(out)],
        ))

    def recip(out, in_):
        if os.environ.get('BASSK_SLOWRECIP'):
            return nc.vector.reciprocal(out=out, in_=in_)
        return nc.vector.reciprocal_approx_fast(out=out, in_=in_)

    def view(ap, dims, off=0):
        return bass.AP(tensor=ap.tensor, offset=ap.offset + off, ap=dims)

    with tile.TileContext(nc) as tc:
        with tc.tile_pool(name="big", bufs=1) as big, \
             tc.tile_pool(name="wres", bufs=1) as wres, \
             tc.tile_pool(name="scr", bufs=4) as scr, \
             tc.tile_pool(name="sml", bufs=3) as sml, \
             tc.tile_pool(name="y2b", bufs=8) as y2b, \
             tc.tile_pool(name="yupp", bufs=2) as yupp, \
             tc.tile_pool(name="xcs", bufs=2) as xcs, \
             tc.tile_pool(name="wstr", bufs=2) as wstr, \
             tc.tile_pool(name="dsc", bufs=2, space="DRAM") as dsc, \
             tc.tile_pool(name="psmm", bufs=3, space="PSUM") as psmm, \
             tc.tile_pool(name="psst", bufs=1, space="PSUM") as psst, \
             tc.tile_pool(name="psln", bufs=1, space="PSUM") as psln, \
             tc.tile_pool(name="psbc", bufs=1, space="PSUM") as psbc:

            # ---- resident constants
            lt = wres.tile([128, 2, 64], F32, name="lt")
            nc.sync.dma_start(out=lt, in_=d_lam[:, :].rearrange("(t p) j -> p t j", p=128))
            cv = wres.tile([128, ncol], F32, name="cv")
            nc.sync.dma_start(out=cv, in_=d_cvec[:, :])
            red = wres.tile([128, 2, 2], F32R, name="red")
            nc.sync.dma_start(out=red, in_=d_red[:, :, :].bitcast(F32R))
            redb = wres.tile([128, 2, 2], BF16, name="redb")
            nc.sync.dma_start(out=redb, in_=d_redb[:, :, :])
            bc8 = wres.tile([8, 8, 128], F32R, name="bc8")
            nc.sync.dma_start(out=bc8, in_=d_bc8[:, :, :].bitcast(F32R))
            epsc = wres.tile([128, 1], F32, name="epsc")
            nc.sync.dma_start(out=epsc, in_=d_eps[:, :])
            i2c = wres.tile([2, 2], F32R, name="i2c")
            nc.sync.dma_start(out=i2c, in_=d_i2[:, :].bitcast(F32R))
            i128c = wres.tile([128, 128], F32R, name="i128c")
            nc.sync.dma_start(out=i128c, in_=d_i128[:, :].bitcast(F32R))
            c2wt = wres.tile([128, 4, 1024], BF16, name="c2wt")
            nc.sync.dma_start(out=c2wt, in_=d_c2[:, :].rearrange("(kt p) m -> p kt m", p=128))

            def col(name, i=0):
                return cv[:, colidx[name] + i:colidx[name] + i + 1]

            # ============ LayerNorm over channels (2 ctiles) ============
            # On-chip stats path: ones-matmul partition reduction -> PE
            # transpose to pixel-major [128, 32, 2] -> tiny stat math ->
            # PE transpose back to [2, S] -> K=2 broadcast matmuls.
            def ln256(Xr, out_wr, name, wb=None, bf16_in=False):
                redm = redb if bf16_in else red
                pst = psln.tile([128, 32, 2], F32, name=f"pst_{name}", tag="pst")
                for ch in range(NCH):
                    sl = slice(ch * CH, (ch + 1) * CH)
                    ps = psst.tile([2, CH], F32, name=f"lnps_{name}", tag="st")
                    for ct in range(2):
                        nc.tensor.matmul(out=ps, lhsT=redm[:, 0, :], rhs=Xr(ct)[:, sl],
                                         start=(ct == 0), stop=False)
                    for ct in range(2):
                        sq = sml.tile([128, CH], BF16,
                                      name=f"sq_{name}", tag="sqc", bufs=3)
                        nc.scalar.activation(
                            out=sq,
                            in_=Xr(ct)[:, sl] if bf16_in else Xr(ct)[:, sl].bitcast(F32),
                            func=Act.Square)
                        nc.tensor.matmul(out=ps, lhsT=redb[:, 1, :], rhs=sq,
                                         start=False, stop=(ct == 1))
                    stc = sml.tile([2, CH], F32R, name=f"stc_{name}", tag="stc", bufs=1)
                    nc.scalar.copy(out=stc, in_=ps)
                    for j in range(4):
                        nc.tensor.matmul(
                            out=pst[:, ch * 4 + j, :],
                            lhsT=stc[:, j * 128:(j + 1) * 128],
                            rhs=i2c[:, :])
                # small stat math in [128, 32, 2] pixel-major layout
                smb = sml.tile([128, 32, 2], F32, name=f"smb_{name}", tag="sm", bufs=1)
                nc.vector.tensor_scalar_mul(out=smb, in0=pst, scalar1=1.0 / C)
                t2 = sml.tile([128, 32], F32, name=f"t2_{name}", tag="t2", bufs=1)
                nc.vector.tensor_tensor(out=t2, in0=smb[:, :, 0], in1=smb[:, :, 0], op=MM)
                nc.vector.tensor_tensor(out=t2, in0=smb[:, :, 1], in1=t2, op=SU)
                nc.scalar.activation(out=t2, in_=t2, func=Act.Sqrt, bias=epsc)
                # stats scattered into one-hot columns (col = 2*sub + q) so
                # the PE transpose lands rows 0-7 partition-aligned; [8, S/4]
                # bcr is 4x narrower in SBUF than a [2, S] row pair.
                sm8 = sml.tile([128, 32, 8], F32R, name=f"sm8_{name}", tag="sm2", bufs=1)
                s8v = sm8.rearrange("p a b -> p (a b)")
                with nc.allow_low_precision(reason="zeroing f32r one-hot scatter"):
                    nc.vector.tensor_scalar_mul(
                        out=s8v, in0=view(epsc[:, :], [epsc[:, :].ap[0], [0, 256]]),
                        scalar1=0.0)

                def s8view(q):
                    return view(s8v[:, :], [s8v.ap[0], [66, 4], [8, 8]], off=q)

                with nc.allow_low_precision(reason="rstd in f32r for PE transpose"):
                    nc.vector.reciprocal(out=s8view(0), in_=t2)            # rstd
                nc.vector.tensor_tensor(out=s8view(1), in0=smb[:, :, 0],
                                        in1=s8view(0).bitcast(F32), op=MM)  # mu*rstd
                bcr = big.tile([8, S // 4], F32R, name=f"bcr_{name}", tag="rowsbig")
                for hoff in range(2):
                    psb8 = psst.tile([8, CH], F32, name=f"psb_{name}", tag="st")
                    for j in range(4):
                        for sub in range(4):
                            nc.tensor.matmul(
                                out=psb8[:, j * 128:(j + 1) * 128],
                                lhsT=sm8[:, (sub * 2 + hoff) * 4 + j, :],
                                rhs=i128c[:, :],
                                start=(sub == 0), stop=(sub == 3))
                    nc.scalar.copy(out=bcr[:, hoff * CH:(hoff + 1) * CH], in_=psb8)
                for ch in range(NCH):
                    sl = slice(ch * CH, (ch + 1) * CH)
                    sub, hoff = ch // 2, ch % 2
                    pr = psbc.tile([128, CH], F32, name=f"pr_{name}", tag="pr")
                    pm = psbc.tile([128, CH], F32, name=f"pm_{name}", tag="pm")
                    nc.tensor.matmul(out=pr, lhsT=bc8[:, 2 * sub + 0, :],
                                     rhs=bcr[:, hoff * CH:(hoff + 1) * CH])
                    nc.tensor.matmul(out=pm, lhsT=bc8[:, 2 * sub + 1, :],
                                     rhs=bcr[:, hoff * CH:(hoff + 1) * CH])
                    for ct in range(2):
                        out_wr(ct, sl, pr, pm)
                if wb is not None:
                    wn, bn_, apfn = wb
                    for ct in range(2):
                        ap = apfn(ct)
                        nc.vector.tensor_scalar(
                            out=ap, in0=ap, scalar1=col(wn, ct), scalar2=col(bn_, ct),
                            op0=MM, op1=AD)

            # ============ S0/S1: load x, LN1 -> xn (bf16) ============
            x0 = big.tile([128, S], F32R, name="x0", tag="A")
            x1 = big.tile([128, S], F32R, name="x1", tag="B")
            nc.sync.dma_start(out=x0, in_=xin[0:128, :].bitcast(F32R))
            nc.sync.dma_start(out=x1, in_=xin[128:256, :].bitcast(F32R))
            mf = big.tile([128, S], F32, name="mf", tag="D")
            nc.sync.dma_start(out=mf, in_=view(mrow[:, :], [[0, 128], [1, S]]))

            xn = big.tile([128, 2, S], BF16, name="xn", tag="Cxn")
            xt = [x0, x1]

            def ln1_wr(ct, sl, pr, pm):
                nc.vector.tensor_tensor(out=xn[:, ct, sl], in0=xt[ct][:, sl].bitcast(F32),
                                        in1=pr, op=MM)
                nc.vector.tensor_tensor(out=xn[:, ct, sl], in0=xn[:, ct, sl],
                                        in1=pm, op=SU)

            ln256(lambda ct: xt[ct][:, :], ln1_wr, "ln1",
                  wb=None if ln1_triv else ("ln1w", "ln1b", lambda ct: xn[:, ct, :]))
            if probe:
                pxn = mkprobe("p_xn", [C, S])
                for ct in range(2):
                    nc.gpsimd.dma_start(out=pxn[128 * ct:128 * (ct + 1), :],
                                        in_=xn[:, ct, :])

            # ============ S2: q_shift diff * mask -> md (bf16) ============
            xn4 = xn.rearrange("p t (h w) -> p t h w", h=H)
            md = big.tile([128, 2, H, W], BF16, name="md", tag="Emd")
            nc.vector.tensor_tensor(out=md[0:64, 0, :, 1:], in0=xn4[0:64, 0, :, 0:63],
                                    in1=xn4[0:64, 0, :, 1:], op=SU)
            nc.vector.tensor_scalar_mul(out=md[0:64, 0, :, 0:1],
                                        in0=xn4[0:64, 0, :, 0:1], scalar1=-1.0)
            nc.vector.tensor_tensor(out=md[64:128, 0, :, 0:63], in0=xn4[64:128, 0, :, 1:],
                                    in1=xn4[64:128, 0, :, 0:63], op=SU)
            nc.vector.tensor_scalar_mul(out=md[64:128, 0, :, 63:64],
                                        in0=xn4[64:128, 0, :, 63:64], scalar1=-1.0)
            nc.gpsimd.tensor_tensor(out=md[0:64, 1, 1:, :], in0=xn4[0:64, 1, 0:63, :],
                                    in1=xn4[0:64, 1, 1:, :], op=SU)
            nc.gpsimd.tensor_scalar_mul(out=md[0:64, 1, 0:1, :],
                                        in0=xn4[0:64, 1, 0:1, :], scalar1=-1.0)
            nc.gpsimd.tensor_tensor(out=md[64:128, 1, 0:63, :], in0=xn4[64:128, 1, 1:, :],
                                    in1=xn4[64:128, 1, 0:63, :], op=SU)
            nc.gpsimd.tensor_scalar_mul(out=md[64:128, 1, 63:64, :],
                                        in0=xn4[64:128, 1, 63:64, :], scalar1=-1.0)
            mdf = md.rearrange("p t h w -> p t (h w)")
            nc.vector.tensor_tensor(out=mdf[:, 0, :], in0=mdf[:, 0, :],
                                    in1=mf, op=MM)
            nc.gpsimd.tensor_tensor(out=mdf[:, 1, :], in0=mdf[:, 1, :],
                                    in1=mf, op=MM)

            # ============ S3: k/v/r matmuls -> E, V, SR; scans ============
            ev = big.tile([128, 2, S], BF16, name="ev", tag="B")
            et = big.tile([128, 2, S], BF16, name="et", tag="A")
            vv = big.tile([128, 2, S], BF16, name="vv", tag="D")
            sr = big.tile([128, 2, S], BF16, name="sr", tag="Fsr")

            def kvloop(dws, evac):
                wxt = wstr.tile([128, 2, 256], BF16, name="wxt", tag="wst", bufs=2)
                wdt = wstr.tile([128, 2, 256], BF16, name="wdt", tag="wst", bufs=2)
                nc.sync.dma_start(out=wxt, in_=dws[0][:, :].rearrange("(kt p) m -> p kt m", p=128))
                nc.sync.dma_start(out=wdt, in_=dws[1][:, :].rearrange("(kt p) m -> p kt m", p=128))
                for mt in range(2):
                    for ch in range(NCH):
                        sl = slice(ch * CH, (ch + 1) * CH)
                        ps = psmm.tile([128, CH], F32, name="kv_ps", tag="mm")
                        for kt in range(2):
                            nc.tensor.matmul(out=ps, lhsT=wxt[:, kt, 128 * mt:128 * (mt + 1)],
                                             rhs=xn[:, kt, sl], start=(kt == 0), stop=False)
                        for kt in range(2):
                            nc.tensor.matmul(out=ps, lhsT=wdt[:, kt, 128 * mt:128 * (mt + 1)],
                                             rhs=mdf[:, kt, sl], start=False, stop=(kt == 1))
                        evac(mt, sl, ps)

            kvloop(d_wk, lambda mt, sl, ps: nc.scalar.activation(
                out=et[:, mt, sl], in_=ps, func=Act.Exp))
            kvloop(d_wv, lambda mt, sl, ps: nc.scalar.copy(out=vv[:, mt, sl], in_=ps))

            nc.vector.tensor_tensor(out=ev, in0=et, in1=vv, op=MM)

            ev4 = ev.rearrange("p t (h w) -> p t h w", h=H)
            et4 = et.rearrange("p t (h w) -> p t h w", h=H)

            kvloop(d_wr, lambda mt, sl, ps: nc.scalar.activation(
                out=sr[:, mt, sl], in_=ps, func=Act.Sigmoid))

            # prime the wst9 slots early so c1 weight DMAs can land during scans
            if not os.environ.get('BASSK_NOPRIME'):
                for i in range(2):
                    pr_ = wstr.tile([128, 1], BF16, name=f"prime{i}", tag="wst9")
                    nc.vector.tensor_copy(out=pr_, in_=xn[:, 0, 0:1])

            # ---- early prefetch + skip-feature GEMM (overlaps scan phase) ----
            wot = wstr.tile([128, 2, 256], BF16, name="wot", tag="wst", bufs=2)
            nc.sync.dma_start(out=wot, in_=d_wo[:, :].rearrange("(kt p) m -> p kt m", p=128))
            spt = wstr.tile([128, 4, 256], F32R, name="spt", tag="wst4", bufs=1)
            nc.sync.dma_start(out=spt,
                              in_=d_sp[:, :].rearrange("(kt p) m -> p kt m", p=128).bitcast(F32R))
            xcp23 = [xcs.tile([128, 66, 66], BF16, name=f"xcs{i}", tag="xcs")
                     for i in range(2)]
            for t in xcp23:
                nc.vector.memset(t[:, 0:1, :], 0.0)
                nc.vector.memset(t[:, 65:66, :], 0.0)
                nc.vector.memset(t[:, 1:65, 0:1], 0.0)
                nc.vector.memset(t[:, 1:65, 65:66], 0.0)
            CQ = CH // 2
            for ch in range(2 * NCH):
                h0 = ch * 4
                sl = slice(ch * CQ, (ch + 1) * CQ)
                skc = sml.tile([128, 4, CQ], F32R, name="skc", tag="skcf", bufs=1)
                nc.sync.dma_start(
                    out=skc,
                    in_=skin[:, sl].rearrange("(kt p) n -> p kt n", p=128).bitcast(F32R))
                for mt in range(2):
                    ps = psmm.tile([128, CQ], F32, name="sp_ps", tag="mm")
                    for kt in range(4):
                        nc.tensor.matmul(out=ps, lhsT=spt[:, kt, 128 * mt:128 * (mt + 1)],
                                         rhs=skc[:, kt, :], start=(kt == 0), stop=(kt == 3))
                    nc.scalar.activation(
                        out=xcp23[mt][:, 1 + h0:5 + h0, 1:65],
                        in_=ps.rearrange("p (a b) -> p a b", a=4),
                        func=Act.Identity, bias=col('spb', mt))

            # ---- c1 group 1 (skip-feature half) early: its inputs are ready
            # before the scans, so the 3x3-conv matmuls fill the PE during the
            # vector-only scan phase; outputs bounce through DRAM until c2.
            dy1 = [dsc.tile([128, S], BF16, name=f"dy1_{mt}", tag=f"dy1{mt}")
                   for mt in range(2)]
            for mt in range(2):
                c1gm = wstr.tile([128, 9, 2, 128], BF16, name="c1gm", tag="wst9")
                nc.sync.dma_start(out=c1gm, in_=d_c1[1, mt, :, :, :, :])
                for ch in range(NCH):
                    h0 = ch * 8
                    ps = psmm.tile([128, CH], F32, name="c1_ps", tag="mm")
                    i = 0
                    for ti in range(9):
                        dy, dx = ti // 3 - 1, ti % 3 - 1
                        for kt in range(2):
                            nc.tensor.matmul(
                                out=ps.rearrange("p (a b) -> p a b", a=8),
                                lhsT=c1gm[:, ti, kt, :],
                                rhs=xcp23[kt][:, 1 + h0 + dy:9 + h0 + dy,
                                              1 + dx:65 + dx],
                                start=(i == 0), stop=(i == 17))
                            i += 1
                    yst = y2b.tile([128, CH], BF16, name="y1g1", tag="y2t")
                    nc.scalar.activation(out=yst, in_=ps, func=Act.Gelu,
                                         bias=col('c1b', 2 + mt))
                    nc.sync.dma_start(out=dy1[mt][:, ch * CH:(ch + 1) * CH], in_=yst)

            # ---- WKV scans: zero-padded 66-wide scan outputs keep every
            # consumer read packed + 4B-aligned (2x DVE mode); vertical
            # orientation runs on the transposed copies so it is packed too.
            lt_ap = lt[:, :, :]

            def lamview(ct, nseq):
                return view(lt_ap, [lt_ap.ap[0], [0, nseq], [1, 64]], off=ct * 64)

            outv = big.tile([128, 2, W, H], BF16, name="outv", tag="D")

            # vertical orientation first (reads pristine ev/et via transposed
            # APs; zero-padded 66-wide scan outputs avoid all edge ops)
            for half in range(2):
                wr_ = slice(half * 32, (half + 1) * 32)
                a = scr.tile([128, 2, 32, 66], BF16, name="av", tag="scrt")
                b = scr.tile([128, 2, 32, 66], BF16, name="bv", tag="scrt")
                nc.vector.memset(a[:, :, :, 0:1], 0.0)
                nc.vector.memset(b[:, :, :, 0:1], 0.0)
                for ct in range(2):
                    base = ct * S + half * 32
                    dv_ev = view(ev[:, :, :], [ev.ap[0], [1, 32], [64, 64]], off=base)
                    dv_et = view(et[:, :, :], [et.ap[0], [1, 32], [64, 64]], off=base)
                    scan_raw(a[:, ct, :, 1:65], lamview(ct, 32), dv_ev)
                    scan_raw(b[:, ct, :, 1:65], lamview(ct, 32), dv_et)
                for ct in range(2):
                    base = ct * S + half * 32
                    dv_ev = view(ev[:, :, :], [ev.ap[0], [1, 32], [64, 64]], off=base)
                    dv_et = view(et[:, :, :], [et.ap[0], [1, 32], [64, 64]], off=base)
                    den = scr.tile([128, 32, 64], F32, name="den", tag="scrt")
                    nc.vector.scalar_tensor_tensor(
                        out=den, in0=dv_et, scalar=col('eu', ct),
                        in1=b[:, ct, :, 0:64], op0=MM, op1=AD)
                    recip(out=den, in_=den)
                    nc.vector.scalar_tensor_tensor(
                        out=outv[:, ct, wr_, :], in0=dv_ev,
                        scalar=col('eu', ct), in1=a[:, ct, :, 0:64], op0=MM, op1=AD)
                    nc.vector.tensor_tensor(out=outv[:, ct, wr_, :],
                                            in0=outv[:, ct, wr_, :], in1=den, op=MM)

            # horizontal orientation; num/out in place on ev, then fold in
            # the transposed vertical output per half so kn stats start early
            for half in range(2):
                hr = slice(half * 32, (half + 1) * 32)
                a = scr.tile([128, 2, 32, 66], BF16, name="ah", tag="scrt")
                b = scr.tile([128, 2, 32, 66], BF16, name="bh", tag="scrt")
                nc.vector.memset(a[:, :, :, 0:1], 0.0)
                nc.vector.memset(b[:, :, :, 0:1], 0.0)
                for ct in range(2):
                    scan_raw(a[:, ct, :, 1:65], lamview(ct, 32), ev4[:, ct, hr, :])
                    scan_raw(b[:, ct, :, 1:65], lamview(ct, 32), et4[:, ct, hr, :])
                for ct in range(2):
                    den = scr.tile([128, 32, 64], F32, name="den2", tag="scrt")
                    nc.vector.scalar_tensor_tensor(
                        out=den, in0=et4[:, ct, hr, :], scalar=col('eu', ct),
                        in1=b[:, ct, :, 0:64], op0=MM, op1=AD)
                    recip(out=den, in_=den)
                    nc.vector.scalar_tensor_tensor(
                        out=ev4[:, ct, hr, :], in0=ev4[:, ct, hr, :],
                        scalar=col('eu', ct), in1=a[:, ct, :, 0:64], op0=MM, op1=AD)
                    nc.vector.tensor_tensor(out=ev4[:, ct, hr, :],
                                            in0=ev4[:, ct, hr, :], in1=den, op=MM)
                for ct in range(2):
                    ovT = view(outv[:, :, :, :], [outv.ap[0], [1, 32], [64, 64]],
                               off=ct * S + half * 32)
                    nc.vector.tensor_tensor(out=ev4[:, ct, hr, :], in0=ev4[:, ct, hr, :],
                                            in1=ovT, op=AD)
            if probe:
                pwkv = mkprobe("p_wkv", [C, S])
                for ct in range(2):
                    nc.gpsimd.dma_start(out=pwkv[128 * ct:128 * (ct + 1), :],
                                        in_=ev[:, ct, :])

            # ============ S4: key-LN, srw, Wo+residual, skip feat ============
            def kn_wr(ct, sl, pr, pm):
                nc.vector.tensor_tensor(out=ev[:, ct, sl], in0=ev[:, ct, sl], in1=pr, op=MM)
                nc.vector.tensor_tensor(out=ev[:, ct, sl], in0=ev[:, ct, sl], in1=pm, op=SU)

            ln256(lambda ct: ev[:, ct, :], kn_wr, "kn", bf16_in=True,
                  wb=None if kn_triv else ("knw", "knb", lambda ct: ev[:, ct, :]))

            nc.vector.tensor_tensor(out=sr, in0=sr, in1=ev, op=MM)   # srw

            xcp = [scr.tile([128, 66, 66], BF16, name=f"xcp{i}", tag="scrt")
                   for i in range(2)] + xcp23
            for t in xcp[:2]:
                nc.vector.memset(t[:, 0:1, :], 0.0)
                nc.vector.memset(t[:, 65:66, :], 0.0)
                nc.vector.memset(t[:, 1:65, 0:1], 0.0)
                nc.vector.memset(t[:, 1:65, 65:66], 0.0)

            for mt in range(2):
                for ch in range(NCH):
                    sl = slice(ch * CH, (ch + 1) * CH)
                    h0 = ch * 8
                    ps = psmm.tile([128, CH], F32, name="wo_ps", tag="mm")
                    for kt in range(2):
                        nc.tensor.matmul(out=ps, lhsT=wot[:, kt, 128 * mt:128 * (mt + 1)],
                                         rhs=sr[:, kt, sl], start=(kt == 0), stop=(kt == 1))
                    nc.vector.tensor_tensor(
                        out=xcp[mt][:, 1 + h0:9 + h0, 1:65],
                        in0=xn4[:, mt, h0:h0 + 8, :],
                        in1=ps.rearrange("p (a b) -> p a b", a=8), op=AD)

            if probe:
                pxc = mkprobe("p_xcat", [CS, S])
                for i in range(4):
                    nc.gpsimd.dma_start(
                        out=pxc[128 * i:128 * (i + 1), :].rearrange("p (a b) -> p a b", a=64),
                        in_=xcp[i][:, 1:65, 1:65])

            # ============ S5: grouped 3x3 conv -> gelu -> y1 (bf16) ============
            y1a = big.tile([128, 2, S], BF16, name="y1a", tag="A")
            if probe and os.environ.get('BASSK_CANARY'):
                cnry = wstr.tile([128, 9, 2, 128], BF16, name="cnry", tag="wst9")
                nc.sync.dma_start(out=cnry, in_=d_c1[1, 0, :, :, :, :])
                marks = [("m0", cnry[:, 0, 0, 0:64]),
                         ("m1", xn[:, 0, 0:64]),
                         ("m2", ev[:, 0, 0:64]),
                         ("m3", sr[:, 0, 0:64])]
                for mi, (mn, mark) in enumerate(marks):
                    stg_c = sml.tile([128, 64], BF16, name=f"cst{mi}",
                                     tag="cst", bufs=4)
                    nc.vector.tensor_tensor(
                        out=stg_c, in0=cnry[:, 0, 0, 0:64],
                        in1=mark, op=Alu.bypass)
                    pc = mkprobe(f"p_cn{mi}", [128, 64])
                    nc.gpsimd.dma_start(out=pc[:, :], in_=stg_c)
            for mt in range(2):
                c1gm = wstr.tile([128, 9, 2, 128], BF16, name="c1gm", tag="wst9")
                nc.sync.dma_start(out=c1gm, in_=d_c1[0, mt, :, :, :, :])
                for ch in range(NCH):
                    h0 = ch * 8
                    ps = psmm.tile([128, CH], F32, name="c1_ps", tag="mm")
                    i = 0
                    for ti in range(9):
                        dy, dx = ti // 3 - 1, ti % 3 - 1
                        for kt in range(2):
                            nc.tensor.matmul(
                                out=ps.rearrange("p (a b) -> p a b", a=8),
                                lhsT=c1gm[:, ti, kt, :],
                                rhs=xcp[kt][:, 1 + h0 + dy:9 + h0 + dy,
                                            1 + dx:65 + dx],
                                start=(i == 0), stop=(i == 17))
                            i += 1
                    nc.scalar.activation(
                        out=y1a[:, mt, ch * CH:(ch + 1) * CH], in_=ps,
                        func=Act.Gelu, bias=col('c1b', mt))

            if probe:
                py1 = mkprobe("p_y1", [CS, S])
                for i in range(2):
                    nc.gpsimd.dma_start(out=py1[128 * i:128 * (i + 1), :],
                                        in_=y1a[:, i, :])
                for mt in range(2):
                    nc.gpsimd.dma_start(out=py1[128 * (2 + mt):128 * (3 + mt), :],
                                        in_=dy1[mt][:, :])

            # ============ S6: c2 -> gelu -> c3 -> gelu(+bn3) -> y3 ============
            c3wt = wstr.tile([128, 8, 256], BF16, name="c3wt", tag="wst4", bufs=1)
            nc.sync.dma_start(out=c3wt, in_=d_c3[:, :].rearrange("(kt p) m -> p kt m", p=128))
            y3 = [big.tile([128, S], BF16, name="y3_0", tag="Emd"),
                  big.tile([128, S], BF16, name="y3_1", tag="D")]
            for ch in range(NCH):
                sl = slice(ch * CH, (ch + 1) * CH)
                rls = []
                for j in range(2):
                    rl = sml.tile([128, CH], BF16, name="y1r", tag="y1r", bufs=2)
                    nc.sync.dma_start(out=rl, in_=dy1[j][:, sl])
                    rls.append(rl)
                ytiles = []
                for mt in range(8):
                    ps = psmm.tile([128, CH], F32, name="c2_ps", tag="mm")
                    for kt in range(4):
                        nc.tensor.matmul(out=ps, lhsT=c2wt[:, kt, 128 * mt:128 * (mt + 1)],
                                         rhs=y1a[:, kt, sl] if kt < 2 else rls[kt - 2],
                                         start=(kt == 0), stop=(kt == 3))
                    yt = y2b.tile([128, CH], BF16, name="y2t", tag="y2t")
                    nc.scalar.activation(out=yt, in_=ps, func=Act.Gelu, bias=col('c2b', mt))
                    ytiles.append(yt)
                for mt in range(2):
                    ps = psmm.tile([128, CH], F32, name="c3_ps", tag="mm")
                    for kt in range(8):
                        nc.tensor.matmul(out=ps, lhsT=c3wt[:, kt, 128 * mt:128 * (mt + 1)],
                                         rhs=ytiles[kt], start=(kt == 0), stop=(kt == 7))
                    nc.scalar.activation(out=y3[mt][:, sl], in_=ps, func=Act.Gelu,
                                         bias=col('c3b', mt))
                    if not bn3_triv:
                        nc.vector.tensor_scalar(out=y3[mt][:, sl],
                                                in0=y3[mt][:, sl],
                                                scalar1=col('g3p', mt),
                                                scalar2=col('b3p', mt), op0=MM, op1=AD)

            if probe:
                py3 = mkprobe("p_y3", [C, S])
                for i in range(2):
                    stg3 = sml.tile([128, S], F32, name=f"stg3_{i}", tag="stg3")
                    nc.vector.tensor_copy(out=stg3, in_=y3[i][:, :])
                    nc.gpsimd.dma_start(out=py3[128 * i:128 * (i + 1), :], in_=stg3)

            # ============ S7: LN2, up-proj, pixel-shuffle DMA out ============
            def ln2_wr(ct, sl, pr, pm):
                nc.vector.tensor_tensor(out=y3[ct][:, sl], in0=y3[ct][:, sl],
                                        in1=pr, op=MM)
                nc.vector.tensor_tensor(out=y3[ct][:, sl], in0=y3[ct][:, sl],
                                        in1=pm, op=SU)

            ln256(lambda ct: y3[ct][:, :], ln2_wr, "ln2", bf16_in=True,
                  wb=None if ln2_triv else ("ln2w", "ln2b", lambda ct: y3[ct][:, :]))

            upt = wstr.tile([128, 2, 512], BF16, name="upt", tag="wst4", bufs=1)
            nc.sync.dma_start(out=upt,
                              in_=d_upb[:, :].rearrange("(kt p) m -> p kt m", p=128))
            for r in range(2):
                for ch in range(NCH):
                    sl = slice(ch * CH, (ch + 1) * CH)
                    h0 = ch * 8
                    ub = yupp.tile([128, 8, 64, 2], BF16, name="ub", tag="ub")
                    for q in range(2):
                        rq = 2 * r + q
                        ps = psmm.tile([128, CH], F32, name="up_ps", tag="mm")
                        for kt in range(2):
                            nc.tensor.matmul(out=ps,
                                             lhsT=upt[:, kt, 128 * rq:128 * (rq + 1)],
                                             rhs=y3[kt][:, sl],
                                             start=(kt == 0), stop=(kt == 1))
                        nc.scalar.activation(out=ub[:, :, :, q],
                                             in_=ps.rearrange("p (a b) -> p a b", a=8),
                                             func=Act.Identity, bias=col('upb', rq))
                    dst = view(yout[:, :, :], [[128 * 128, 128], [256, 8], [1, 128]],
                               off=(2 * h0 + r) * 128)
                    nc.sync.dma_start(out=dst, in_=ub.rearrange("p a b q -> p a (b q)"))

    nc.compile()
    return nc, const_inputs


def _get_nc(weights, probe=False):
    import hashlib
    hsh = hashlib.sha1()
    for k in sorted(weights):
        hsh.update(k.encode())
        hsh.update(np.ascontiguousarray(weights[k]).tobytes())
    key = (hsh.hexdigest(), probe)
    if key not in _CACHE:
        _CACHE[key] = _build(weights, probe=probe)
    return _CACHE[key]


def kernel(**inputs):
    from concourse.bass_utils import run_bass_kernel_spmd

    x = np.asarray(inputs['x'], np.float32)
    skip = np.asarray(inputs['skip'], np.float32)
    mask = np.asarray(inputs['saliency_mask'], np.float32)
    weights = {k: np.asarray(v, np.float32) for k, v in inputs.items()
               if k not in ('x', 'skip', 'saliency_mask')}

    probe = bool(os.environ.get('BASSK_PROBE'))
    nc, const_inputs = _get_nc(weights, probe=probe)

    in_maps = []
    for b in range(B):
        m = dict(
            xin=np.ascontiguousarray(x[b].reshape(C, S)),
            skin=np.ascontiguousarray(skip[b].reshape(CS, S)),
            mrow=np.ascontiguousarray(mask[b].reshape(1, S)),
        )
        m.update(const_inputs)
        in_maps.append(m)
    res = run_bass_kernel_spmd(nc, in_maps, core_ids=list(range(B)),
                               trace=bool(os.environ.get('BASSK_TRACE')))
    kernel.last_results = res
    out = np.stack([np.asarray(res.results[b]['yout'], np.float32) for b in range(B)], axis=0)
    return out

